# revision 6
# baseline (speedup 1.0000x reference)
"""AdaptiveScaleRoutingMoE block on 8 TRN2 NeuronCores.

Strategy: data-parallel over batch (B=32 -> 4 per core). All weights
replicated. Per (scale, batch) pair one 512-token tile, 12 tiles/core.

Precision: router L1 in split-bf16 (hi/lo, 3 cross terms, exact gelu via the
Erf LUT), router L2 in f32 (top-2 selection is rank-sensitive), experts bf16
with f32 accumulation. The top-2-of-6 local-expert gather is replaced by a
mask built from the second max; the weighted combine runs token-major via
scalar_tensor_tensor with per-partition router-weight columns. Balance-loss
entropy uses ln(sum_exp) - sum(p*logit) (log-softmax identity), partial sums
reduced on host.
"""
import numpy as np
import ml_dtypes
from contextlib import ExitStack

import concourse.bass as bass
import concourse.tile as tile
from concourse import bacc, mybir
from concourse.bass_utils import run_bass_kernel_spmd

F32 = mybir.dt.float32
BF16 = mybir.dt.bfloat16
AF = mybir.ActivationFunctionType
ALU = mybir.AluOpType
AX = mybir.AxisListType

S, B, C, E = 3, 32, 512, 256
H, O, NE, NS, NL = 512, 256, 8, 2, 6
NCORES = 8
BC = B // NCORES          # batches per core
NT = S * BC               # token tiles per core (one per (s,b)), each T tokens
T = C                     # 512 tokens per tile
TCH = T // 128            # 4 token chunks
ECH = E // 128            # 2
HCH = H // 128            # 4
OCH = O // 128            # 2
KCC = 896 // 128          # 7 concat chunks (771 padded to 896)
RT2 = float(1.0 / np.sqrt(2.0))

_bf = ml_dtypes.bfloat16


def _to_bf(a):
    return np.ascontiguousarray(np.asarray(a, np.float32).astype(_bf))


def _feat_major(w):
    """[K, M] weight -> SBUF lhsT layout [128, K/128, M]."""
    k, m = w.shape
    return np.ascontiguousarray(
        np.asarray(w, np.float32).reshape(k // 128, 128, m).transpose(1, 0, 2))


def _bias_chunks(b):
    """[F] bias -> [128, F/128] per-partition layout."""
    f = b.shape[0]
    return np.ascontiguousarray(np.asarray(b, np.float32).reshape(f // 128, 128).T)


def build_graph(host):
    """host: dict of prepped numpy weight arrays. Returns compiled nc."""
    nc = bacc.Bacc("TRN2", target_bir_lowering=False, debug=False,
                   num_devices=NCORES)

    dram = {}

    def din(name, shape, dt):
        dram[name] = nc.dram_tensor(name, list(shape), dt, kind="ExternalInput")
        return dram[name]

    din("xhi", [NT, ECH, 128, T], BF16)
    din("xlo", [NT, ECH, 128, T], BF16)
    for k, v in host.items():
        din(k, v.shape, BF16 if v.dtype == _bf else F32)

    out_ext = nc.dram_tensor("out", [BC, C, O], F32, kind="ExternalOutput")
    ent_ext = nc.dram_tensor("ent", [1, 1], F32, kind="ExternalOutput")
    sw_bounce = nc.dram_tensor("sw_bounce", [BC * S], F32)
    tr_bounce = nc.dram_tensor("tr_bounce", [S, BC], F32)
    mean_bounce = nc.dram_tensor("mean_bounce", [NT, O], F32)

    have_rb2 = "w_rb2bc" in host
    have_bout = "w_boutbc" in host
    have_ob2 = "w_ob2bc" in host

    with tile.TileContext(nc) as tc, ExitStack() as ctx:
        nx = nc
        wp = ctx.enter_context(tc.tile_pool(name="weights", bufs=1))
        xp = ctx.enter_context(tc.tile_pool(name="xstash", bufs=1))
        stash = ctx.enter_context(tc.tile_pool(name="stash", bufs=1))
        hwork = ctx.enter_context(tc.tile_pool(name="hwork", bufs=2))
        ewp = ctx.enter_context(tc.tile_pool(name="ewp", bufs=1))
        ghp = ctx.enter_context(tc.tile_pool(name="ghp", bufs=1))
        sm = ctx.enter_context(tc.tile_pool(name="small", bufs=3))
        acc = ctx.enter_context(tc.tile_pool(name="accp", bufs=2))
        pp512 = ctx.enter_context(tc.tile_pool(name="pp512", bufs=2, space="PSUM"))
        pp256 = ctx.enter_context(tc.tile_pool(name="pp256", bufs=4, space="PSUM"))
        pps = ctx.enter_context(tc.tile_pool(name="pps", bufs=2, space="PSUM"))

        def wt(name, dt=BF16):
            a = host[name]
            t_ = wp.tile(list(a.shape), dt, tag=name)
            nx.sync.dma_start(t_[:], dram[name][:])
            return t_

        w1hi = wt("w_w1hi"); w1lo = wt("w_w1lo")
        rw2 = wt("w_rw2", F32)
        rb1s = wt("w_rb1s", F32)          # router_b1 / sqrt(2), [128, HCH]
        aW1 = wt("w_aW1"); ab1 = wt("w_ab1", F32)
        gW2 = wt("w_gW2"); gb2 = wt("w_gb2", F32)
        aWo = wt("w_aWo")
        tew = wt("w_tew"); teb = wt("w_teb", F32)
        wcw = wt("w_wcw"); wcb = wt("w_wcb", F32)
        ow1 = wt("w_ow1"); ob1 = wt("w_ob1", F32)
        ow2 = wt("w_ow2")
        ident = wt("w_ident")
        rb2bc = wt("w_rb2bc", F32) if have_rb2 else None
        boutbc = wt("w_boutbc", F32) if have_bout else None
        ob2bc = wt("w_ob2bc", F32) if have_ob2 else None

        onesb = wp.tile([128, 1], BF16, tag="onesb")
        nx.vector.memset(onesb[:], 1.0)
        onesf = wp.tile([128, 1], F32, tag="onesf")
        nx.vector.memset(onesf[:], 1.0)

        xhi = xp.tile([128, ECH, NT, T], BF16, tag="xhi")
        for i in range(NT):
            for ec in range(ECH):
                nx.sync.dma_start(xhi[:, ec, i, :], dram["xhi"][i, ec])

        logit_st = stash.tile([128, TCH, NT, NE], F32, tag="logit")
        wst = stash.tile([128, TCH, NT, NE], F32, tag="wst")
        ent_st = stash.tile([128, TCH * NT], F32, tag="ent")
        pst = stash.tile([128, TCH, NT, O], BF16, tag="pst")
        mlin = stash.tile([1, NT, O], F32, tag="mlin")
        mfm = stash.tile([128, OCH, NT], BF16, tag="mfm")
        reprs = stash.tile([128, OCH, NT], BF16, tag="reprs")
        ccat = stash.tile([128, KCC, BC], BF16, tag="ccat")
        th_sb = stash.tile([S, BC], F32, tag="th")
        swb = stash.tile([128, BC * S], F32, tag="swb")

        # ---------------- Phase R: router L1 (Erf) + L2 (f32) --------------
        for i in range(NT):
            xlo_t = hwork.tile([128, ECH, T], BF16, tag="xlo")
            for ec in range(ECH):
                nx.sync.dma_start(xlo_t[:, ec, :], dram["xlo"][i, ec])
            h1 = hwork.tile([128, HCH, T], F32, tag="h1")
            for mc in range(HCH):
                ps = pp512.tile([128, T], F32, tag="ps512")
                terms = [(w1hi, xhi[:, ec, i, :]) for ec in range(ECH)]
                terms += [(w1lo, xhi[:, ec, i, :]) for ec in range(ECH)]
                terms += [(w1hi, xlo_t[:, ec, :]) for ec in range(ECH)]
                # order: (hi,xhi) ec0,ec1 | (lo,xhi) | (hi,xlo)
                n_terms = len(terms)
                for j, (lhs, r) in enumerate(terms):
                    ec = j % ECH
                    nx.tensor.matmul(
                        ps[:], lhs[:, ec, bass.ts(mc, 128)], r,
                        start=(j == 0), stop=(j == n_terms - 1))
                erf = hwork.tile([128, T], F32, tag="erf")
                # erf((z + b1)/sqrt(2)); z in psum; rb1s = b1/sqrt(2).
                nx.scalar.activation(erf[:], ps[:], AF.Erf,
                                     bias=rb1s[:, mc:mc + 1], scale=RT2)
                # h1 = (erf + 1) * z = 2*gelu(z)  (0.5 folded into rw2)
                nx.vector.scalar_tensor_tensor(
                    h1[:, mc, :], erf[:], 1.0, ps[:], op0=ALU.add, op1=ALU.mult)
            for tc_ in range(TCH):
                pl = pps.tile([128, NE], F32, tag="ppsmall")
                for kc in range(HCH):
                    nx.tensor.matmul(
                        pl[:], h1[:, kc, bass.ts(tc_, 128)], rw2[:, kc, :],
                        start=(kc == 0), stop=(kc == HCH - 1))
                if have_rb2:
                    nx.vector.tensor_tensor(pl[:], pl[:], rb2bc[:, :NE],
                                            op=ALU.add)
                nx.vector.tensor_copy(logit_st[:, tc_, i, :], pl[:])

        tc.no_sync_barrier()
        # ---------------- Phase W: softmax + entropy + top-2 ---------------
        for i in range(NT):
            for tc_ in range(TCH):
                lgt = logit_st[:, tc_, i, :]
                e8 = sm.tile([128, NE], F32, tag="e8")
                nx.scalar.activation(e8[:], lgt, AF.Exp)
                ssum = sm.tile([128, 1], F32, tag="ssum")
                nx.vector.reduce_sum(ssum[:], e8[:], axis=AX.X)
                rcp = sm.tile([128, 1], F32, tag="rcp")
                nx.vector.reciprocal(rcp[:], ssum[:])
                w8 = wst[:, tc_, i, :]
                nx.vector.tensor_scalar(w8, e8[:], rcp[:], None, op0=ALU.mult)
                lns = sm.tile([128, 1], F32, tag="lns")
                nx.scalar.activation(lns[:], ssum[:], AF.Ln)
                pl8 = sm.tile([128, NE], F32, tag="pl8")
                nx.vector.tensor_tensor(pl8[:], w8, lgt, op=ALU.mult)
                spl = sm.tile([128, 1], F32, tag="spl")
                nx.vector.reduce_sum(spl[:], pl8[:], axis=AX.X)
                nx.vector.tensor_tensor(
                    ent_st[:, i * TCH + tc_:i * TCH + tc_ + 1], lns[:], spl[:],
                    op=ALU.subtract)
                # top-2 of local expert weights
                wl = wst[:, tc_, i, NS:NE]
                m1 = sm.tile([128, 1], F32, tag="m1")
                nx.vector.reduce_max(m1[:], wl, axis=AX.X)
                eq = sm.tile([128, NL], F32, tag="eq")
                nx.vector.tensor_scalar(eq[:], wl, m1[:], None, op0=ALU.is_equal)
                w2 = sm.tile([128, NL], F32, tag="w2")
                nx.vector.scalar_tensor_tensor(
                    w2[:], eq[:], -1e30, wl, op0=ALU.mult, op1=ALU.add)
                m2 = sm.tile([128, 1], F32, tag="m2")
                nx.vector.reduce_max(m2[:], w2[:], axis=AX.X)
                sel = sm.tile([128, NL], F32, tag="sel")
                nx.vector.tensor_scalar(sel[:], wl, m2[:], None, op0=ALU.is_ge)
                nx.vector.tensor_tensor(wl, wl, sel[:], op=ALU.mult)

        tc.no_sync_barrier()
        # ---------------- Phase E: experts (Gelu) + combine + means --------
        for i in range(NT):
            eh = ewp.tile([128, HCH, NE, T], BF16, tag="eh")
            for n in range(NE):
                for mc in range(HCH):
                    ps = pp512.tile([128, T], F32, tag="ps512")
                    for ec in range(ECH):
                        nx.tensor.matmul(
                            ps[:], aW1[:, ec, n, bass.ts(mc, 128)],
                            xhi[:, ec, i, :],
                            start=(ec == 0), stop=(ec == ECH - 1))
                    nx.scalar.activation(eh[:, mc, n, :], ps[:], AF.Gelu,
                                         bias=ab1[:, n, mc:mc + 1])
            gh2 = ghp.tile([128, HCH, NS, T], BF16, tag="gh2")
            for n in range(NS):
                for mc in range(HCH):
                    ps = pp512.tile([128, T], F32, tag="ps512")
                    for kc in range(HCH):
                        nx.tensor.matmul(
                            ps[:], gW2[:, kc, n, bass.ts(mc, 128)],
                            eh[:, kc, n, :],
                            start=(kc == 0), stop=(kc == HCH - 1))
                    nx.scalar.activation(gh2[:, mc, n, :], ps[:], AF.Gelu,
                                         bias=gb2[:, n, mc:mc + 1])
            for tc_ in range(TCH):
                ac = acc.tile([128, O], F32, tag="acc")
                for n in range(NE):
                    src = gh2 if n < NS else eh
                    pn = pp256.tile([128, O], F32, tag="pexp")
                    for kc in range(HCH):
                        nx.tensor.matmul(
                            pn[:], src[:, kc, n, bass.ts(tc_, 128)],
                            aWo[:, kc, n, :],
                            start=(kc == 0), stop=(kc == HCH - 1))
                    if have_bout:
                        nx.vector.tensor_tensor(
                            pn[:], pn[:], boutbc[:, bass.ts(n, O)], op=ALU.add)
                    if n == 0:
                        nx.vector.tensor_scalar(
                            ac[:], pn[:], wst[:, tc_, i, 0:1], None,
                            op0=ALU.mult)
                    else:
                        nx.vector.scalar_tensor_tensor(
                            ac[:], pn[:], wst[:, tc_, i, n:n + 1], ac[:],
                            op0=ALU.mult, op1=ALU.add)
                nx.vector.tensor_copy(pst[:, tc_, i, :], ac[:])
            pm = pps.tile([1, O], F32, tag="ppsmall")
            for tc_ in range(TCH):
                nx.tensor.matmul(pm[:], onesb[:], pst[:, tc_, i, :],
                                 start=(tc_ == 0), stop=(tc_ == TCH - 1))
            nx.vector.tensor_copy(mlin[:, i, :], pm[:])
        # means -> feature-major [128, OCH, NT] via DRAM bounce
        nx.sync.dma_start(mean_bounce[:], mlin[0, :, :])
        mfm_f = acc.tile([128, OCH, NT], F32, tag="mfmf")
        for oc in range(OCH):
            nx.sync.dma_start(
                mfm_f[:, oc, :],
                mean_bounce[:, oc * 128:(oc + 1) * 128].transpose([1, 0]))
        nx.vector.tensor_copy(mfm[:], mfm_f[:])
        for mc in range(OCH):
            ps = pps.tile([128, NT], F32, tag="ppsmall")
            for kc in range(OCH):
                nx.tensor.matmul(ps[:], tew[:, kc, bass.ts(mc, 128)],
                                 mfm[:, kc, :],
                                 start=(kc == 0), stop=(kc == OCH - 1))
            nx.scalar.activation(reprs[:, mc, :], ps[:], AF.Gelu,
                                 bias=teb[:, mc:mc + 1])
        # concat [771 padded 896, BC]; rows s*256+o2 from reprs, 768+s from wm
        nx.vector.memset(ccat[:], 0.0)
        for s in range(S):
            for oc in range(OCH):
                nx.sync.dma_start(ccat[:, s * OCH + oc, :],
                                  reprs[:, oc, s * BC:(s + 1) * BC])
        nx.sync.dma_start(ccat[0:S, KCC - 1, :], dram["w_wmbc"][:])
        pw = pps.tile([S, BC], F32, tag="ppsmall")
        for kc in range(KCC):
            nx.tensor.matmul(pw[:], wcw[:, kc, :], ccat[:, kc, :],
                             start=(kc == 0), stop=(kc == KCC - 1))
        nx.scalar.activation(th_sb[:], pw[:], AF.Tanh, bias=wcb[0:S, 0:1])

        tc.no_sync_barrier()
        # ---------------- Phase G: softplus + scale softmax + entropy ------
        ee = sm.tile([S, BC], F32, tag="ee")
        nx.scalar.activation(ee[:], th_sb[:], AF.Exp)
        nx.vector.tensor_scalar(ee[:], ee[:], 1.0, None, op0=ALU.add)
        raw = sm.tile([S, BC], F32, tag="raw")
        nx.scalar.activation(raw[:], ee[:], AF.Ln)
        nx.sync.dma_start(tr_bounce[:], raw[:])
        rawt = sm.tile([BC, S], F32, tag="rawt")
        nx.sync.dma_start(rawt[:], tr_bounce.ap().transpose([1, 0]))
        ex = sm.tile([BC, S], F32, tag="ex")
        nx.scalar.activation(ex[:], rawt[:], AF.Exp)
        ssum2 = sm.tile([BC, 1], F32, tag="ssum2")
        nx.vector.reduce_sum(ssum2[:], ex[:], axis=AX.X)
        rcp2 = sm.tile([BC, 1], F32, tag="rcp2")
        nx.vector.reciprocal(rcp2[:], ssum2[:])
        swt = sm.tile([BC, S], F32, tag="swt")
        nx.vector.tensor_scalar(swt[:], ex[:], rcp2[:], None, op0=ALU.mult)
        nx.sync.dma_start(sw_bounce[:], swt[:])
        nx.sync.dma_start(swb[:], sw_bounce.ap().partition_broadcast(128))
        entv = sm.tile([128, 1], F32, tag="entv")
        nx.vector.reduce_sum(entv[:], ent_st[:], axis=AX.X)
        pe_ = pps.tile([1, 1], F32, tag="ppsmall")
        nx.tensor.matmul(pe_[:], onesf[:], entv[:], start=True, stop=True)
        esb = sm.tile([1, 1], F32, tag="esb")
        nx.vector.tensor_copy(esb[:], pe_[:])
        nx.sync.dma_start(ent_ext[:], esb[:])

        tc.no_sync_barrier()
        # ---------------- Phase OUT: scale-weighted sum + output MLP -------
        for b in range(BC):
            wacc = acc.tile([128, TCH, O], BF16, tag="wacc")
            for tc_ in range(TCH):
                tmp = sm.tile([128, O], F32, tag="wtmp")
                nx.vector.tensor_scalar(
                    tmp[:], pst[:, tc_, 0 * BC + b, :],
                    swb[:, b * S:b * S + 1], None, op0=ALU.mult)
                nx.vector.scalar_tensor_tensor(
                    tmp[:], pst[:, tc_, 1 * BC + b, :],
                    swb[:, b * S + 1:b * S + 2], tmp[:],
                    op0=ALU.mult, op1=ALU.add)
                nx.vector.scalar_tensor_tensor(
                    wacc[:, tc_, :], pst[:, tc_, 2 * BC + b, :],
                    swb[:, b * S + 2:b * S + 3], tmp[:],
                    op0=ALU.mult, op1=ALU.add)
            wfm = ghp.tile([128, OCH, T], BF16, tag="wfm")
            for tc_ in range(TCH):
                for oc in range(OCH):
                    pt = pp256.tile([128, 128], BF16, tag="pexp")
                    nx.tensor.transpose(pt[:], wacc[:, tc_, bass.ts(oc, 128)],
                                        ident[:])
                    nx.vector.tensor_copy(wfm[:, oc, bass.ts(tc_, 128)], pt[:])
            o1 = ghp.tile([128, OCH, T], BF16, tag="o1")
            for mc in range(OCH):
                ps = pp512.tile([128, T], F32, tag="ps512")
                for kc in range(OCH):
                    nx.tensor.matmul(ps[:], ow1[:, kc, bass.ts(mc, 128)],
                                     wfm[:, kc, :],
                                     start=(kc == 0), stop=(kc == OCH - 1))
                nx.scalar.activation(o1[:, mc, :], ps[:], AF.Gelu,
                                     bias=ob1[:, mc:mc + 1])
            for tc_ in range(TCH):
                pf = pp256.tile([128, O], F32, tag="pexp")
                for kc in range(OCH):
                    nx.tensor.matmul(pf[:], o1[:, kc, bass.ts(tc_, 128)],
                                     ow2[:, kc, :],
                                     start=(kc == 0), stop=(kc == OCH - 1))
                if have_ob2:
                    nx.vector.tensor_tensor(pf[:], pf[:], ob2bc[:], op=ALU.add)
                osb = sm.tile([128, O], F32, tag="osb")
                nx.vector.tensor_copy(osb[:], pf[:])
                nx.sync.dma_start(out_ext[b, bass.ts(tc_, 128), :], osb[:])

    nc.compile()
    return nc


def prep_weights(inp):
    h = {}
    w1 = np.asarray(inp["router_w1"], np.float32)
    w1hi = w1.astype(_bf).astype(np.float32)
    h["w_w1hi"] = _to_bf(_feat_major(w1hi))
    h["w_w1lo"] = _to_bf(_feat_major(w1 - w1hi))
    h["w_rw2"] = _feat_major(np.asarray(inp["router_w2"], np.float32) * 0.5)
    h["w_rb1s"] = _bias_chunks(np.asarray(inp["router_b1"]) * RT2)
    aW1 = np.concatenate([inp["gW1"], inp["lW1"]], 0)
    h["w_aW1"] = _to_bf(np.stack([_feat_major(aW1[n]) for n in range(NE)], 2))
    ab1 = np.concatenate([inp["gb1"], inp["lb1"]], 0)
    h["w_ab1"] = np.ascontiguousarray(
        np.stack([_bias_chunks(ab1[n]) for n in range(NE)], 1))
    h["w_gW2"] = _to_bf(np.stack(
        [_feat_major(np.asarray(inp["gW2"])[n]) for n in range(NS)], 2))
    h["w_gb2"] = np.ascontiguousarray(
        np.stack([_bias_chunks(np.asarray(inp["gb2"])[n]) for n in range(NS)], 1))
    aWo = np.concatenate([inp["gW3"], inp["lW2"]], 0)
    h["w_aWo"] = _to_bf(np.stack([_feat_major(aWo[n]) for n in range(NE)], 2))
    h["w_tew"] = _to_bf(_feat_major(np.asarray(inp["te_w"], np.float32) / C))
    h["w_teb"] = _bias_chunks(inp["te_b"])
    wcw = np.zeros((896, S), np.float32)
    wcw[:768 + S] = np.asarray(inp["wc_w"], np.float32)
    h["w_wcw"] = _to_bf(_feat_major(wcw))
    wcb = np.zeros((128, 1), np.float32)
    wcb[:S, 0] = np.asarray(inp["wc_b"])
    h["w_wcb"] = wcb
    h["w_ow1"] = _to_bf(_feat_major(inp["out_w1"]))
    h["w_ob1"] = _bias_chunks(inp["out_b1"])
    h["w_ow2"] = _to_bf(_feat_major(inp["out_w2"]))
    h["w_ident"] = _to_bf(np.eye(128, dtype=np.float32))
    h["w_wmbc"] = _to_bf(np.broadcast_to(
        np.asarray(inp["weight_memory"], np.float32).reshape(S, 1), (S, BC)))
    if np.any(np.asarray(inp["router_b2"]) != 0):
        h["w_rb2bc"] = np.ascontiguousarray(np.broadcast_to(
            np.asarray(inp["router_b2"], np.float32), (128, NE)))
    bout = np.concatenate([inp["gb3"], inp["lb2"]], 0)
    if np.any(bout != 0):
        h["w_boutbc"] = np.ascontiguousarray(np.broadcast_to(
            np.asarray(bout, np.float32).reshape(1, NE * O), (128, NE * O)))
    if np.any(np.asarray(inp["out_b2"]) != 0):
        h["w_ob2bc"] = np.ascontiguousarray(np.broadcast_to(
            np.asarray(inp["out_b2"], np.float32), (128, O)))
    return h


def prep_x(xs):
    """xs [S,B,C,E] f32 -> per-core (xhi, xlo), each [NT, ECH, 128, T] bf16."""
    out = []
    for c in range(NCORES):
        blk = np.asarray(xs, np.float32)[:, c * BC:(c + 1) * BC]
        xt = blk.transpose(0, 1, 3, 2).reshape(NT, ECH, 128, T)
        hi = xt.astype(_bf)
        lo = (xt - hi.astype(np.float32)).astype(_bf)
        out.append((np.ascontiguousarray(hi), np.ascontiguousarray(lo)))
    return out


_CACHE = {}


def build_in_maps(inputs):
    host = prep_weights(inputs)
    key = "graph:" + ",".join(sorted(host))
    if key not in _CACHE:
        _CACHE[key] = build_graph(host)
    nc = _CACHE[key]
    xs_shards = prep_x(inputs["xs"])
    in_maps = []
    for c in range(NCORES):
        m = dict(host)
        m["xhi"], m["xlo"] = xs_shards[c]
        in_maps.append(m)
    return nc, in_maps


def assemble(results):
    outs = np.concatenate(
        [results[c]["out"][None] for c in range(NCORES)], 0).reshape(B, C, O)
    ent_sum = sum(float(results[c]["ent"][0, 0]) for c in range(NCORES))
    bl = np.float32(0.1 * ent_sum / (S * B * C))
    return outs, bl


def kernel(**inputs):
    nc, in_maps = build_in_maps(inputs)
    res = run_bass_kernel_spmd(nc, in_maps, core_ids=list(range(NCORES)))
    return assemble(res.results)


# revision 7
# speedup vs baseline: 1.1207x; 1.1207x over previous
"""AdaptiveScaleRoutingMoE block on 8 TRN2 NeuronCores.

Strategy: data-parallel over batch (B=32 -> 4 per core). All weights
replicated. Per (scale, batch) pair one 512-token tile, 12 tiles/core.

Precision: router L1 in split-bf16 (hi/lo, 3 cross terms, exact gelu via the
Erf LUT), router L2 in f32 (top-2 selection is rank-sensitive), experts bf16
with f32 accumulation. The top-2-of-6 local-expert gather is replaced by a
mask built from the second max; the weighted combine runs token-major via
scalar_tensor_tensor with per-partition router-weight columns. Balance-loss
entropy uses ln(sum_exp) - sum(p*logit) (log-softmax identity), partial sums
reduced on host.
"""
import numpy as np
import ml_dtypes
from contextlib import ExitStack

import concourse.bass as bass
import concourse.tile as tile
from concourse import bacc, mybir
from concourse.bass_utils import run_bass_kernel_spmd

F32 = mybir.dt.float32
BF16 = mybir.dt.bfloat16
AF = mybir.ActivationFunctionType
ALU = mybir.AluOpType
AX = mybir.AxisListType

S, B, C, E = 3, 32, 512, 256
H, O, NE, NS, NL = 512, 256, 8, 2, 6
NCORES = 8
BC = B // NCORES          # batches per core
NT = S * BC               # token tiles per core (one per (s,b)), each T tokens
T = C                     # 512 tokens per tile
TCH = T // 128            # 4 token chunks
ECH = E // 128            # 2
HCH = H // 128            # 4
OCH = O // 128            # 2
KCC = 896 // 128          # 7 concat chunks (771 padded to 896)
RT2 = float(1.0 / np.sqrt(2.0))

_bf = ml_dtypes.bfloat16


def _to_bf(a):
    return np.ascontiguousarray(np.asarray(a, np.float32).astype(_bf))


def _feat_major(w):
    """[K, M] weight -> SBUF lhsT layout [128, K/128, M]."""
    k, m = w.shape
    return np.ascontiguousarray(
        np.asarray(w, np.float32).reshape(k // 128, 128, m).transpose(1, 0, 2))


def _bias_chunks(b):
    """[F] bias -> [128, F/128] per-partition layout."""
    f = b.shape[0]
    return np.ascontiguousarray(np.asarray(b, np.float32).reshape(f // 128, 128).T)


def build_graph(host):
    """host: dict of prepped numpy weight arrays. Returns compiled nc."""
    nc = bacc.Bacc("TRN2", target_bir_lowering=False, debug=False,
                   num_devices=NCORES)

    dram = {}

    def din(name, shape, dt):
        dram[name] = nc.dram_tensor(name, list(shape), dt, kind="ExternalInput")
        return dram[name]

    din("xhi", [NT, ECH, 128, T], BF16)
    din("xlo", [NT, ECH, 128, T], BF16)
    for k, v in host.items():
        din(k, v.shape, BF16 if v.dtype == _bf else F32)

    out_ext = nc.dram_tensor("out", [BC, C, O], F32, kind="ExternalOutput")
    ent_ext = nc.dram_tensor("ent", [1, 1], F32, kind="ExternalOutput")
    sw_bounce = nc.dram_tensor("sw_bounce", [BC * S], F32)
    tr_bounce = nc.dram_tensor("tr_bounce", [S, BC], F32)
    mean_bounce = nc.dram_tensor("mean_bounce", [NT, O], F32)

    have_rb2 = "w_rb2bc" in host
    have_bout = "w_boutbc" in host
    have_ob2 = "w_ob2bc" in host

    with tile.TileContext(nc) as tc, ExitStack() as ctx:
        nx = nc
        wp = ctx.enter_context(tc.tile_pool(name="weights", bufs=1))
        xp = ctx.enter_context(tc.tile_pool(name="xstash", bufs=1))
        stash = ctx.enter_context(tc.tile_pool(name="stash", bufs=1))
        hwork = ctx.enter_context(tc.tile_pool(name="hwork", bufs=2))
        ewp = ctx.enter_context(tc.tile_pool(name="ewp", bufs=1))
        ghp = ctx.enter_context(tc.tile_pool(name="ghp", bufs=1))
        sm = ctx.enter_context(tc.tile_pool(name="small", bufs=3))
        acc = ctx.enter_context(tc.tile_pool(name="accp", bufs=2))
        pp512 = ctx.enter_context(tc.tile_pool(name="pp512", bufs=2, space="PSUM"))
        pp256 = ctx.enter_context(tc.tile_pool(name="pp256", bufs=4, space="PSUM"))
        pps = ctx.enter_context(tc.tile_pool(name="pps", bufs=2, space="PSUM"))

        def wt(name, dt=BF16):
            a = host[name]
            t_ = wp.tile(list(a.shape), dt, tag=name)
            nx.sync.dma_start(t_[:], dram[name][:])
            return t_

        w1hi = wt("w_w1hi"); w1lo = wt("w_w1lo")
        rw2 = wt("w_rw2", F32)
        rb1s = wt("w_rb1s", F32)          # router_b1 / sqrt(2), [128, HCH]
        aW1 = wt("w_aW1"); ab1 = wt("w_ab1", F32)
        gW2 = wt("w_gW2"); gb2 = wt("w_gb2", F32)
        aWo = wt("w_aWo")
        tew = wt("w_tew"); teb = wt("w_teb", F32)
        wcw = wt("w_wcw"); wcb = wt("w_wcb", F32)
        ow1 = wt("w_ow1"); ob1 = wt("w_ob1", F32)
        ow2 = wt("w_ow2")
        ident = wt("w_ident")
        rb2bc = wt("w_rb2bc", F32) if have_rb2 else None
        boutbc = wt("w_boutbc", F32) if have_bout else None
        ob2bc = wt("w_ob2bc", F32) if have_ob2 else None

        onesb = wp.tile([128, 1], BF16, tag="onesb")
        nx.vector.memset(onesb[:], 1.0)
        onesf = wp.tile([128, 1], F32, tag="onesf")
        nx.vector.memset(onesf[:], 1.0)

        xhi = xp.tile([128, ECH, NT, T], BF16, tag="xhi")
        for i in range(NT):
            for ec in range(ECH):
                nx.sync.dma_start(xhi[:, ec, i, :], dram["xhi"][i, ec])

        logit_st = stash.tile([128, TCH, NT, NE], F32, tag="logit")
        wst = stash.tile([128, TCH, NT, NE], F32, tag="wst")
        ssum_st = stash.tile([128, TCH * NT], F32, tag="ssum_st")
        spl_st = stash.tile([128, TCH * NT], F32, tag="spl_st")
        pst = stash.tile([128, TCH, NT, O], BF16, tag="pst")
        mlin = stash.tile([1, NT, O], F32, tag="mlin")
        mfm = stash.tile([128, OCH, NT], BF16, tag="mfm")
        reprs = stash.tile([128, OCH, NT], BF16, tag="reprs")
        ccat = stash.tile([128, KCC, BC], BF16, tag="ccat")
        th_sb = stash.tile([S, BC], F32, tag="th")
        swb = stash.tile([128, BC * S], F32, tag="swb")

        # ---------------- Phase R: router L1 (Erf) + L2 (f32) --------------
        for i in range(NT):
            xlo_t = hwork.tile([128, ECH, T], BF16, tag="xlo")
            for ec in range(ECH):
                nx.sync.dma_start(xlo_t[:, ec, :], dram["xlo"][i, ec])
            h1 = hwork.tile([128, HCH, T], F32, tag="h1")
            for mc in range(HCH):
                ps = pp512.tile([128, T], F32, tag="ps512")
                terms = [(w1hi, xhi[:, ec, i, :]) for ec in range(ECH)]
                terms += [(w1lo, xhi[:, ec, i, :]) for ec in range(ECH)]
                terms += [(w1hi, xlo_t[:, ec, :]) for ec in range(ECH)]
                # order: (hi,xhi) ec0,ec1 | (lo,xhi) | (hi,xlo)
                n_terms = len(terms)
                for j, (lhs, r) in enumerate(terms):
                    ec = j % ECH
                    nx.tensor.matmul(
                        ps[:], lhs[:, ec, bass.ts(mc, 128)], r,
                        start=(j == 0), stop=(j == n_terms - 1))
                erf = hwork.tile([128, T], F32, tag="erf")
                # erf((z + b1)/sqrt(2)); z in psum; rb1s = b1/sqrt(2).
                nx.scalar.activation(erf[:], ps[:], AF.Erf,
                                     bias=rb1s[:, mc:mc + 1], scale=RT2)
                # h1 = (erf + 1) * z = 2*gelu(z)  (0.5 folded into rw2)
                nx.vector.scalar_tensor_tensor(
                    h1[:, mc, :], erf[:], 1.0, ps[:], op0=ALU.add, op1=ALU.mult)
            for tc_ in range(TCH):
                pl = pps.tile([128, NE], F32, tag="ppsmall")
                for kc in range(HCH):
                    nx.tensor.matmul(
                        pl[:], h1[:, kc, bass.ts(tc_, 128)], rw2[:, kc, :],
                        start=(kc == 0), stop=(kc == HCH - 1))
                if have_rb2:
                    nx.vector.tensor_tensor(pl[:], pl[:], rb2bc[:, :NE],
                                            op=ALU.add)
                nx.vector.tensor_copy(logit_st[:, tc_, i, :], pl[:])

        tc.no_sync_barrier()
        # ---------------- Phase W: softmax + entropy + top-2 ---------------
        for i in range(NT):
            for tc_ in range(TCH):
                lgt = logit_st[:, tc_, i, :]
                col = i * TCH + tc_
                e8 = sm.tile([128, NE], F32, tag="e8")
                nx.scalar.activation(e8[:], lgt, AF.Exp)
                ssum = ssum_st[:, col:col + 1]
                nx.vector.reduce_sum(ssum, e8[:], axis=AX.X)
                rcp = sm.tile([128, 1], F32, tag="rcp")
                nx.vector.reciprocal(rcp[:], ssum)
                w8 = wst[:, tc_, i, :]
                nx.vector.tensor_scalar(w8, e8[:], rcp[:], None, op0=ALU.mult)
                pl8 = sm.tile([128, NE], F32, tag="pl8")
                nx.vector.tensor_tensor(pl8[:], w8, lgt, op=ALU.mult)
                nx.vector.reduce_sum(spl_st[:, col:col + 1], pl8[:], axis=AX.X)
                # top-2 of local expert weights
                wl = wst[:, tc_, i, NS:NE]
                m1 = sm.tile([128, 1], F32, tag="m1")
                nx.vector.reduce_max(m1[:], wl, axis=AX.X)
                eq = sm.tile([128, NL], F32, tag="eq")
                nx.vector.tensor_scalar(eq[:], wl, m1[:], None, op0=ALU.is_equal)
                w2 = sm.tile([128, NL], F32, tag="w2")
                nx.vector.scalar_tensor_tensor(
                    w2[:], eq[:], -1e30, wl, op0=ALU.mult, op1=ALU.add)
                m2 = sm.tile([128, 1], F32, tag="m2")
                nx.vector.reduce_max(m2[:], w2[:], axis=AX.X)
                sel = sm.tile([128, NL], F32, tag="sel")
                nx.vector.tensor_scalar(sel[:], wl, m2[:], None, op0=ALU.is_ge)
                nx.vector.tensor_tensor(wl, wl, sel[:], op=ALU.mult)

        tc.no_sync_barrier()
        # ---------------- Phase E: experts (Gelu) + combine + means --------
        for i in range(NT):
            eh = ewp.tile([128, HCH, NE, T], BF16, tag="eh")
            for n in range(NE):
                for mc in range(HCH):
                    ps = pp512.tile([128, T], F32, tag="ps512")
                    for ec in range(ECH):
                        nx.tensor.matmul(
                            ps[:], aW1[:, ec, n, bass.ts(mc, 128)],
                            xhi[:, ec, i, :],
                            start=(ec == 0), stop=(ec == ECH - 1))
                    nx.scalar.activation(eh[:, mc, n, :], ps[:], AF.Gelu,
                                         bias=ab1[:, n, mc:mc + 1])
            gh2 = ghp.tile([128, HCH, NS, T], BF16, tag="gh2")
            for n in range(NS):
                for mc in range(HCH):
                    ps = pp512.tile([128, T], F32, tag="ps512")
                    for kc in range(HCH):
                        nx.tensor.matmul(
                            ps[:], gW2[:, kc, n, bass.ts(mc, 128)],
                            eh[:, kc, n, :],
                            start=(kc == 0), stop=(kc == HCH - 1))
                    nx.scalar.activation(gh2[:, mc, n, :], ps[:], AF.Gelu,
                                         bias=gb2[:, n, mc:mc + 1])
            for tc_ in range(TCH):
                ac = acc.tile([128, O], F32, tag="acc")
                for n in range(NE):
                    src = gh2 if n < NS else eh
                    pn = pp256.tile([128, O], F32, tag="pexp")
                    for kc in range(HCH):
                        nx.tensor.matmul(
                            pn[:], src[:, kc, n, bass.ts(tc_, 128)],
                            aWo[:, kc, n, :],
                            start=(kc == 0), stop=(kc == HCH - 1))
                    if have_bout:
                        nx.vector.tensor_tensor(
                            pn[:], pn[:], boutbc[:, bass.ts(n, O)], op=ALU.add)
                    if n == 0:
                        nx.vector.tensor_scalar(
                            ac[:], pn[:], wst[:, tc_, i, 0:1], None,
                            op0=ALU.mult)
                    else:
                        nx.vector.scalar_tensor_tensor(
                            ac[:], pn[:], wst[:, tc_, i, n:n + 1], ac[:],
                            op0=ALU.mult, op1=ALU.add)
                nx.vector.tensor_copy(pst[:, tc_, i, :], ac[:])
            pm = pps.tile([1, O], F32, tag="ppsmall")
            for tc_ in range(TCH):
                nx.tensor.matmul(pm[:], onesb[:], pst[:, tc_, i, :],
                                 start=(tc_ == 0), stop=(tc_ == TCH - 1))
            nx.vector.tensor_copy(mlin[:, i, :], pm[:])
        # means -> feature-major [128, OCH, NT] via DRAM bounce
        nx.sync.dma_start(mean_bounce[:], mlin[0, :, :])
        mfm_f = acc.tile([128, OCH, NT], F32, tag="mfmf")
        for oc in range(OCH):
            nx.sync.dma_start(
                mfm_f[:, oc, :],
                mean_bounce[:, oc * 128:(oc + 1) * 128].transpose([1, 0]))
        nx.vector.tensor_copy(mfm[:], mfm_f[:])
        for mc in range(OCH):
            ps = pps.tile([128, NT], F32, tag="ppsmall")
            for kc in range(OCH):
                nx.tensor.matmul(ps[:], tew[:, kc, bass.ts(mc, 128)],
                                 mfm[:, kc, :],
                                 start=(kc == 0), stop=(kc == OCH - 1))
            nx.scalar.activation(reprs[:, mc, :], ps[:], AF.Gelu,
                                 bias=teb[:, mc:mc + 1])
        # concat [771 padded 896, BC]; rows s*256+o2 from reprs, 768+s from wm
        nx.vector.memset(ccat[:], 0.0)
        for s in range(S):
            for oc in range(OCH):
                nx.sync.dma_start(ccat[:, s * OCH + oc, :],
                                  reprs[:, oc, s * BC:(s + 1) * BC])
        nx.sync.dma_start(ccat[0:S, KCC - 1, :], dram["w_wmbc"][:])
        pw = pps.tile([S, BC], F32, tag="ppsmall")
        for kc in range(KCC):
            nx.tensor.matmul(pw[:], wcw[:, kc, :], ccat[:, kc, :],
                             start=(kc == 0), stop=(kc == KCC - 1))
        nx.scalar.activation(th_sb[:], pw[:], AF.Tanh, bias=wcb[0:S, 0:1])

        tc.no_sync_barrier()
        # ---------------- Phase G: softplus + scale softmax + entropy ------
        ee = sm.tile([S, BC], F32, tag="ee")
        nx.scalar.activation(ee[:], th_sb[:], AF.Exp)
        nx.vector.tensor_scalar(ee[:], ee[:], 1.0, None, op0=ALU.add)
        raw = sm.tile([S, BC], F32, tag="raw")
        nx.scalar.activation(raw[:], ee[:], AF.Ln)
        nx.sync.dma_start(tr_bounce[:], raw[:])
        rawt = sm.tile([BC, S], F32, tag="rawt")
        nx.sync.dma_start(rawt[:], tr_bounce.ap().transpose([1, 0]))
        ex = sm.tile([BC, S], F32, tag="ex")
        nx.scalar.activation(ex[:], rawt[:], AF.Exp)
        ssum2 = sm.tile([BC, 1], F32, tag="ssum2")
        nx.vector.reduce_sum(ssum2[:], ex[:], axis=AX.X)
        rcp2 = sm.tile([BC, 1], F32, tag="rcp2")
        nx.vector.reciprocal(rcp2[:], ssum2[:])
        swt = sm.tile([BC, S], F32, tag="swt")
        nx.vector.tensor_scalar(swt[:], ex[:], rcp2[:], None, op0=ALU.mult)
        nx.sync.dma_start(sw_bounce[:], swt[:])
        nx.sync.dma_start(swb[:], sw_bounce.ap().partition_broadcast(128))
        lns_all = sm.tile([128, TCH * NT], F32, tag="lns_all")
        nx.scalar.activation(lns_all[:], ssum_st[:], AF.Ln)
        ent_all = sm.tile([128, TCH * NT], F32, tag="ent_all")
        nx.vector.tensor_tensor(ent_all[:], lns_all[:], spl_st[:],
                                op=ALU.subtract)
        entv = sm.tile([128, 1], F32, tag="entv")
        nx.vector.reduce_sum(entv[:], ent_all[:], axis=AX.X)
        pe_ = pps.tile([1, 1], F32, tag="ppsmall")
        nx.tensor.matmul(pe_[:], onesf[:], entv[:], start=True, stop=True)
        esb = sm.tile([1, 1], F32, tag="esb")
        nx.vector.tensor_copy(esb[:], pe_[:])
        nx.sync.dma_start(ent_ext[:], esb[:])

        tc.no_sync_barrier()
        # ---------------- Phase OUT: scale-weighted sum + output MLP -------
        for b in range(BC):
            wacc = acc.tile([128, TCH, O], BF16, tag="wacc")
            for tc_ in range(TCH):
                tmp = sm.tile([128, O], F32, tag="wtmp")
                nx.vector.tensor_scalar(
                    tmp[:], pst[:, tc_, 0 * BC + b, :],
                    swb[:, b * S:b * S + 1], None, op0=ALU.mult)
                nx.vector.scalar_tensor_tensor(
                    tmp[:], pst[:, tc_, 1 * BC + b, :],
                    swb[:, b * S + 1:b * S + 2], tmp[:],
                    op0=ALU.mult, op1=ALU.add)
                nx.vector.scalar_tensor_tensor(
                    wacc[:, tc_, :], pst[:, tc_, 2 * BC + b, :],
                    swb[:, b * S + 2:b * S + 3], tmp[:],
                    op0=ALU.mult, op1=ALU.add)
            wfm = ghp.tile([128, OCH, T], BF16, tag="wfm")
            for tc_ in range(TCH):
                for oc in range(OCH):
                    pt = pp256.tile([128, 128], BF16, tag="pexp")
                    nx.tensor.transpose(pt[:], wacc[:, tc_, bass.ts(oc, 128)],
                                        ident[:])
                    nx.vector.tensor_copy(wfm[:, oc, bass.ts(tc_, 128)], pt[:])
            o1 = ghp.tile([128, OCH, T], BF16, tag="o1")
            for mc in range(OCH):
                ps = pp512.tile([128, T], F32, tag="ps512")
                for kc in range(OCH):
                    nx.tensor.matmul(ps[:], ow1[:, kc, bass.ts(mc, 128)],
                                     wfm[:, kc, :],
                                     start=(kc == 0), stop=(kc == OCH - 1))
                nx.scalar.activation(o1[:, mc, :], ps[:], AF.Gelu,
                                     bias=ob1[:, mc:mc + 1])
            for tc_ in range(TCH):
                pf = pp256.tile([128, O], F32, tag="pexp")
                for kc in range(OCH):
                    nx.tensor.matmul(pf[:], o1[:, kc, bass.ts(tc_, 128)],
                                     ow2[:, kc, :],
                                     start=(kc == 0), stop=(kc == OCH - 1))
                if have_ob2:
                    nx.vector.tensor_tensor(pf[:], pf[:], ob2bc[:], op=ALU.add)
                osb = sm.tile([128, O], F32, tag="osb")
                nx.vector.tensor_copy(osb[:], pf[:])
                nx.sync.dma_start(out_ext[b, bass.ts(tc_, 128), :], osb[:])

    nc.compile()
    return nc


def prep_weights(inp):
    h = {}
    w1 = np.asarray(inp["router_w1"], np.float32)
    w1hi = w1.astype(_bf).astype(np.float32)
    h["w_w1hi"] = _to_bf(_feat_major(w1hi))
    h["w_w1lo"] = _to_bf(_feat_major(w1 - w1hi))
    h["w_rw2"] = _feat_major(np.asarray(inp["router_w2"], np.float32) * 0.5)
    h["w_rb1s"] = _bias_chunks(np.asarray(inp["router_b1"]) * RT2)
    aW1 = np.concatenate([inp["gW1"], inp["lW1"]], 0)
    h["w_aW1"] = _to_bf(np.stack([_feat_major(aW1[n]) for n in range(NE)], 2))
    ab1 = np.concatenate([inp["gb1"], inp["lb1"]], 0)
    h["w_ab1"] = np.ascontiguousarray(
        np.stack([_bias_chunks(ab1[n]) for n in range(NE)], 1))
    h["w_gW2"] = _to_bf(np.stack(
        [_feat_major(np.asarray(inp["gW2"])[n]) for n in range(NS)], 2))
    h["w_gb2"] = np.ascontiguousarray(
        np.stack([_bias_chunks(np.asarray(inp["gb2"])[n]) for n in range(NS)], 1))
    aWo = np.concatenate([inp["gW3"], inp["lW2"]], 0)
    h["w_aWo"] = _to_bf(np.stack([_feat_major(aWo[n]) for n in range(NE)], 2))
    h["w_tew"] = _to_bf(_feat_major(np.asarray(inp["te_w"], np.float32) / C))
    h["w_teb"] = _bias_chunks(inp["te_b"])
    wcw = np.zeros((896, S), np.float32)
    wcw[:768 + S] = np.asarray(inp["wc_w"], np.float32)
    h["w_wcw"] = _to_bf(_feat_major(wcw))
    wcb = np.zeros((128, 1), np.float32)
    wcb[:S, 0] = np.asarray(inp["wc_b"])
    h["w_wcb"] = wcb
    h["w_ow1"] = _to_bf(_feat_major(inp["out_w1"]))
    h["w_ob1"] = _bias_chunks(inp["out_b1"])
    h["w_ow2"] = _to_bf(_feat_major(inp["out_w2"]))
    h["w_ident"] = _to_bf(np.eye(128, dtype=np.float32))
    h["w_wmbc"] = _to_bf(np.broadcast_to(
        np.asarray(inp["weight_memory"], np.float32).reshape(S, 1), (S, BC)))
    if np.any(np.asarray(inp["router_b2"]) != 0):
        h["w_rb2bc"] = np.ascontiguousarray(np.broadcast_to(
            np.asarray(inp["router_b2"], np.float32), (128, NE)))
    bout = np.concatenate([inp["gb3"], inp["lb2"]], 0)
    if np.any(bout != 0):
        h["w_boutbc"] = np.ascontiguousarray(np.broadcast_to(
            np.asarray(bout, np.float32).reshape(1, NE * O), (128, NE * O)))
    if np.any(np.asarray(inp["out_b2"]) != 0):
        h["w_ob2bc"] = np.ascontiguousarray(np.broadcast_to(
            np.asarray(inp["out_b2"], np.float32), (128, O)))
    return h


def prep_x(xs):
    """xs [S,B,C,E] f32 -> per-core (xhi, xlo), each [NT, ECH, 128, T] bf16."""
    out = []
    for c in range(NCORES):
        blk = np.asarray(xs, np.float32)[:, c * BC:(c + 1) * BC]
        xt = blk.transpose(0, 1, 3, 2).reshape(NT, ECH, 128, T)
        hi = xt.astype(_bf)
        lo = (xt - hi.astype(np.float32)).astype(_bf)
        out.append((np.ascontiguousarray(hi), np.ascontiguousarray(lo)))
    return out


_CACHE = {}


def build_in_maps(inputs):
    host = prep_weights(inputs)
    key = "graph:" + ",".join(sorted(host))
    if key not in _CACHE:
        _CACHE[key] = build_graph(host)
    nc = _CACHE[key]
    xs_shards = prep_x(inputs["xs"])
    in_maps = []
    for c in range(NCORES):
        m = dict(host)
        m["xhi"], m["xlo"] = xs_shards[c]
        in_maps.append(m)
    return nc, in_maps


def assemble(results):
    outs = np.concatenate(
        [results[c]["out"][None] for c in range(NCORES)], 0).reshape(B, C, O)
    ent_sum = sum(float(results[c]["ent"][0, 0]) for c in range(NCORES))
    bl = np.float32(0.1 * ent_sum / (S * B * C))
    return outs, bl


def kernel(**inputs):
    nc, in_maps = build_in_maps(inputs)
    res = run_bass_kernel_spmd(nc, in_maps, core_ids=list(range(NCORES)))
    return assemble(res.results)


# revision 8
# speedup vs baseline: 1.1814x; 1.0542x over previous
"""AdaptiveScaleRoutingMoE block on 8 TRN2 NeuronCores.

Strategy: data-parallel over batch (B=32 -> 4 per core). All weights
replicated. Per (scale, batch) pair one 512-token tile, 12 tiles/core.

Precision: router L1 in split-bf16 (hi/lo, 3 cross terms, exact gelu via the
Erf LUT), router L2 in f32 (top-2 selection is rank-sensitive), experts bf16
with f32 accumulation. The top-2-of-6 local-expert gather is replaced by a
mask built from the second max; the weighted combine runs token-major via
scalar_tensor_tensor with per-partition router-weight columns. Balance-loss
entropy uses ln(sum_exp) - sum(p*logit) (log-softmax identity), partial sums
reduced on host.
"""
import numpy as np
import ml_dtypes
from contextlib import ExitStack

import concourse.bass as bass
import concourse.tile as tile
from concourse import bacc, mybir
from concourse.bass_utils import run_bass_kernel_spmd

F32 = mybir.dt.float32
BF16 = mybir.dt.bfloat16
AF = mybir.ActivationFunctionType
ALU = mybir.AluOpType
AX = mybir.AxisListType

S, B, C, E = 3, 32, 512, 256
H, O, NE, NS, NL = 512, 256, 8, 2, 6
NCORES = 8
BC = B // NCORES          # batches per core
NT = S * BC               # token tiles per core (one per (s,b)), each T tokens
T = C                     # 512 tokens per tile
TCH = T // 128            # 4 token chunks
ECH = E // 128            # 2
HCH = H // 128            # 4
OCH = O // 128            # 2
KCC = 896 // 128          # 7 concat chunks (771 padded to 896)
RT2 = float(1.0 / np.sqrt(2.0))

_bf = ml_dtypes.bfloat16


def _to_bf(a):
    return np.ascontiguousarray(np.asarray(a, np.float32).astype(_bf))


def _feat_major(w):
    """[K, M] weight -> SBUF lhsT layout [128, K/128, M]."""
    k, m = w.shape
    return np.ascontiguousarray(
        np.asarray(w, np.float32).reshape(k // 128, 128, m).transpose(1, 0, 2))


def _bias_chunks(b):
    """[F] bias -> [128, F/128] per-partition layout."""
    f = b.shape[0]
    return np.ascontiguousarray(np.asarray(b, np.float32).reshape(f // 128, 128).T)


def build_graph(host):
    """host: dict of prepped numpy weight arrays. Returns compiled nc."""
    nc = bacc.Bacc("TRN2", target_bir_lowering=False, debug=False,
                   num_devices=NCORES)

    dram = {}

    def din(name, shape, dt):
        dram[name] = nc.dram_tensor(name, list(shape), dt, kind="ExternalInput")
        return dram[name]

    din("xhi", [NT, ECH, 128, T], BF16)
    din("xlo", [NT, ECH, 128, T], BF16)
    for k, v in host.items():
        din(k, v.shape, BF16 if v.dtype == _bf else F32)

    out_ext = nc.dram_tensor("out", [BC, C, O], F32, kind="ExternalOutput")
    ent_ext = nc.dram_tensor("ent", [1, 1], F32, kind="ExternalOutput")
    sw_bounce = nc.dram_tensor("sw_bounce", [BC * S], F32)
    tr_bounce = nc.dram_tensor("tr_bounce", [S, BC], F32)
    mean_bounce = nc.dram_tensor("mean_bounce", [NT, O], F32)

    have_rb2 = "w_rb2bc" in host
    have_bout = "w_boutbc" in host
    have_ob2 = "w_ob2bc" in host

    with tile.TileContext(nc) as tc, ExitStack() as ctx:
        nx = nc
        wp = ctx.enter_context(tc.tile_pool(name="weights", bufs=1))
        xp = ctx.enter_context(tc.tile_pool(name="xstash", bufs=1))
        stash = ctx.enter_context(tc.tile_pool(name="stash", bufs=1))
        hwork = ctx.enter_context(tc.tile_pool(name="hwork", bufs=2))
        ewp = ctx.enter_context(tc.tile_pool(name="ewp", bufs=1))
        ghp = ctx.enter_context(tc.tile_pool(name="ghp", bufs=1))
        sm = ctx.enter_context(tc.tile_pool(name="small", bufs=3))
        acc = ctx.enter_context(tc.tile_pool(name="accp", bufs=2))
        pp512 = ctx.enter_context(tc.tile_pool(name="pp512", bufs=2, space="PSUM"))
        pp256 = ctx.enter_context(tc.tile_pool(name="pp256", bufs=4, space="PSUM"))
        pps = ctx.enter_context(tc.tile_pool(name="pps", bufs=2, space="PSUM"))

        def wt(name, dt=BF16):
            a = host[name]
            t_ = wp.tile(list(a.shape), dt, tag=name)
            nx.sync.dma_start(t_[:], dram[name][:])
            return t_

        # DMA priority order: what phase R needs first
        w1hi = wt("w_w1hi"); w1lo = wt("w_w1lo")
        rw2 = wt("w_rw2", F32)
        rb1s = wt("w_rb1s", F32)          # router_b1 / sqrt(2), [128, HCH]
        xhi = xp.tile([128, ECH, NT, T], BF16, tag="xhi")
        for i in range(NT):
            for ec in range(ECH):
                nx.sync.dma_start(xhi[:, ec, i, :], dram["xhi"][i, ec])
        aW1 = wt("w_aW1"); ab1 = wt("w_ab1", F32)
        gW2 = wt("w_gW2"); gb2 = wt("w_gb2", F32)
        aWo = wt("w_aWo")
        tew = wt("w_tew"); teb = wt("w_teb", F32)
        wcw = wt("w_wcw"); wcb = wt("w_wcb", F32)
        ow1 = wt("w_ow1"); ob1 = wt("w_ob1", F32)
        ow2 = wt("w_ow2")
        ident = wt("w_ident")
        rb2bc = wt("w_rb2bc", F32) if have_rb2 else None
        boutbc = wt("w_boutbc", F32) if have_bout else None
        ob2bc = wt("w_ob2bc", F32) if have_ob2 else None

        onesb = wp.tile([128, 1], BF16, tag="onesb")
        nx.vector.memset(onesb[:], 1.0)
        onesf = wp.tile([128, 1], F32, tag="onesf")
        nx.vector.memset(onesf[:], 1.0)

        logit_st = stash.tile([128, TCH, NT, NE], F32, tag="logit")
        wst = stash.tile([128, TCH, NT, NE], F32, tag="wst")
        ssum_st = stash.tile([128, TCH * NT], F32, tag="ssum_st")
        spl_st = stash.tile([128, TCH * NT], F32, tag="spl_st")
        pst = stash.tile([128, TCH, NT, O], BF16, tag="pst")
        mlin = stash.tile([1, NT, O], F32, tag="mlin")
        mfm = stash.tile([128, OCH, NT], BF16, tag="mfm")
        reprs = stash.tile([128, OCH, NT], BF16, tag="reprs")
        ccat = stash.tile([128, KCC, BC], BF16, tag="ccat")
        th_sb = stash.tile([S, BC], F32, tag="th")
        swb = stash.tile([128, BC * S], F32, tag="swb")

        # ---------------- Phase R: router L1 (Erf) + L2 (f32) --------------
        for i in range(NT):
            xlo_t = hwork.tile([128, ECH, T], BF16, tag="xlo")
            for ec in range(ECH):
                nx.sync.dma_start(xlo_t[:, ec, :], dram["xlo"][i, ec])
            h1 = hwork.tile([128, HCH, T], F32, tag="h1")
            for mc in range(HCH):
                ps = pp512.tile([128, T], F32, tag="ps512")
                terms = [(w1hi, xhi[:, ec, i, :]) for ec in range(ECH)]
                terms += [(w1lo, xhi[:, ec, i, :]) for ec in range(ECH)]
                terms += [(w1hi, xlo_t[:, ec, :]) for ec in range(ECH)]
                # order: (hi,xhi) ec0,ec1 | (lo,xhi) | (hi,xlo)
                n_terms = len(terms)
                for j, (lhs, r) in enumerate(terms):
                    ec = j % ECH
                    nx.tensor.matmul(
                        ps[:], lhs[:, ec, bass.ts(mc, 128)], r,
                        start=(j == 0), stop=(j == n_terms - 1))
                erf = hwork.tile([128, T], F32, tag="erf")
                # erf((z + b1)/sqrt(2)); z in psum; rb1s = b1/sqrt(2).
                nx.scalar.activation(erf[:], ps[:], AF.Erf,
                                     bias=rb1s[:, mc:mc + 1], scale=RT2)
                # h1 = (erf + 1) * z = 2*gelu(z)  (0.5 folded into rw2)
                nx.vector.scalar_tensor_tensor(
                    h1[:, mc, :], erf[:], 1.0, ps[:], op0=ALU.add, op1=ALU.mult)
            for tc_ in range(TCH):
                pl = pps.tile([128, NE], F32, tag="ppsmall")
                for kc in range(HCH):
                    nx.tensor.matmul(
                        pl[:], h1[:, kc, bass.ts(tc_, 128)], rw2[:, kc, :],
                        start=(kc == 0), stop=(kc == HCH - 1))
                if have_rb2:
                    nx.vector.tensor_tensor(pl[:], pl[:], rb2bc[:, :NE],
                                            op=ALU.add)
                nx.vector.tensor_copy(logit_st[:, tc_, i, :], pl[:])

        tc.no_sync_barrier()
        # ---------------- Phase W: batched softmax + entropy + top-2 -------
        # All 48 chunks at once on [128, TCH, NT, *] views of the stashes.
        ssum3 = ssum_st[:].rearrange("p (a b) -> p a b", a=TCH)
        spl3 = spl_st[:].rearrange("p (a b) -> p a b", a=TCH)
        nx.scalar.activation(wst[:], logit_st[:], AF.Exp)
        nx.vector.reduce_sum(ssum3, wst[:], axis=AX.X)
        rcpa = sm.tile([128, TCH, NT, 1], F32, tag="rcpa")
        nx.vector.reciprocal(rcpa[:], ssum3)
        nx.vector.tensor_tensor(
            wst[:], wst[:], rcpa[:].broadcast_to([128, TCH, NT, NE]),
            op=ALU.mult)
        pl8a = sm.tile([128, TCH, NT, NE], F32, tag="pl8a")
        nx.vector.tensor_tensor(pl8a[:], wst[:], logit_st[:], op=ALU.mult)
        nx.vector.reduce_sum(spl3, pl8a[:], axis=AX.X)
        wl = wst[:, :, :, NS:NE]
        m1a = sm.tile([128, TCH, NT, 1], F32, tag="rcpa")
        nx.vector.reduce_max(m1a[:], wl, axis=AX.X)
        eqa = sm.tile([128, TCH, NT, NL], F32, tag="pl8a")
        nx.vector.tensor_tensor(
            eqa[:], wl, m1a[:].broadcast_to([128, TCH, NT, NL]),
            op=ALU.is_equal)
        w2a = sm.tile([128, TCH, NT, NL], F32, tag="w2a")
        nx.vector.scalar_tensor_tensor(
            w2a[:], eqa[:], -1e30, wl, op0=ALU.mult, op1=ALU.add)
        m2a = sm.tile([128, TCH, NT, 1], F32, tag="rcpa")
        nx.vector.reduce_max(m2a[:], w2a[:], axis=AX.X)
        sela = sm.tile([128, TCH, NT, NL], F32, tag="pl8a")
        nx.vector.tensor_tensor(
            sela[:], wl, m2a[:].broadcast_to([128, TCH, NT, NL]),
            op=ALU.is_ge)
        nx.vector.tensor_tensor(wl, wl, sela[:], op=ALU.mult)

        tc.no_sync_barrier()
        # ---------------- Phase E: experts (Gelu) + combine + means --------
        for i in range(NT):
            eh = ewp.tile([128, HCH, NE, T], BF16, tag="eh")
            for n in range(NE):
                for mc in range(HCH):
                    ps = pp512.tile([128, T], F32, tag="ps512")
                    for ec in range(ECH):
                        nx.tensor.matmul(
                            ps[:], aW1[:, ec, n, bass.ts(mc, 128)],
                            xhi[:, ec, i, :],
                            start=(ec == 0), stop=(ec == ECH - 1))
                    nx.scalar.activation(eh[:, mc, n, :], ps[:], AF.Gelu,
                                         bias=ab1[:, n, mc:mc + 1])
            gh2 = ghp.tile([128, HCH, NS, T], BF16, tag="gh2")
            for n in range(NS):
                for mc in range(HCH):
                    ps = pp512.tile([128, T], F32, tag="ps512")
                    for kc in range(HCH):
                        nx.tensor.matmul(
                            ps[:], gW2[:, kc, n, bass.ts(mc, 128)],
                            eh[:, kc, n, :],
                            start=(kc == 0), stop=(kc == HCH - 1))
                    nx.scalar.activation(gh2[:, mc, n, :], ps[:], AF.Gelu,
                                         bias=gb2[:, n, mc:mc + 1])
            for tc_ in range(TCH):
                ac = acc.tile([128, O], F32, tag="acc")
                for n in range(NE):
                    src = gh2 if n < NS else eh
                    pn = pp256.tile([128, O], F32, tag="pexp")
                    for kc in range(HCH):
                        nx.tensor.matmul(
                            pn[:], src[:, kc, n, bass.ts(tc_, 128)],
                            aWo[:, kc, n, :],
                            start=(kc == 0), stop=(kc == HCH - 1))
                    if have_bout:
                        nx.vector.tensor_tensor(
                            pn[:], pn[:], boutbc[:, bass.ts(n, O)], op=ALU.add)
                    if n == 0:
                        nx.vector.tensor_scalar(
                            ac[:], pn[:], wst[:, tc_, i, 0:1], None,
                            op0=ALU.mult)
                    else:
                        nx.vector.scalar_tensor_tensor(
                            ac[:], pn[:], wst[:, tc_, i, n:n + 1], ac[:],
                            op0=ALU.mult, op1=ALU.add)
                nx.vector.tensor_copy(pst[:, tc_, i, :], ac[:])
            pm = pps.tile([1, O], F32, tag="ppsmall")
            for tc_ in range(TCH):
                nx.tensor.matmul(pm[:], onesb[:], pst[:, tc_, i, :],
                                 start=(tc_ == 0), stop=(tc_ == TCH - 1))
            nx.vector.tensor_copy(mlin[:, i, :], pm[:])
        # means -> feature-major [128, OCH, NT] via DRAM bounce
        nx.sync.dma_start(mean_bounce[:], mlin[0, :, :])
        mfm_f = acc.tile([128, OCH, NT], F32, tag="mfmf")
        for oc in range(OCH):
            nx.sync.dma_start(
                mfm_f[:, oc, :],
                mean_bounce[:, oc * 128:(oc + 1) * 128].transpose([1, 0]))
        nx.vector.tensor_copy(mfm[:], mfm_f[:])
        for mc in range(OCH):
            ps = pps.tile([128, NT], F32, tag="ppsmall")
            for kc in range(OCH):
                nx.tensor.matmul(ps[:], tew[:, kc, bass.ts(mc, 128)],
                                 mfm[:, kc, :],
                                 start=(kc == 0), stop=(kc == OCH - 1))
            nx.scalar.activation(reprs[:, mc, :], ps[:], AF.Gelu,
                                 bias=teb[:, mc:mc + 1])
        # concat [771 padded 896, BC]; rows s*256+o2 from reprs, 768+s from wm
        nx.vector.memset(ccat[:], 0.0)
        for s in range(S):
            for oc in range(OCH):
                nx.sync.dma_start(ccat[:, s * OCH + oc, :],
                                  reprs[:, oc, s * BC:(s + 1) * BC])
        nx.sync.dma_start(ccat[0:S, KCC - 1, :], dram["w_wmbc"][:])
        pw = pps.tile([S, BC], F32, tag="ppsmall")
        for kc in range(KCC):
            nx.tensor.matmul(pw[:], wcw[:, kc, :], ccat[:, kc, :],
                             start=(kc == 0), stop=(kc == KCC - 1))
        nx.scalar.activation(th_sb[:], pw[:], AF.Tanh, bias=wcb[0:S, 0:1])

        tc.no_sync_barrier()
        # ---------------- Phase G: softplus + scale softmax + entropy ------
        ee = sm.tile([S, BC], F32, tag="ee")
        nx.scalar.activation(ee[:], th_sb[:], AF.Exp)
        nx.vector.tensor_scalar(ee[:], ee[:], 1.0, None, op0=ALU.add)
        raw = sm.tile([S, BC], F32, tag="raw")
        nx.scalar.activation(raw[:], ee[:], AF.Ln)
        nx.sync.dma_start(tr_bounce[:], raw[:])
        rawt = sm.tile([BC, S], F32, tag="rawt")
        nx.sync.dma_start(rawt[:], tr_bounce.ap().transpose([1, 0]))
        ex = sm.tile([BC, S], F32, tag="ex")
        nx.scalar.activation(ex[:], rawt[:], AF.Exp)
        ssum2 = sm.tile([BC, 1], F32, tag="ssum2")
        nx.vector.reduce_sum(ssum2[:], ex[:], axis=AX.X)
        rcp2 = sm.tile([BC, 1], F32, tag="rcp2")
        nx.vector.reciprocal(rcp2[:], ssum2[:])
        swt = sm.tile([BC, S], F32, tag="swt")
        nx.vector.tensor_scalar(swt[:], ex[:], rcp2[:], None, op0=ALU.mult)
        nx.sync.dma_start(sw_bounce[:], swt[:])
        nx.sync.dma_start(swb[:], sw_bounce.ap().partition_broadcast(128))
        lns_all = sm.tile([128, TCH * NT], F32, tag="lns_all")
        nx.scalar.activation(lns_all[:], ssum_st[:], AF.Ln)
        ent_all = sm.tile([128, TCH * NT], F32, tag="ent_all")
        nx.vector.tensor_tensor(ent_all[:], lns_all[:], spl_st[:],
                                op=ALU.subtract)
        entv = sm.tile([128, 1], F32, tag="entv")
        nx.vector.reduce_sum(entv[:], ent_all[:], axis=AX.X)
        pe_ = pps.tile([1, 1], F32, tag="ppsmall")
        nx.tensor.matmul(pe_[:], onesf[:], entv[:], start=True, stop=True)
        esb = sm.tile([1, 1], F32, tag="esb")
        nx.vector.tensor_copy(esb[:], pe_[:])
        nx.sync.dma_start(ent_ext[:], esb[:])

        tc.no_sync_barrier()
        # ---------------- Phase OUT: scale-weighted sum + output MLP -------
        for b in range(BC):
            wacc = acc.tile([128, TCH, O], BF16, tag="wacc")
            for tc_ in range(TCH):
                tmp = sm.tile([128, O], F32, tag="wtmp")
                nx.vector.tensor_scalar(
                    tmp[:], pst[:, tc_, 0 * BC + b, :],
                    swb[:, b * S:b * S + 1], None, op0=ALU.mult)
                nx.vector.scalar_tensor_tensor(
                    tmp[:], pst[:, tc_, 1 * BC + b, :],
                    swb[:, b * S + 1:b * S + 2], tmp[:],
                    op0=ALU.mult, op1=ALU.add)
                nx.vector.scalar_tensor_tensor(
                    wacc[:, tc_, :], pst[:, tc_, 2 * BC + b, :],
                    swb[:, b * S + 2:b * S + 3], tmp[:],
                    op0=ALU.mult, op1=ALU.add)
            wfm = ghp.tile([128, OCH, T], BF16, tag="wfm")
            for tc_ in range(TCH):
                for oc in range(OCH):
                    pt = pp256.tile([128, 128], BF16, tag="pexp")
                    nx.tensor.transpose(pt[:], wacc[:, tc_, bass.ts(oc, 128)],
                                        ident[:])
                    nx.vector.tensor_copy(wfm[:, oc, bass.ts(tc_, 128)], pt[:])
            o1 = ghp.tile([128, OCH, T], BF16, tag="o1")
            for mc in range(OCH):
                ps = pp512.tile([128, T], F32, tag="ps512")
                for kc in range(OCH):
                    nx.tensor.matmul(ps[:], ow1[:, kc, bass.ts(mc, 128)],
                                     wfm[:, kc, :],
                                     start=(kc == 0), stop=(kc == OCH - 1))
                nx.scalar.activation(o1[:, mc, :], ps[:], AF.Gelu,
                                     bias=ob1[:, mc:mc + 1])
            for tc_ in range(TCH):
                pf = pp256.tile([128, O], F32, tag="pexp")
                for kc in range(OCH):
                    nx.tensor.matmul(pf[:], o1[:, kc, bass.ts(tc_, 128)],
                                     ow2[:, kc, :],
                                     start=(kc == 0), stop=(kc == OCH - 1))
                if have_ob2:
                    nx.vector.tensor_tensor(pf[:], pf[:], ob2bc[:], op=ALU.add)
                osb = sm.tile([128, O], F32, tag="osb")
                nx.vector.tensor_copy(osb[:], pf[:])
                nx.sync.dma_start(out_ext[b, bass.ts(tc_, 128), :], osb[:])

    nc.compile()
    return nc


def prep_weights(inp):
    h = {}
    w1 = np.asarray(inp["router_w1"], np.float32)
    w1hi = w1.astype(_bf).astype(np.float32)
    h["w_w1hi"] = _to_bf(_feat_major(w1hi))
    h["w_w1lo"] = _to_bf(_feat_major(w1 - w1hi))
    h["w_rw2"] = _feat_major(np.asarray(inp["router_w2"], np.float32) * 0.5)
    h["w_rb1s"] = _bias_chunks(np.asarray(inp["router_b1"]) * RT2)
    aW1 = np.concatenate([inp["gW1"], inp["lW1"]], 0)
    h["w_aW1"] = _to_bf(np.stack([_feat_major(aW1[n]) for n in range(NE)], 2))
    ab1 = np.concatenate([inp["gb1"], inp["lb1"]], 0)
    h["w_ab1"] = np.ascontiguousarray(
        np.stack([_bias_chunks(ab1[n]) for n in range(NE)], 1))
    h["w_gW2"] = _to_bf(np.stack(
        [_feat_major(np.asarray(inp["gW2"])[n]) for n in range(NS)], 2))
    h["w_gb2"] = np.ascontiguousarray(
        np.stack([_bias_chunks(np.asarray(inp["gb2"])[n]) for n in range(NS)], 1))
    aWo = np.concatenate([inp["gW3"], inp["lW2"]], 0)
    h["w_aWo"] = _to_bf(np.stack([_feat_major(aWo[n]) for n in range(NE)], 2))
    h["w_tew"] = _to_bf(_feat_major(np.asarray(inp["te_w"], np.float32) / C))
    h["w_teb"] = _bias_chunks(inp["te_b"])
    wcw = np.zeros((896, S), np.float32)
    wcw[:768 + S] = np.asarray(inp["wc_w"], np.float32)
    h["w_wcw"] = _to_bf(_feat_major(wcw))
    wcb = np.zeros((128, 1), np.float32)
    wcb[:S, 0] = np.asarray(inp["wc_b"])
    h["w_wcb"] = wcb
    h["w_ow1"] = _to_bf(_feat_major(inp["out_w1"]))
    h["w_ob1"] = _bias_chunks(inp["out_b1"])
    h["w_ow2"] = _to_bf(_feat_major(inp["out_w2"]))
    h["w_ident"] = _to_bf(np.eye(128, dtype=np.float32))
    h["w_wmbc"] = _to_bf(np.broadcast_to(
        np.asarray(inp["weight_memory"], np.float32).reshape(S, 1), (S, BC)))
    if np.any(np.asarray(inp["router_b2"]) != 0):
        h["w_rb2bc"] = np.ascontiguousarray(np.broadcast_to(
            np.asarray(inp["router_b2"], np.float32), (128, NE)))
    bout = np.concatenate([inp["gb3"], inp["lb2"]], 0)
    if np.any(bout != 0):
        h["w_boutbc"] = np.ascontiguousarray(np.broadcast_to(
            np.asarray(bout, np.float32).reshape(1, NE * O), (128, NE * O)))
    if np.any(np.asarray(inp["out_b2"]) != 0):
        h["w_ob2bc"] = np.ascontiguousarray(np.broadcast_to(
            np.asarray(inp["out_b2"], np.float32), (128, O)))
    return h


def prep_x(xs):
    """xs [S,B,C,E] f32 -> per-core (xhi, xlo), each [NT, ECH, 128, T] bf16."""
    out = []
    for c in range(NCORES):
        blk = np.asarray(xs, np.float32)[:, c * BC:(c + 1) * BC]
        xt = blk.transpose(0, 1, 3, 2).reshape(NT, ECH, 128, T)
        hi = xt.astype(_bf)
        lo = (xt - hi.astype(np.float32)).astype(_bf)
        out.append((np.ascontiguousarray(hi), np.ascontiguousarray(lo)))
    return out


_CACHE = {}


def build_in_maps(inputs):
    host = prep_weights(inputs)
    key = "graph:" + ",".join(sorted(host))
    if key not in _CACHE:
        _CACHE[key] = build_graph(host)
    nc = _CACHE[key]
    xs_shards = prep_x(inputs["xs"])
    in_maps = []
    for c in range(NCORES):
        m = dict(host)
        m["xhi"], m["xlo"] = xs_shards[c]
        in_maps.append(m)
    return nc, in_maps


def assemble(results):
    outs = np.concatenate(
        [results[c]["out"][None] for c in range(NCORES)], 0).reshape(B, C, O)
    ent_sum = sum(float(results[c]["ent"][0, 0]) for c in range(NCORES))
    bl = np.float32(0.1 * ent_sum / (S * B * C))
    return outs, bl


def kernel(**inputs):
    nc, in_maps = build_in_maps(inputs)
    res = run_bass_kernel_spmd(nc, in_maps, core_ids=list(range(NCORES)))
    return assemble(res.results)


# revision 10
# speedup vs baseline: 1.1887x; 1.0062x over previous
"""AdaptiveScaleRoutingMoE block on 8 TRN2 NeuronCores.

Strategy: data-parallel over batch (B=32 -> 4 per core). All weights
replicated. Per (scale, batch) pair one 512-token tile, 12 tiles/core.

Precision: router L1 in split-bf16 (hi/lo, 3 cross terms, exact gelu via the
Erf LUT), router L2 in f32 (top-2 selection is rank-sensitive), experts bf16
with f32 accumulation. The top-2-of-6 local-expert gather is replaced by a
mask built from the second max; the weighted combine runs token-major via
scalar_tensor_tensor with per-partition router-weight columns. Balance-loss
entropy uses ln(sum_exp) - sum(p*logit) (log-softmax identity), partial sums
reduced on host.
"""
import numpy as np
import ml_dtypes
from contextlib import ExitStack

import concourse.bass as bass
import concourse.tile as tile
from concourse import bacc, mybir
from concourse.bass_utils import run_bass_kernel_spmd

F32 = mybir.dt.float32
BF16 = mybir.dt.bfloat16
AF = mybir.ActivationFunctionType
ALU = mybir.AluOpType
AX = mybir.AxisListType

S, B, C, E = 3, 32, 512, 256
H, O, NE, NS, NL = 512, 256, 8, 2, 6
NCORES = 8
BC = B // NCORES          # batches per core
NT = S * BC               # token tiles per core (one per (s,b)), each T tokens
T = C                     # 512 tokens per tile
TCH = T // 128            # 4 token chunks
ECH = E // 128            # 2
HCH = H // 128            # 4
OCH = O // 128            # 2
KCC = 896 // 128          # 7 concat chunks (771 padded to 896)
RT2 = float(1.0 / np.sqrt(2.0))

_bf = ml_dtypes.bfloat16


def _to_bf(a):
    return np.ascontiguousarray(np.asarray(a, np.float32).astype(_bf))


def _feat_major(w):
    """[K, M] weight -> SBUF lhsT layout [128, K/128, M]."""
    k, m = w.shape
    return np.ascontiguousarray(
        np.asarray(w, np.float32).reshape(k // 128, 128, m).transpose(1, 0, 2))


def _bias_chunks(b):
    """[F] bias -> [128, F/128] per-partition layout."""
    f = b.shape[0]
    return np.ascontiguousarray(np.asarray(b, np.float32).reshape(f // 128, 128).T)


def build_graph(host):
    """host: dict of prepped numpy weight arrays. Returns compiled nc."""
    nc = bacc.Bacc("TRN2", target_bir_lowering=False, debug=False,
                   num_devices=NCORES)

    dram = {}

    def din(name, shape, dt):
        dram[name] = nc.dram_tensor(name, list(shape), dt, kind="ExternalInput")
        return dram[name]

    din("xhi", [NT, ECH, 128, T], BF16)
    din("xlo", [NT, ECH, 128, T], BF16)
    for k, v in host.items():
        din(k, v.shape, BF16 if v.dtype == _bf else F32)

    out_ext = nc.dram_tensor("out", [BC, C, O], F32, kind="ExternalOutput")
    ent_ext = nc.dram_tensor("ent", [1, 1], F32, kind="ExternalOutput")
    sw_bounce = nc.dram_tensor("sw_bounce", [BC * S], F32)
    tr_bounce = nc.dram_tensor("tr_bounce", [S, BC], F32)
    mean_bounce = nc.dram_tensor("mean_bounce", [NT, O], F32)

    have_rb2 = "w_rb2bc" in host
    have_bout = "w_boutbc" in host
    have_ob2 = "w_ob2bc" in host

    with tile.TileContext(nc) as tc, ExitStack() as ctx:
        nx = nc
        wp = ctx.enter_context(tc.tile_pool(name="weights", bufs=1))
        xp = ctx.enter_context(tc.tile_pool(name="xstash", bufs=1))
        stash = ctx.enter_context(tc.tile_pool(name="stash", bufs=1))
        hwork = ctx.enter_context(tc.tile_pool(name="hwork", bufs=2))
        ewp = ctx.enter_context(tc.tile_pool(name="ewp", bufs=1))
        ghp = ctx.enter_context(tc.tile_pool(name="ghp", bufs=1))
        sm = ctx.enter_context(tc.tile_pool(name="small", bufs=3))
        acc = ctx.enter_context(tc.tile_pool(name="accp", bufs=2))
        pp512 = ctx.enter_context(tc.tile_pool(name="pp512", bufs=2, space="PSUM"))
        pp256 = ctx.enter_context(tc.tile_pool(name="pp256", bufs=2, space="PSUM"))
        pps = ctx.enter_context(tc.tile_pool(name="pps", bufs=2, space="PSUM"))

        def wt(name, dt=BF16):
            a = host[name]
            t_ = wp.tile(list(a.shape), dt, tag=name)
            nx.sync.dma_start(t_[:], dram[name][:])
            return t_

        # DMA priority order: what phase R needs first
        w1hi = wt("w_w1hi"); w1lo = wt("w_w1lo")
        rw2 = wt("w_rw2", F32)
        have_rb1 = "w_rb1s" in host
        have_ab1 = "w_ab1" in host
        have_gb2 = "w_gb2" in host
        rb1s = wt("w_rb1s", F32) if have_rb1 else None
        xhi = xp.tile([128, ECH, NT, T], BF16, tag="xhi")
        for i in range(NT):
            for ec in range(ECH):
                nx.sync.dma_start(xhi[:, ec, i, :], dram["xhi"][i, ec])
        aW1 = wt("w_aW1")
        ab1 = wt("w_ab1", F32) if have_ab1 else None
        gW2 = wt("w_gW2")
        gb2 = wt("w_gb2", F32) if have_gb2 else None
        aWo = wt("w_aWo")
        tew = wt("w_tew"); teb = wt("w_teb", F32)
        wcw = wt("w_wcw"); wcb = wt("w_wcb", F32)
        ow1 = wt("w_ow1"); ob1 = wt("w_ob1", F32)
        ow2 = wt("w_ow2")
        ident = wt("w_ident")
        rb2bc = wt("w_rb2bc", F32) if have_rb2 else None
        boutbc = wt("w_boutbc", F32) if have_bout else None
        ob2bc = wt("w_ob2bc", F32) if have_ob2 else None

        onesb = wp.tile([128, 1], BF16, tag="onesb")
        nx.vector.memset(onesb[:], 1.0)
        onesf = wp.tile([128, 1], F32, tag="onesf")
        nx.vector.memset(onesf[:], 1.0)

        logit_st = stash.tile([128, TCH, NT, NE], F32, tag="logit")
        wst = stash.tile([128, TCH, NT, NE], F32, tag="wst")
        ssum_st = stash.tile([128, TCH * NT], F32, tag="ssum_st")
        spl_st = stash.tile([128, TCH * NT], F32, tag="spl_st")
        pst = stash.tile([128, TCH, NT, O], BF16, tag="pst")
        mlin = stash.tile([1, NT, O], F32, tag="mlin")
        mfm = stash.tile([128, OCH, NT], BF16, tag="mfm")
        reprs = stash.tile([128, OCH, NT], BF16, tag="reprs")
        ccat = stash.tile([128, KCC, BC], BF16, tag="ccat")
        th_sb = stash.tile([S, BC], F32, tag="th")
        swb = stash.tile([128, BC * S], F32, tag="swb")

        # ---------------- Phase R: router L1 (Erf) + L2 (f32) --------------
        xlo_tiles = {}
        def fetch_xlo(i):
            t_ = hwork.tile([128, ECH, T], BF16, tag="xlo")
            for ec in range(ECH):
                nx.sync.dma_start(t_[:, ec, :], dram["xlo"][i, ec])
            xlo_tiles[i] = t_
        fetch_xlo(0)
        for i in range(NT):
            if i + 1 < NT:
                fetch_xlo(i + 1)
            xlo_t = xlo_tiles.pop(i)
            h1 = hwork.tile([128, HCH, T], F32, tag="h1")
            for mcp in range(HCH // 2):
                ps = pp512.tile([128, 2, T], F32, tag="ps512")
                for mh in range(2):
                    mc = mcp * 2 + mh
                    terms = [(w1hi, xhi[:, ec, i, :]) for ec in range(ECH)]
                    terms += [(w1lo, xhi[:, ec, i, :]) for ec in range(ECH)]
                    terms += [(w1hi, xlo_t[:, ec, :]) for ec in range(ECH)]
                    n_terms = len(terms)
                    for j, (lhs, r) in enumerate(terms):
                        ec = j % ECH
                        nx.tensor.matmul(
                            ps[:, mh, :], lhs[:, ec, bass.ts(mc, 128)], r,
                            start=(j == 0), stop=(j == n_terms - 1))
                erf = hwork.tile([128, 2, T], F32, tag="erf")
                # erf((z + b1)/sqrt(2)); z in psum; rb1s = b1/sqrt(2).
                if have_rb1:
                    for mh in range(2):
                        nx.scalar.activation(
                            erf[:, mh, :], ps[:, mh, :], AF.Erf,
                            bias=rb1s[:, mcp * 2 + mh:mcp * 2 + mh + 1],
                            scale=RT2)
                else:
                    nx.scalar.activation(erf[:], ps[:], AF.Erf, scale=RT2)
                # h1 = (erf + 1) * z = 2*gelu(z)  (0.5 folded into rw2)
                nx.vector.scalar_tensor_tensor(
                    h1[:, mcp * 2:mcp * 2 + 2, :], erf[:], 1.0, ps[:],
                    op0=ALU.add, op1=ALU.mult)
            for tc_ in range(TCH):
                pl = pps.tile([128, NE], F32, tag="ppsmall")
                for kc in range(HCH):
                    nx.tensor.matmul(
                        pl[:], h1[:, kc, bass.ts(tc_, 128)], rw2[:, kc, :],
                        start=(kc == 0), stop=(kc == HCH - 1))
                if have_rb2:
                    nx.vector.tensor_tensor(pl[:], pl[:], rb2bc[:, :NE],
                                            op=ALU.add)
                nx.vector.tensor_copy(logit_st[:, tc_, i, :], pl[:])

        tc.no_sync_barrier()
        # ---------------- Phase W: batched softmax + entropy + top-2 -------
        # All 48 chunks at once on [128, TCH, NT, *] views of the stashes.
        ssum3 = ssum_st[:].rearrange("p (a b) -> p a b", a=TCH)
        spl3 = spl_st[:].rearrange("p (a b) -> p a b", a=TCH)
        nx.scalar.activation(wst[:], logit_st[:], AF.Exp)
        nx.vector.reduce_sum(ssum3, wst[:], axis=AX.X)
        rcpa = sm.tile([128, TCH, NT, 1], F32, tag="rcpa")
        nx.vector.reciprocal(rcpa[:], ssum3)
        nx.vector.tensor_tensor(
            wst[:], wst[:], rcpa[:].broadcast_to([128, TCH, NT, NE]),
            op=ALU.mult)
        pl8a = sm.tile([128, TCH, NT, NE], F32, tag="pl8a")
        nx.vector.tensor_tensor(pl8a[:], wst[:], logit_st[:], op=ALU.mult)
        nx.vector.reduce_sum(spl3, pl8a[:], axis=AX.X)
        wl = wst[:, :, :, NS:NE]
        m1a = sm.tile([128, TCH, NT, 1], F32, tag="rcpa")
        nx.vector.reduce_max(m1a[:], wl, axis=AX.X)
        eqa = sm.tile([128, TCH, NT, NL], F32, tag="pl8a")
        nx.vector.tensor_tensor(
            eqa[:], wl, m1a[:].broadcast_to([128, TCH, NT, NL]),
            op=ALU.is_equal)
        w2a = sm.tile([128, TCH, NT, NL], F32, tag="w2a")
        nx.vector.scalar_tensor_tensor(
            w2a[:], eqa[:], -1e30, wl, op0=ALU.mult, op1=ALU.add)
        m2a = sm.tile([128, TCH, NT, 1], F32, tag="rcpa")
        nx.vector.reduce_max(m2a[:], w2a[:], axis=AX.X)
        sela = sm.tile([128, TCH, NT, NL], F32, tag="pl8a")
        nx.vector.tensor_tensor(
            sela[:], wl, m2a[:].broadcast_to([128, TCH, NT, NL]),
            op=ALU.is_ge)
        nx.vector.tensor_tensor(wl, wl, sela[:], op=ALU.mult)

        tc.no_sync_barrier()
        # ---------------- Phase E: experts (Gelu) + combine + means --------
        for i in range(NT):
            eh = ewp.tile([128, NE, HCH, T], BF16, tag="eh")
            for n in range(NE):
                for mcp in range(HCH // 2):
                    ps = pp512.tile([128, 2, T], F32, tag="ps512")
                    for mh in range(2):
                        mc = mcp * 2 + mh
                        for ec in range(ECH):
                            nx.tensor.matmul(
                                ps[:, mh, :], aW1[:, ec, n, bass.ts(mc, 128)],
                                xhi[:, ec, i, :],
                                start=(ec == 0), stop=(ec == ECH - 1))
                    if have_ab1:
                        for mh in range(2):
                            nx.scalar.activation(
                                eh[:, n, mcp * 2 + mh, :], ps[:, mh, :],
                                AF.Gelu,
                                bias=ab1[:, n, mcp * 2 + mh:mcp * 2 + mh + 1])
                    else:
                        nx.scalar.activation(
                            eh[:, n, mcp * 2:mcp * 2 + 2, :], ps[:], AF.Gelu)
            gh2 = ghp.tile([128, NS, HCH, T], BF16, tag="gh2")
            for n in range(NS):
                for mcp in range(HCH // 2):
                    ps = pp512.tile([128, 2, T], F32, tag="ps512")
                    for mh in range(2):
                        mc = mcp * 2 + mh
                        for kc in range(HCH):
                            nx.tensor.matmul(
                                ps[:, mh, :], gW2[:, kc, n, bass.ts(mc, 128)],
                                eh[:, n, kc, :],
                                start=(kc == 0), stop=(kc == HCH - 1))
                    if have_gb2:
                        for mh in range(2):
                            nx.scalar.activation(
                                gh2[:, n, mcp * 2 + mh, :], ps[:, mh, :],
                                AF.Gelu,
                                bias=gb2[:, n, mcp * 2 + mh:mcp * 2 + mh + 1])
                    else:
                        nx.scalar.activation(
                            gh2[:, n, mcp * 2:mcp * 2 + 2, :], ps[:], AF.Gelu)
            for tc_ in range(TCH):
                ac = acc.tile([128, O], F32, tag="acc")
                for n in range(NE):
                    src = gh2 if n < NS else eh
                    pn = pp256.tile([128, O], F32, tag="pexp")
                    for kc in range(HCH):
                        nx.tensor.matmul(
                            pn[:], src[:, n, kc, bass.ts(tc_, 128)],
                            aWo[:, kc, n, :],
                            start=(kc == 0), stop=(kc == HCH - 1))
                    if have_bout:
                        nx.vector.tensor_tensor(
                            pn[:], pn[:], boutbc[:, bass.ts(n, O)], op=ALU.add)
                    if n == 0:
                        nx.vector.tensor_scalar(
                            ac[:], pn[:], wst[:, tc_, i, 0:1], None,
                            op0=ALU.mult)
                    else:
                        nx.vector.scalar_tensor_tensor(
                            ac[:], pn[:], wst[:, tc_, i, n:n + 1], ac[:],
                            op0=ALU.mult, op1=ALU.add)
                nx.vector.tensor_copy(pst[:, tc_, i, :], ac[:])
            pm = pps.tile([1, O], F32, tag="ppsmall")
            for tc_ in range(TCH):
                nx.tensor.matmul(pm[:], onesb[:], pst[:, tc_, i, :],
                                 start=(tc_ == 0), stop=(tc_ == TCH - 1))
            nx.vector.tensor_copy(mlin[:, i, :], pm[:])
        # means -> feature-major [128, OCH, NT] via DRAM bounce
        nx.sync.dma_start(mean_bounce[:], mlin[0, :, :])
        mfm_f = acc.tile([128, OCH, NT], F32, tag="mfmf")
        for oc in range(OCH):
            nx.sync.dma_start(
                mfm_f[:, oc, :],
                mean_bounce[:, oc * 128:(oc + 1) * 128].transpose([1, 0]))
        nx.vector.tensor_copy(mfm[:], mfm_f[:])
        for mc in range(OCH):
            ps = pps.tile([128, NT], F32, tag="ppsmall")
            for kc in range(OCH):
                nx.tensor.matmul(ps[:], tew[:, kc, bass.ts(mc, 128)],
                                 mfm[:, kc, :],
                                 start=(kc == 0), stop=(kc == OCH - 1))
            nx.scalar.activation(reprs[:, mc, :], ps[:], AF.Gelu,
                                 bias=teb[:, mc:mc + 1])
        # concat [771 padded 896, BC]; rows s*256+o2 from reprs, 768+s from wm
        nx.vector.memset(ccat[:], 0.0)
        for s in range(S):
            for oc in range(OCH):
                nx.sync.dma_start(ccat[:, s * OCH + oc, :],
                                  reprs[:, oc, s * BC:(s + 1) * BC])
        nx.sync.dma_start(ccat[0:S, KCC - 1, :], dram["w_wmbc"][:])
        pw = pps.tile([S, BC], F32, tag="ppsmall")
        for kc in range(KCC):
            nx.tensor.matmul(pw[:], wcw[:, kc, :], ccat[:, kc, :],
                             start=(kc == 0), stop=(kc == KCC - 1))
        nx.scalar.activation(th_sb[:], pw[:], AF.Tanh, bias=wcb[0:S, 0:1])

        tc.no_sync_barrier()
        # ---------------- Phase G: softplus + scale softmax + entropy ------
        ee = sm.tile([S, BC], F32, tag="ee")
        nx.scalar.activation(ee[:], th_sb[:], AF.Exp)
        nx.vector.tensor_scalar(ee[:], ee[:], 1.0, None, op0=ALU.add)
        raw = sm.tile([S, BC], F32, tag="raw")
        nx.scalar.activation(raw[:], ee[:], AF.Ln)
        nx.sync.dma_start(tr_bounce[:], raw[:])
        rawt = sm.tile([BC, S], F32, tag="rawt")
        nx.sync.dma_start(rawt[:], tr_bounce.ap().transpose([1, 0]))
        ex = sm.tile([BC, S], F32, tag="ex")
        nx.scalar.activation(ex[:], rawt[:], AF.Exp)
        ssum2 = sm.tile([BC, 1], F32, tag="ssum2")
        nx.vector.reduce_sum(ssum2[:], ex[:], axis=AX.X)
        rcp2 = sm.tile([BC, 1], F32, tag="rcp2")
        nx.vector.reciprocal(rcp2[:], ssum2[:])
        swt = sm.tile([BC, S], F32, tag="swt")
        nx.vector.tensor_scalar(swt[:], ex[:], rcp2[:], None, op0=ALU.mult)
        nx.sync.dma_start(sw_bounce[:], swt[:])
        nx.sync.dma_start(swb[:], sw_bounce.ap().partition_broadcast(128))
        lns_all = sm.tile([128, TCH * NT], F32, tag="lns_all")
        nx.scalar.activation(lns_all[:], ssum_st[:], AF.Ln)
        ent_all = sm.tile([128, TCH * NT], F32, tag="ent_all")
        nx.vector.tensor_tensor(ent_all[:], lns_all[:], spl_st[:],
                                op=ALU.subtract)
        entv = sm.tile([128, 1], F32, tag="entv")
        nx.vector.reduce_sum(entv[:], ent_all[:], axis=AX.X)
        pe_ = pps.tile([1, 1], F32, tag="ppsmall")
        nx.tensor.matmul(pe_[:], onesf[:], entv[:], start=True, stop=True)
        esb = sm.tile([1, 1], F32, tag="esb")
        nx.vector.tensor_copy(esb[:], pe_[:])
        nx.sync.dma_start(ent_ext[:], esb[:])

        tc.no_sync_barrier()
        # ---------------- Phase OUT: scale-weighted sum + output MLP -------
        for b in range(BC):
            wacc = acc.tile([128, TCH, O], BF16, tag="wacc")
            for tc_ in range(TCH):
                tmp = sm.tile([128, O], F32, tag="wtmp")
                nx.vector.tensor_scalar(
                    tmp[:], pst[:, tc_, 0 * BC + b, :],
                    swb[:, b * S:b * S + 1], None, op0=ALU.mult)
                nx.vector.scalar_tensor_tensor(
                    tmp[:], pst[:, tc_, 1 * BC + b, :],
                    swb[:, b * S + 1:b * S + 2], tmp[:],
                    op0=ALU.mult, op1=ALU.add)
                nx.vector.scalar_tensor_tensor(
                    wacc[:, tc_, :], pst[:, tc_, 2 * BC + b, :],
                    swb[:, b * S + 2:b * S + 3], tmp[:],
                    op0=ALU.mult, op1=ALU.add)
            wfm = ghp.tile([128, OCH, T], BF16, tag="wfm")
            for tc_ in range(TCH):
                for oc in range(OCH):
                    pt = pp256.tile([128, 128], BF16, tag="pexp")
                    nx.tensor.transpose(pt[:], wacc[:, tc_, bass.ts(oc, 128)],
                                        ident[:])
                    nx.vector.tensor_copy(wfm[:, oc, bass.ts(tc_, 128)], pt[:])
            o1 = ghp.tile([128, OCH, T], BF16, tag="o1")
            for mc in range(OCH):
                ps = pp512.tile([128, T], F32, tag="ps512")
                for kc in range(OCH):
                    nx.tensor.matmul(ps[:], ow1[:, kc, bass.ts(mc, 128)],
                                     wfm[:, kc, :],
                                     start=(kc == 0), stop=(kc == OCH - 1))
                nx.scalar.activation(o1[:, mc, :], ps[:], AF.Gelu,
                                     bias=ob1[:, mc:mc + 1])
            for tc_ in range(TCH):
                pf = pp256.tile([128, O], F32, tag="pexp")
                for kc in range(OCH):
                    nx.tensor.matmul(pf[:], o1[:, kc, bass.ts(tc_, 128)],
                                     ow2[:, kc, :],
                                     start=(kc == 0), stop=(kc == OCH - 1))
                if have_ob2:
                    nx.vector.tensor_tensor(pf[:], pf[:], ob2bc[:], op=ALU.add)
                osb = sm.tile([128, O], F32, tag="osb")
                nx.vector.tensor_copy(osb[:], pf[:])
                nx.sync.dma_start(out_ext[b, bass.ts(tc_, 128), :], osb[:])

    nc.compile()
    return nc


def prep_weights(inp):
    h = {}
    w1 = np.asarray(inp["router_w1"], np.float32)
    w1hi = w1.astype(_bf).astype(np.float32)
    h["w_w1hi"] = _to_bf(_feat_major(w1hi))
    h["w_w1lo"] = _to_bf(_feat_major(w1 - w1hi))
    h["w_rw2"] = _feat_major(np.asarray(inp["router_w2"], np.float32) * 0.5)
    if np.any(np.asarray(inp["router_b1"]) != 0):
        h["w_rb1s"] = _bias_chunks(np.asarray(inp["router_b1"]) * RT2)
    aW1 = np.concatenate([inp["gW1"], inp["lW1"]], 0)
    h["w_aW1"] = _to_bf(np.stack([_feat_major(aW1[n]) for n in range(NE)], 2))
    ab1 = np.concatenate([inp["gb1"], inp["lb1"]], 0)
    if np.any(ab1 != 0):
        h["w_ab1"] = np.ascontiguousarray(
            np.stack([_bias_chunks(ab1[n]) for n in range(NE)], 1))
    h["w_gW2"] = _to_bf(np.stack(
        [_feat_major(np.asarray(inp["gW2"])[n]) for n in range(NS)], 2))
    if np.any(np.asarray(inp["gb2"]) != 0):
        h["w_gb2"] = np.ascontiguousarray(np.stack(
            [_bias_chunks(np.asarray(inp["gb2"])[n]) for n in range(NS)], 1))
    aWo = np.concatenate([inp["gW3"], inp["lW2"]], 0)
    h["w_aWo"] = _to_bf(np.stack([_feat_major(aWo[n]) for n in range(NE)], 2))
    h["w_tew"] = _to_bf(_feat_major(np.asarray(inp["te_w"], np.float32) / C))
    h["w_teb"] = _bias_chunks(inp["te_b"])
    wcw = np.zeros((896, S), np.float32)
    wcw[:768 + S] = np.asarray(inp["wc_w"], np.float32)
    h["w_wcw"] = _to_bf(_feat_major(wcw))
    wcb = np.zeros((128, 1), np.float32)
    wcb[:S, 0] = np.asarray(inp["wc_b"])
    h["w_wcb"] = wcb
    h["w_ow1"] = _to_bf(_feat_major(inp["out_w1"]))
    h["w_ob1"] = _bias_chunks(inp["out_b1"])
    h["w_ow2"] = _to_bf(_feat_major(inp["out_w2"]))
    h["w_ident"] = _to_bf(np.eye(128, dtype=np.float32))
    h["w_wmbc"] = _to_bf(np.broadcast_to(
        np.asarray(inp["weight_memory"], np.float32).reshape(S, 1), (S, BC)))
    if np.any(np.asarray(inp["router_b2"]) != 0):
        h["w_rb2bc"] = np.ascontiguousarray(np.broadcast_to(
            np.asarray(inp["router_b2"], np.float32), (128, NE)))
    bout = np.concatenate([inp["gb3"], inp["lb2"]], 0)
    if np.any(bout != 0):
        h["w_boutbc"] = np.ascontiguousarray(np.broadcast_to(
            np.asarray(bout, np.float32).reshape(1, NE * O), (128, NE * O)))
    if np.any(np.asarray(inp["out_b2"]) != 0):
        h["w_ob2bc"] = np.ascontiguousarray(np.broadcast_to(
            np.asarray(inp["out_b2"], np.float32), (128, O)))
    return h


def prep_x(xs):
    """xs [S,B,C,E] f32 -> per-core (xhi, xlo), each [NT, ECH, 128, T] bf16."""
    out = []
    for c in range(NCORES):
        blk = np.asarray(xs, np.float32)[:, c * BC:(c + 1) * BC]
        xt = blk.transpose(0, 1, 3, 2).reshape(NT, ECH, 128, T)
        hi = xt.astype(_bf)
        lo = (xt - hi.astype(np.float32)).astype(_bf)
        out.append((np.ascontiguousarray(hi), np.ascontiguousarray(lo)))
    return out


_CACHE = {}


def build_in_maps(inputs):
    host = prep_weights(inputs)
    key = "graph:" + ",".join(sorted(host))
    if key not in _CACHE:
        _CACHE[key] = build_graph(host)
    nc = _CACHE[key]
    xs_shards = prep_x(inputs["xs"])
    in_maps = []
    for c in range(NCORES):
        m = dict(host)
        m["xhi"], m["xlo"] = xs_shards[c]
        in_maps.append(m)
    return nc, in_maps


def assemble(results):
    outs = np.concatenate(
        [results[c]["out"][None] for c in range(NCORES)], 0).reshape(B, C, O)
    ent_sum = sum(float(results[c]["ent"][0, 0]) for c in range(NCORES))
    bl = np.float32(0.1 * ent_sum / (S * B * C))
    return outs, bl


def kernel(**inputs):
    nc, in_maps = build_in_maps(inputs)
    res = run_bass_kernel_spmd(nc, in_maps, core_ids=list(range(NCORES)))
    return assemble(res.results)


# revision 12
# speedup vs baseline: 1.2499x; 1.0514x over previous
"""AdaptiveScaleRoutingMoE block on 8 TRN2 NeuronCores.

Strategy: data-parallel over batch (B=32 -> 4 per core). All weights
replicated. Per (scale, batch) pair one 512-token tile, 12 tiles/core.

Precision: router L1 in split-bf16 (hi/lo, 3 cross terms, exact gelu via the
Erf LUT), router L2 in f32 (top-2 selection is rank-sensitive), experts bf16
with f32 accumulation. The top-2-of-6 local-expert gather is replaced by a
mask built from the second max; the weighted combine runs token-major via
scalar_tensor_tensor with per-partition router-weight columns. Balance-loss
entropy uses ln(sum_exp) - sum(p*logit) (log-softmax identity), partial sums
reduced on host.
"""
import numpy as np
import ml_dtypes
from contextlib import ExitStack

import concourse.bass as bass
import concourse.tile as tile
from concourse import bacc, mybir
from concourse.bass_utils import run_bass_kernel_spmd

F32 = mybir.dt.float32
BF16 = mybir.dt.bfloat16
AF = mybir.ActivationFunctionType
ALU = mybir.AluOpType
AX = mybir.AxisListType

S, B, C, E = 3, 32, 512, 256
H, O, NE, NS, NL = 512, 256, 8, 2, 6
NCORES = 8
BC = B // NCORES          # batches per core
NT = S * BC               # token tiles per core (one per (s,b)), each T tokens
T = C                     # 512 tokens per tile
TCH = T // 128            # 4 token chunks
ECH = E // 128            # 2
HCH = H // 128            # 4
OCH = O // 128            # 2
KCC = 896 // 128          # 7 concat chunks (771 padded to 896)
RT2 = float(1.0 / np.sqrt(2.0))

_bf = ml_dtypes.bfloat16


def _to_bf(a):
    return np.ascontiguousarray(np.asarray(a, np.float32).astype(_bf))


def _feat_major(w):
    """[K, M] weight -> SBUF lhsT layout [128, K/128, M]."""
    k, m = w.shape
    return np.ascontiguousarray(
        np.asarray(w, np.float32).reshape(k // 128, 128, m).transpose(1, 0, 2))


def _bias_chunks(b):
    """[F] bias -> [128, F/128] per-partition layout."""
    f = b.shape[0]
    return np.ascontiguousarray(np.asarray(b, np.float32).reshape(f // 128, 128).T)


def build_graph(host):
    """host: dict of prepped numpy weight arrays. Returns compiled nc."""
    nc = bacc.Bacc("TRN2", target_bir_lowering=False, debug=False,
                   num_devices=NCORES)

    dram = {}

    def din(name, shape, dt):
        dram[name] = nc.dram_tensor(name, list(shape), dt, kind="ExternalInput")
        return dram[name]

    din("xhi", [NT, ECH, 128, T], BF16)
    din("xlo", [NT, ECH, 128, T], BF16)
    for k, v in host.items():
        din(k, v.shape, BF16 if v.dtype == _bf else F32)

    out_ext = nc.dram_tensor("out", [BC, C, O], F32, kind="ExternalOutput")
    ent_ext = nc.dram_tensor("ent", [1, 1], F32, kind="ExternalOutput")
    sw_bounce = nc.dram_tensor("sw_bounce", [BC * S], F32)
    tr_bounce = nc.dram_tensor("tr_bounce", [S, BC], F32)
    mean_bounce = nc.dram_tensor("mean_bounce", [NT, O], F32)

    have_rb2 = "w_rb2bc" in host
    have_bout = "w_boutbc" in host
    have_ob2 = "w_ob2bc" in host

    with tile.TileContext(nc) as tc, ExitStack() as ctx:
        nx = nc
        wp = ctx.enter_context(tc.tile_pool(name="weights", bufs=1))
        xp = ctx.enter_context(tc.tile_pool(name="xstash", bufs=1))
        stash = ctx.enter_context(tc.tile_pool(name="stash", bufs=1))
        hwork = ctx.enter_context(tc.tile_pool(name="hwork", bufs=2))
        ewp = ctx.enter_context(tc.tile_pool(name="ewp", bufs=1))
        ghp = ctx.enter_context(tc.tile_pool(name="ghp", bufs=1))
        sm = ctx.enter_context(tc.tile_pool(name="small", bufs=3))
        acc = ctx.enter_context(tc.tile_pool(name="accp", bufs=2))
        pp512 = ctx.enter_context(tc.tile_pool(name="pp512", bufs=2, space="PSUM"))
        pp256 = ctx.enter_context(tc.tile_pool(name="pp256", bufs=3, space="PSUM"))
        pps = ctx.enter_context(tc.tile_pool(name="pps", bufs=1, space="PSUM"))

        def wt(name, dt=BF16):
            a = host[name]
            t_ = wp.tile(list(a.shape), dt, tag=name)
            nx.sync.dma_start(t_[:], dram[name][:])
            return t_

        # DMA priority order: what phase R needs first
        w1hi = wt("w_w1hi"); w1lo = wt("w_w1lo")
        rw2 = wt("w_rw2", F32)
        have_rb1 = "w_rb1s" in host
        have_ab1 = "w_ab1" in host
        have_gb2 = "w_gb2" in host
        rb1s = wt("w_rb1s", F32) if have_rb1 else None
        rb2bc = wt("w_rb2bc", F32) if have_rb2 else None
        xhi = xp.tile([128, ECH, NT, T], BF16, tag="xhi")
        for i in range(NT):
            for ec in range(ECH):
                nx.sync.dma_start(xhi[:, ec, i, :], dram["xhi"][i, ec])
        onesb = wp.tile([128, 1], BF16, tag="onesb")
        nx.vector.memset(onesb[:], 1.0)
        onesf = wp.tile([128, 1], F32, tag="onesf")
        nx.vector.memset(onesf[:], 1.0)

        logit_st = stash.tile([128, TCH, NT, NE], F32, tag="logit")
        wst = stash.tile([128, TCH, NT, NE], F32, tag="wst")
        ssum_st = stash.tile([128, TCH * NT], F32, tag="ssum_st")
        spl_st = stash.tile([128, TCH * NT], F32, tag="spl_st")
        pst = stash.tile([128, TCH, NT, O], BF16, tag="pst")
        mlin = stash.tile([1, NT, O], F32, tag="mlin")
        mfm = stash.tile([128, OCH, NT], BF16, tag="mfm")
        reprs = stash.tile([128, OCH, NT], BF16, tag="reprs")
        ccat = stash.tile([128, KCC, BC], BF16, tag="ccat")
        th_sb = stash.tile([S, BC], F32, tag="th")
        swb = stash.tile([128, BC * S], F32, tag="swb")

        # ---------------- Phase R: router L1 (Erf) + L2 (f32) --------------
        xlo_tiles = {}
        def fetch_xlo(i):
            t_ = hwork.tile([128, ECH, T], BF16, tag="xlo")
            for ec in range(ECH):
                nx.sync.dma_start(t_[:, ec, :], dram["xlo"][i, ec])
            xlo_tiles[i] = t_
        fetch_xlo(0)
        for i in range(NT):
            if i + 1 < NT:
                fetch_xlo(i + 1)
            xlo_t = xlo_tiles.pop(i)
            h1 = hwork.tile([128, HCH, T], F32, tag="h1")
            for mcp in range(HCH // 2):
                ps = pp512.tile([128, 2, T], F32, tag="ps512")
                for mh in range(2):
                    mc = mcp * 2 + mh
                    terms = [(w1hi, xhi[:, ec, i, :]) for ec in range(ECH)]
                    terms += [(w1lo, xhi[:, ec, i, :]) for ec in range(ECH)]
                    terms += [(w1hi, xlo_t[:, ec, :]) for ec in range(ECH)]
                    n_terms = len(terms)
                    for j, (lhs, r) in enumerate(terms):
                        ec = j % ECH
                        nx.tensor.matmul(
                            ps[:, mh, :], lhs[:, ec, bass.ts(mc, 128)], r,
                            start=(j == 0), stop=(j == n_terms - 1))
                erf = hwork.tile([128, 2, T], F32, tag="erf")
                # erf((z + b1)/sqrt(2)); z in psum; rb1s = b1/sqrt(2).
                if have_rb1:
                    for mh in range(2):
                        nx.scalar.activation(
                            erf[:, mh, :], ps[:, mh, :], AF.Erf,
                            bias=rb1s[:, mcp * 2 + mh:mcp * 2 + mh + 1],
                            scale=RT2)
                else:
                    nx.scalar.activation(erf[:], ps[:], AF.Erf, scale=RT2)
                # h1 = (erf + 1) * z = 2*gelu(z)  (0.5 folded into rw2)
                nx.vector.scalar_tensor_tensor(
                    h1[:, mcp * 2:mcp * 2 + 2, :], erf[:], 1.0, ps[:],
                    op0=ALU.add, op1=ALU.mult)
            for tc_ in range(TCH):
                pl = pps.tile([128, NE], F32, tag="ppsmall")
                for kc in range(HCH):
                    nx.tensor.matmul(
                        pl[:], h1[:, kc, bass.ts(tc_, 128)], rw2[:, kc, :],
                        start=(kc == 0), stop=(kc == HCH - 1))
                if have_rb2:
                    nx.vector.tensor_tensor(pl[:], pl[:], rb2bc[:, :NE],
                                            op=ALU.add)
                nx.vector.tensor_copy(logit_st[:, tc_, i, :], pl[:])

        # bulk weights: issued after R's x DMAs so they don't starve phase R
        aW1 = wt("w_aW1")
        ab1 = wt("w_ab1", F32) if have_ab1 else None
        gW2 = wt("w_gW2")
        gb2 = wt("w_gb2", F32) if have_gb2 else None
        aWo = wt("w_aWo")
        tew = wt("w_tew"); teb = wt("w_teb", F32)
        wcw = wt("w_wcw"); wcb = wt("w_wcb", F32)
        ow1 = wt("w_ow1"); ob1 = wt("w_ob1", F32)
        ow2 = wt("w_ow2")
        ident = wt("w_ident")
        boutbc = wt("w_boutbc", F32) if have_bout else None
        ob2bc = wt("w_ob2bc", F32) if have_ob2 else None

        tc.no_sync_barrier()
        # ---------------- Phase W: batched softmax + entropy + top-2 -------
        # All 48 chunks at once on [128, TCH, NT, *] views of the stashes.
        ssum3 = ssum_st[:].rearrange("p (a b) -> p a b", a=TCH)
        spl3 = spl_st[:].rearrange("p (a b) -> p a b", a=TCH)
        nx.scalar.activation(wst[:], logit_st[:], AF.Exp)
        nx.vector.reduce_sum(ssum3, wst[:], axis=AX.X)
        rcpa = sm.tile([128, TCH, NT, 1], F32, tag="rcpa")
        nx.vector.reciprocal(rcpa[:], ssum3)
        nx.vector.tensor_tensor(
            wst[:], wst[:], rcpa[:].broadcast_to([128, TCH, NT, NE]),
            op=ALU.mult)
        pl8a = sm.tile([128, TCH, NT, NE], F32, tag="pl8a")
        nx.vector.tensor_tensor(pl8a[:], wst[:], logit_st[:], op=ALU.mult)
        nx.vector.reduce_sum(spl3, pl8a[:], axis=AX.X)
        wl = wst[:, :, :, NS:NE]
        m1a = sm.tile([128, TCH, NT, 1], F32, tag="rcpa")
        nx.vector.reduce_max(m1a[:], wl, axis=AX.X)
        eqa = sm.tile([128, TCH, NT, NL], F32, tag="pl8a")
        nx.vector.tensor_tensor(
            eqa[:], wl, m1a[:].broadcast_to([128, TCH, NT, NL]),
            op=ALU.is_equal)
        w2a = sm.tile([128, TCH, NT, NL], F32, tag="w2a")
        nx.vector.scalar_tensor_tensor(
            w2a[:], eqa[:], -1e30, wl, op0=ALU.mult, op1=ALU.add)
        m2a = sm.tile([128, TCH, NT, 1], F32, tag="rcpa")
        nx.vector.reduce_max(m2a[:], w2a[:], axis=AX.X)
        sela = sm.tile([128, TCH, NT, NL], F32, tag="pl8a")
        nx.vector.tensor_tensor(
            sela[:], wl, m2a[:].broadcast_to([128, TCH, NT, NL]),
            op=ALU.is_ge)
        nx.vector.tensor_tensor(wl, wl, sela[:], op=ALU.mult)

        tc.no_sync_barrier()
        # ---------------- Phase E: experts (Gelu) + combine + means --------
        for i in range(NT):
            eh = ewp.tile([128, NE, HCH, T], BF16, tag="eh")
            for n in range(NE):
                for mcp in range(HCH // 2):
                    ps = pp512.tile([128, 2, T], F32, tag="ps512")
                    for mh in range(2):
                        mc = mcp * 2 + mh
                        for ec in range(ECH):
                            nx.tensor.matmul(
                                ps[:, mh, :], aW1[:, ec, n, bass.ts(mc, 128)],
                                xhi[:, ec, i, :],
                                start=(ec == 0), stop=(ec == ECH - 1))
                    if have_ab1:
                        for mh in range(2):
                            nx.scalar.activation(
                                eh[:, n, mcp * 2 + mh, :], ps[:, mh, :],
                                AF.Gelu,
                                bias=ab1[:, n, mcp * 2 + mh:mcp * 2 + mh + 1])
                    else:
                        nx.scalar.activation(
                            eh[:, n, mcp * 2:mcp * 2 + 2, :], ps[:], AF.Gelu)
            gh2 = ghp.tile([128, NS, HCH, T], BF16, tag="gh2")
            for n in range(NS):
                for mcp in range(HCH // 2):
                    ps = pp512.tile([128, 2, T], F32, tag="ps512")
                    for mh in range(2):
                        mc = mcp * 2 + mh
                        for kc in range(HCH):
                            nx.tensor.matmul(
                                ps[:, mh, :], gW2[:, kc, n, bass.ts(mc, 128)],
                                eh[:, n, kc, :],
                                start=(kc == 0), stop=(kc == HCH - 1))
                    if have_gb2:
                        for mh in range(2):
                            nx.scalar.activation(
                                gh2[:, n, mcp * 2 + mh, :], ps[:, mh, :],
                                AF.Gelu,
                                bias=gb2[:, n, mcp * 2 + mh:mcp * 2 + mh + 1])
                    else:
                        nx.scalar.activation(
                            gh2[:, n, mcp * 2:mcp * 2 + 2, :], ps[:], AF.Gelu)
            for tc_ in range(TCH):
                ac = acc.tile([128, O], F32, tag="acc")
                for n in range(NE):
                    src = gh2 if n < NS else eh
                    pn = pp256.tile([128, O], F32, tag="pexp")
                    for kc in range(HCH):
                        nx.tensor.matmul(
                            pn[:], src[:, n, kc, bass.ts(tc_, 128)],
                            aWo[:, kc, n, :],
                            start=(kc == 0), stop=(kc == HCH - 1))
                    if have_bout:
                        nx.vector.tensor_tensor(
                            pn[:], pn[:], boutbc[:, bass.ts(n, O)], op=ALU.add)
                    if n == 0:
                        nx.vector.tensor_scalar(
                            ac[:], pn[:], wst[:, tc_, i, 0:1], None,
                            op0=ALU.mult)
                    else:
                        nx.vector.scalar_tensor_tensor(
                            ac[:], pn[:], wst[:, tc_, i, n:n + 1], ac[:],
                            op0=ALU.mult, op1=ALU.add)
                nx.vector.tensor_copy(pst[:, tc_, i, :], ac[:])
            pm = pps.tile([1, O], F32, tag="ppsmall")
            for tc_ in range(TCH):
                nx.tensor.matmul(pm[:], onesb[:], pst[:, tc_, i, :],
                                 start=(tc_ == 0), stop=(tc_ == TCH - 1))
            nx.vector.tensor_copy(mlin[:, i, :], pm[:])
        # means -> feature-major [128, OCH, NT] via DRAM bounce
        nx.sync.dma_start(mean_bounce[:], mlin[0, :, :])
        mfm_f = acc.tile([128, OCH, NT], F32, tag="mfmf")
        for oc in range(OCH):
            nx.sync.dma_start(
                mfm_f[:, oc, :],
                mean_bounce[:, oc * 128:(oc + 1) * 128].transpose([1, 0]))
        nx.vector.tensor_copy(mfm[:], mfm_f[:])
        for mc in range(OCH):
            ps = pps.tile([128, NT], F32, tag="ppsmall")
            for kc in range(OCH):
                nx.tensor.matmul(ps[:], tew[:, kc, bass.ts(mc, 128)],
                                 mfm[:, kc, :],
                                 start=(kc == 0), stop=(kc == OCH - 1))
            nx.scalar.activation(reprs[:, mc, :], ps[:], AF.Gelu,
                                 bias=teb[:, mc:mc + 1])
        # concat [771 padded 896, BC]; rows s*256+o2 from reprs, 768+s from wm
        nx.vector.memset(ccat[:], 0.0)
        for s in range(S):
            for oc in range(OCH):
                nx.sync.dma_start(ccat[:, s * OCH + oc, :],
                                  reprs[:, oc, s * BC:(s + 1) * BC])
        nx.sync.dma_start(ccat[0:S, KCC - 1, :], dram["w_wmbc"][:])
        pw = pps.tile([S, BC], F32, tag="ppsmall")
        for kc in range(KCC):
            nx.tensor.matmul(pw[:], wcw[:, kc, :], ccat[:, kc, :],
                             start=(kc == 0), stop=(kc == KCC - 1))
        nx.scalar.activation(th_sb[:], pw[:], AF.Tanh, bias=wcb[0:S, 0:1])

        tc.no_sync_barrier()
        # ---------------- Phase G: softplus + scale softmax + entropy ------
        ee = sm.tile([S, BC], F32, tag="ee")
        nx.scalar.activation(ee[:], th_sb[:], AF.Exp)
        nx.vector.tensor_scalar(ee[:], ee[:], 1.0, None, op0=ALU.add)
        raw = sm.tile([S, BC], F32, tag="raw")
        nx.scalar.activation(raw[:], ee[:], AF.Ln)
        nx.sync.dma_start(tr_bounce[:], raw[:])
        rawt = sm.tile([BC, S], F32, tag="rawt")
        nx.sync.dma_start(rawt[:], tr_bounce.ap().transpose([1, 0]))
        ex = sm.tile([BC, S], F32, tag="ex")
        nx.scalar.activation(ex[:], rawt[:], AF.Exp)
        ssum2 = sm.tile([BC, 1], F32, tag="ssum2")
        nx.vector.reduce_sum(ssum2[:], ex[:], axis=AX.X)
        rcp2 = sm.tile([BC, 1], F32, tag="rcp2")
        nx.vector.reciprocal(rcp2[:], ssum2[:])
        swt = sm.tile([BC, S], F32, tag="swt")
        nx.vector.tensor_scalar(swt[:], ex[:], rcp2[:], None, op0=ALU.mult)
        nx.sync.dma_start(sw_bounce[:], swt[:])
        nx.sync.dma_start(swb[:], sw_bounce.ap().partition_broadcast(128))
        lns_all = sm.tile([128, TCH * NT], F32, tag="lns_all")
        nx.scalar.activation(lns_all[:], ssum_st[:], AF.Ln)
        ent_all = sm.tile([128, TCH * NT], F32, tag="ent_all")
        nx.vector.tensor_tensor(ent_all[:], lns_all[:], spl_st[:],
                                op=ALU.subtract)
        entv = sm.tile([128, 1], F32, tag="entv")
        nx.vector.reduce_sum(entv[:], ent_all[:], axis=AX.X)
        pe_ = pps.tile([1, 1], F32, tag="ppsmall")
        nx.tensor.matmul(pe_[:], onesf[:], entv[:], start=True, stop=True)
        esb = sm.tile([1, 1], F32, tag="esb")
        nx.vector.tensor_copy(esb[:], pe_[:])
        nx.sync.dma_start(ent_ext[:], esb[:])

        tc.no_sync_barrier()
        # ---------------- Phase OUT: scale-weighted sum + output MLP -------
        for b in range(BC):
            wacc = acc.tile([128, TCH, O], BF16, tag="wacc")
            for tc_ in range(TCH):
                tmp = sm.tile([128, O], F32, tag="wtmp")
                nx.vector.tensor_scalar(
                    tmp[:], pst[:, tc_, 0 * BC + b, :],
                    swb[:, b * S:b * S + 1], None, op0=ALU.mult)
                nx.vector.scalar_tensor_tensor(
                    tmp[:], pst[:, tc_, 1 * BC + b, :],
                    swb[:, b * S + 1:b * S + 2], tmp[:],
                    op0=ALU.mult, op1=ALU.add)
                nx.vector.scalar_tensor_tensor(
                    wacc[:, tc_, :], pst[:, tc_, 2 * BC + b, :],
                    swb[:, b * S + 2:b * S + 3], tmp[:],
                    op0=ALU.mult, op1=ALU.add)
            wfm = ghp.tile([128, OCH, T], BF16, tag="wfm")
            for tc_ in range(TCH):
                for oc in range(OCH):
                    pt = pp256.tile([128, 128], BF16, tag="pexp")
                    nx.tensor.transpose(pt[:], wacc[:, tc_, bass.ts(oc, 128)],
                                        ident[:])
                    nx.vector.tensor_copy(wfm[:, oc, bass.ts(tc_, 128)], pt[:])
            o1 = ghp.tile([128, OCH, T], BF16, tag="o1")
            for mc in range(OCH):
                ps = pp512.tile([128, T], F32, tag="ps512")
                for kc in range(OCH):
                    nx.tensor.matmul(ps[:], ow1[:, kc, bass.ts(mc, 128)],
                                     wfm[:, kc, :],
                                     start=(kc == 0), stop=(kc == OCH - 1))
                nx.scalar.activation(o1[:, mc, :], ps[:], AF.Gelu,
                                     bias=ob1[:, mc:mc + 1])
            for tc_ in range(TCH):
                pf = pp256.tile([128, O], F32, tag="pexp")
                for kc in range(OCH):
                    nx.tensor.matmul(pf[:], o1[:, kc, bass.ts(tc_, 128)],
                                     ow2[:, kc, :],
                                     start=(kc == 0), stop=(kc == OCH - 1))
                if have_ob2:
                    nx.vector.tensor_tensor(pf[:], pf[:], ob2bc[:], op=ALU.add)
                osb = sm.tile([128, O], F32, tag="osb")
                nx.vector.tensor_copy(osb[:], pf[:])
                nx.sync.dma_start(out_ext[b, bass.ts(tc_, 128), :], osb[:])

    nc.compile()
    return nc


def prep_weights(inp):
    h = {}
    w1 = np.asarray(inp["router_w1"], np.float32)
    w1hi = w1.astype(_bf).astype(np.float32)
    h["w_w1hi"] = _to_bf(_feat_major(w1hi))
    h["w_w1lo"] = _to_bf(_feat_major(w1 - w1hi))
    h["w_rw2"] = _feat_major(np.asarray(inp["router_w2"], np.float32) * 0.5)
    if np.any(np.asarray(inp["router_b1"]) != 0):
        h["w_rb1s"] = _bias_chunks(np.asarray(inp["router_b1"]) * RT2)
    aW1 = np.concatenate([inp["gW1"], inp["lW1"]], 0)
    h["w_aW1"] = _to_bf(np.stack([_feat_major(aW1[n]) for n in range(NE)], 2))
    ab1 = np.concatenate([inp["gb1"], inp["lb1"]], 0)
    if np.any(ab1 != 0):
        h["w_ab1"] = np.ascontiguousarray(
            np.stack([_bias_chunks(ab1[n]) for n in range(NE)], 1))
    h["w_gW2"] = _to_bf(np.stack(
        [_feat_major(np.asarray(inp["gW2"])[n]) for n in range(NS)], 2))
    if np.any(np.asarray(inp["gb2"]) != 0):
        h["w_gb2"] = np.ascontiguousarray(np.stack(
            [_bias_chunks(np.asarray(inp["gb2"])[n]) for n in range(NS)], 1))
    aWo = np.concatenate([inp["gW3"], inp["lW2"]], 0)
    h["w_aWo"] = _to_bf(np.stack([_feat_major(aWo[n]) for n in range(NE)], 2))
    h["w_tew"] = _to_bf(_feat_major(np.asarray(inp["te_w"], np.float32) / C))
    h["w_teb"] = _bias_chunks(inp["te_b"])
    wcw = np.zeros((896, S), np.float32)
    wcw[:768 + S] = np.asarray(inp["wc_w"], np.float32)
    h["w_wcw"] = _to_bf(_feat_major(wcw))
    wcb = np.zeros((128, 1), np.float32)
    wcb[:S, 0] = np.asarray(inp["wc_b"])
    h["w_wcb"] = wcb
    h["w_ow1"] = _to_bf(_feat_major(inp["out_w1"]))
    h["w_ob1"] = _bias_chunks(inp["out_b1"])
    h["w_ow2"] = _to_bf(_feat_major(inp["out_w2"]))
    h["w_ident"] = _to_bf(np.eye(128, dtype=np.float32))
    h["w_wmbc"] = _to_bf(np.broadcast_to(
        np.asarray(inp["weight_memory"], np.float32).reshape(S, 1), (S, BC)))
    if np.any(np.asarray(inp["router_b2"]) != 0):
        h["w_rb2bc"] = np.ascontiguousarray(np.broadcast_to(
            np.asarray(inp["router_b2"], np.float32), (128, NE)))
    bout = np.concatenate([inp["gb3"], inp["lb2"]], 0)
    if np.any(bout != 0):
        h["w_boutbc"] = np.ascontiguousarray(np.broadcast_to(
            np.asarray(bout, np.float32).reshape(1, NE * O), (128, NE * O)))
    if np.any(np.asarray(inp["out_b2"]) != 0):
        h["w_ob2bc"] = np.ascontiguousarray(np.broadcast_to(
            np.asarray(inp["out_b2"], np.float32), (128, O)))
    return h


def prep_x(xs):
    """xs [S,B,C,E] f32 -> per-core (xhi, xlo), each [NT, ECH, 128, T] bf16."""
    out = []
    for c in range(NCORES):
        blk = np.asarray(xs, np.float32)[:, c * BC:(c + 1) * BC]
        xt = blk.transpose(0, 1, 3, 2).reshape(NT, ECH, 128, T)
        hi = xt.astype(_bf)
        lo = (xt - hi.astype(np.float32)).astype(_bf)
        out.append((np.ascontiguousarray(hi), np.ascontiguousarray(lo)))
    return out


_CACHE = {}


def build_in_maps(inputs):
    host = prep_weights(inputs)
    key = "graph:" + ",".join(sorted(host))
    if key not in _CACHE:
        _CACHE[key] = build_graph(host)
    nc = _CACHE[key]
    xs_shards = prep_x(inputs["xs"])
    in_maps = []
    for c in range(NCORES):
        m = dict(host)
        m["xhi"], m["xlo"] = xs_shards[c]
        in_maps.append(m)
    return nc, in_maps


def assemble(results):
    outs = np.concatenate(
        [results[c]["out"][None] for c in range(NCORES)], 0).reshape(B, C, O)
    ent_sum = sum(float(results[c]["ent"][0, 0]) for c in range(NCORES))
    bl = np.float32(0.1 * ent_sum / (S * B * C))
    return outs, bl


def kernel(**inputs):
    nc, in_maps = build_in_maps(inputs)
    res = run_bass_kernel_spmd(nc, in_maps, core_ids=list(range(NCORES)))
    return assemble(res.results)


# revision 19
# speedup vs baseline: 1.3016x; 1.0414x over previous
"""AdaptiveScaleRoutingMoE block on 8 TRN2 NeuronCores.

Strategy: data-parallel over batch (B=32 -> 4 per core). All weights
replicated. Per (scale, batch) pair one 512-token tile, 12 tiles/core.

Precision: router L1 in split-bf16 (hi/lo, 3 cross terms, exact gelu via the
Erf LUT), router L2 in f32 (top-2 selection is rank-sensitive), experts bf16
with f32 accumulation. The top-2-of-6 local-expert gather is replaced by a
mask built from the second max; the weighted combine runs token-major via
scalar_tensor_tensor with per-partition router-weight columns. Balance-loss
entropy uses ln(sum_exp) - sum(p*logit) (log-softmax identity), partial sums
reduced on host.
"""
import numpy as np
import ml_dtypes
from contextlib import ExitStack

import concourse.bass as bass
import concourse.tile as tile
from concourse import bacc, mybir
from concourse.bass_utils import run_bass_kernel_spmd

F32 = mybir.dt.float32
BF16 = mybir.dt.bfloat16
AF = mybir.ActivationFunctionType
ALU = mybir.AluOpType
AX = mybir.AxisListType

S, B, C, E = 3, 32, 512, 256
H, O, NE, NS, NL = 512, 256, 8, 2, 6
NCORES = 8
BC = B // NCORES          # batches per core
NT = S * BC               # token tiles per core (one per (s,b)), each T tokens
T = C                     # 512 tokens per tile
TCH = T // 128            # 4 token chunks
ECH = E // 128            # 2
HCH = H // 128            # 4
OCH = O // 128            # 2
KCC = 896 // 128          # 7 concat chunks (771 padded to 896)
RT2 = float(1.0 / np.sqrt(2.0))

_bf = ml_dtypes.bfloat16


def _to_bf(a):
    return np.ascontiguousarray(np.asarray(a, np.float32).astype(_bf))


def _feat_major(w):
    """[K, M] weight -> SBUF lhsT layout [128, K/128, M]."""
    k, m = w.shape
    return np.ascontiguousarray(
        np.asarray(w, np.float32).reshape(k // 128, 128, m).transpose(1, 0, 2))


def _bias_chunks(b):
    """[F] bias -> [128, F/128] per-partition layout."""
    f = b.shape[0]
    return np.ascontiguousarray(np.asarray(b, np.float32).reshape(f // 128, 128).T)


def build_graph(host):
    """host: dict of prepped numpy weight arrays. Returns compiled nc."""
    nc = bacc.Bacc("TRN2", target_bir_lowering=False, debug=False,
                   num_devices=NCORES)

    dram = {}

    def din(name, shape, dt):
        dram[name] = nc.dram_tensor(name, list(shape), dt, kind="ExternalInput")
        return dram[name]

    din("xhi", [NT, ECH, 128, T], BF16)
    din("xlo", [NT, ECH, 128, T], BF16)
    for k, v in host.items():
        din(k, v.shape, BF16 if v.dtype == _bf else F32)

    out_ext = nc.dram_tensor("out", [BC, C, O], F32, kind="ExternalOutput")
    ent_ext = nc.dram_tensor("ent", [1, 1], F32, kind="ExternalOutput")

    have_rb2 = "w_rb2bc" in host
    have_bout = "w_boutbc" in host
    have_ob2 = "w_ob2bc" in host

    with tile.TileContext(nc) as tc, ExitStack() as ctx:
        nx = nc
        wp = ctx.enter_context(tc.tile_pool(name="weights", bufs=1))
        xp = ctx.enter_context(tc.tile_pool(name="xstash", bufs=1))
        stash = ctx.enter_context(tc.tile_pool(name="stash", bufs=1))
        hwork = ctx.enter_context(tc.tile_pool(name="hwork", bufs=2))
        ewp = ctx.enter_context(tc.tile_pool(name="ewp", bufs=1))
        ghp = ctx.enter_context(tc.tile_pool(name="ghp", bufs=1))
        sm = ctx.enter_context(tc.tile_pool(name="small", bufs=3))
        acc = ctx.enter_context(tc.tile_pool(name="accp", bufs=2))
        drp = ctx.enter_context(tc.tile_pool(name="drbounce", bufs=1, space="DRAM"))
        pp512 = ctx.enter_context(tc.tile_pool(name="pp512", bufs=2, space="PSUM"))
        pp256 = ctx.enter_context(tc.tile_pool(name="pp256", bufs=3, space="PSUM"))
        pps = ctx.enter_context(tc.tile_pool(name="pps", bufs=1, space="PSUM"))

        def wt(name, dt=BF16):
            a = host[name]
            t_ = wp.tile(list(a.shape), dt, tag=name)
            nx.sync.dma_start(t_[:], dram[name][:])
            return t_

        # DMA priority order: what phase R needs first
        w1hi = wt("w_w1hi"); w1lo = wt("w_w1lo")
        rw2 = wt("w_rw2", F32)
        have_rb1 = "w_rb1s" in host
        have_ab1 = "w_ab1" in host
        have_gb2 = "w_gb2" in host
        rb1s = wt("w_rb1s", F32) if have_rb1 else None
        rb2bc = wt("w_rb2bc", F32) if have_rb2 else None
        xhi = xp.tile([128, ECH, NT, T], BF16, tag="xhi")
        for i in range(NT):
            for ec in range(ECH):
                nx.sync.dma_start(xhi[:, ec, i, :], dram["xhi"][i, ec])
        onesb = wp.tile([128, 1], BF16, tag="onesb")
        nx.vector.memset(onesb[:], 1.0)
        onesf = wp.tile([128, 1], F32, tag="onesf")
        nx.vector.memset(onesf[:], 1.0)

        logit_st = stash.tile([128, TCH, NT, NE], F32, tag="logit")
        wst = stash.tile([128, TCH, NT, NE], F32, tag="wst")
        ssum_st = stash.tile([128, TCH * NT], F32, tag="ssum_st")
        spl_st = stash.tile([128, TCH * NT], F32, tag="spl_st")
        pst = stash.tile([128, TCH, NT, O], BF16, tag="pst")
        mlin = stash.tile([1, NT, O], F32, tag="mlin")
        mfm = stash.tile([128, OCH, NT], BF16, tag="mfm")
        reprs = stash.tile([128, OCH, NT], BF16, tag="reprs")
        ccat = stash.tile([128, KCC, BC], BF16, tag="ccat")
        th_sb = stash.tile([S, BC], F32, tag="th")
        swb = stash.tile([128, BC * S], F32, tag="swb")
        sw_bounce = drp.tile([BC * S], F32, tag="swb_d")
        tr_bounce = drp.tile([S, BC], F32, tag="trb_d")
        mean_bounce = drp.tile([NT, O], F32, tag="meanb_d")

        # ---------------- Phase R: router L1 (Erf) + L2 (f32) --------------
        xlo_tiles = {}
        def fetch_xlo(i):
            t_ = hwork.tile([128, ECH, T], BF16, tag="xlo")
            for ec in range(ECH):
                nx.sync.dma_start(t_[:, ec, :], dram["xlo"][i, ec])
            xlo_tiles[i] = t_
        fetch_xlo(0)
        for i in range(NT):
            if i + 1 < NT:
                fetch_xlo(i + 1)
            xlo_t = xlo_tiles.pop(i)
            h1 = hwork.tile([128, HCH, T], F32, tag="h1")
            for mcp in range(HCH // 2):
                ps = pp512.tile([128, 2, T], F32, tag="ps512")
                for mh in range(2):
                    mc = mcp * 2 + mh
                    terms = [(w1hi, xhi[:, ec, i, :]) for ec in range(ECH)]
                    terms += [(w1lo, xhi[:, ec, i, :]) for ec in range(ECH)]
                    terms += [(w1hi, xlo_t[:, ec, :]) for ec in range(ECH)]
                    n_terms = len(terms)
                    for j, (lhs, r) in enumerate(terms):
                        ec = j % ECH
                        nx.tensor.matmul(
                            ps[:, mh, :], lhs[:, ec, bass.ts(mc, 128)], r,
                            start=(j == 0), stop=(j == n_terms - 1))
                erf = hwork.tile([128, 2, T], F32, tag="erf")
                # erf((z + b1)/sqrt(2)); z in psum; rb1s = b1/sqrt(2).
                if have_rb1:
                    for mh in range(2):
                        nx.scalar.activation(
                            erf[:, mh, :], ps[:, mh, :], AF.Erf,
                            bias=rb1s[:, mcp * 2 + mh:mcp * 2 + mh + 1],
                            scale=RT2)
                else:
                    nx.scalar.activation(erf[:], ps[:], AF.Erf, scale=RT2)
                # h1 = (erf + 1) * z = 2*gelu(z)  (0.5 folded into rw2)
                nx.vector.scalar_tensor_tensor(
                    h1[:, mcp * 2:mcp * 2 + 2, :], erf[:], 1.0, ps[:],
                    op0=ALU.add, op1=ALU.mult)
            for tc_ in range(TCH):
                pl = pps.tile([128, NE], F32, tag="ppsmall")
                for kc in range(HCH):
                    nx.tensor.matmul(
                        pl[:], h1[:, kc, bass.ts(tc_, 128)], rw2[:, kc, :],
                        start=(kc == 0), stop=(kc == HCH - 1))
                if have_rb2:
                    nx.vector.tensor_tensor(pl[:], pl[:], rb2bc[:, :NE],
                                            op=ALU.add)
                nx.vector.tensor_copy(logit_st[:, tc_, i, :], pl[:])

        # bulk weights: issued after R's x DMAs so they don't starve phase R
        aW1 = wt("w_aW1")
        ab1 = wt("w_ab1", F32) if have_ab1 else None
        gW2 = wt("w_gW2")
        gb2 = wt("w_gb2", F32) if have_gb2 else None
        aWo = wt("w_aWo")
        tew = wt("w_tew"); teb = wt("w_teb", F32)
        wcw = wt("w_wcw"); wcb = wt("w_wcb", F32)
        ow1 = wt("w_ow1"); ob1 = wt("w_ob1", F32)
        ow2 = wt("w_ow2")
        ident = wt("w_ident")
        boutbc = wt("w_boutbc", F32) if have_bout else None
        ob2bc = wt("w_ob2bc", F32) if have_ob2 else None

        tc.no_sync_barrier()
        # ---------------- Phase W: batched softmax + entropy + top-2 -------
        # All 48 chunks at once on [128, TCH, NT, *] views of the stashes.
        ssum3 = ssum_st[:].rearrange("p (a b) -> p a b", a=TCH)
        spl3 = spl_st[:].rearrange("p (a b) -> p a b", a=TCH)
        nx.scalar.activation(wst[:], logit_st[:], AF.Exp)
        nx.vector.reduce_sum(ssum3, wst[:], axis=AX.X)
        rcpa = sm.tile([128, TCH, NT, 1], F32, tag="rcpa")
        nx.vector.reciprocal(rcpa[:], ssum3)
        nx.vector.tensor_tensor(
            wst[:], wst[:], rcpa[:].broadcast_to([128, TCH, NT, NE]),
            op=ALU.mult)
        pl8a = sm.tile([128, TCH, NT, NE], F32, tag="pl8a")
        nx.vector.tensor_tensor(pl8a[:], wst[:], logit_st[:], op=ALU.mult)
        nx.vector.reduce_sum(spl3, pl8a[:], axis=AX.X)
        wl = wst[:, :, :, NS:NE]
        m1a = sm.tile([128, TCH, NT, 1], F32, tag="rcpa")
        nx.vector.reduce_max(m1a[:], wl, axis=AX.X)
        eqa = sm.tile([128, TCH, NT, NL], F32, tag="pl8a")
        nx.vector.tensor_tensor(
            eqa[:], wl, m1a[:].broadcast_to([128, TCH, NT, NL]),
            op=ALU.is_equal)
        w2a = sm.tile([128, TCH, NT, NL], F32, tag="w2a")
        nx.vector.scalar_tensor_tensor(
            w2a[:], eqa[:], -1e30, wl, op0=ALU.mult, op1=ALU.add)
        m2a = sm.tile([128, TCH, NT, 1], F32, tag="rcpa")
        nx.vector.reduce_max(m2a[:], w2a[:], axis=AX.X)
        sela = sm.tile([128, TCH, NT, NL], F32, tag="pl8a")
        nx.vector.tensor_tensor(
            sela[:], wl, m2a[:].broadcast_to([128, TCH, NT, NL]),
            op=ALU.is_ge)
        nx.vector.tensor_tensor(wl, wl, sela[:], op=ALU.mult)

        tc.no_sync_barrier()
        # ---------------- Phase E: experts (Gelu) + combine + means --------
        # Experts run in two half-groups of 4 so the eh buffer double-buffers
        # across tiles (cross-tile pipelining) at no extra SBUF cost.
        for i in range(NT):
            acq = acc.tile([128, TCH, O], F32, tag="acc4")
            for half in range(2):
                eh = ewp.tile([128, 4, HCH, T], BF16, tag="eh")
                for nn in range(4):
                    n = half * 4 + nn
                    for mcp in range(HCH // 2):
                        ps = pp512.tile([128, 2, T], F32, tag="ps512")
                        for mh in range(2):
                            mc = mcp * 2 + mh
                            for ec in range(ECH):
                                nx.tensor.matmul(
                                    ps[:, mh, :],
                                    aW1[:, ec, n, bass.ts(mc, 128)],
                                    xhi[:, ec, i, :],
                                    start=(ec == 0), stop=(ec == ECH - 1))
                        if have_ab1:
                            for mh in range(2):
                                nx.scalar.activation(
                                    eh[:, nn, mcp * 2 + mh, :], ps[:, mh, :],
                                    AF.Gelu,
                                    bias=ab1[:, n,
                                             mcp * 2 + mh:mcp * 2 + mh + 1])
                        else:
                            nx.scalar.activation(
                                eh[:, nn, mcp * 2:mcp * 2 + 2, :], ps[:],
                                AF.Gelu)
                if half == 0:
                    gh2 = ghp.tile([128, NS, HCH, T], BF16, tag="gh2")
                    for n in range(NS):
                        for mcp in range(HCH // 2):
                            ps = pp512.tile([128, 2, T], F32, tag="ps512")
                            for mh in range(2):
                                mc = mcp * 2 + mh
                                for kc in range(HCH):
                                    nx.tensor.matmul(
                                        ps[:, mh, :],
                                        gW2[:, kc, n, bass.ts(mc, 128)],
                                        eh[:, n, kc, :],
                                        start=(kc == 0), stop=(kc == HCH - 1))
                            if have_gb2:
                                for mh in range(2):
                                    nx.scalar.activation(
                                        gh2[:, n, mcp * 2 + mh, :],
                                        ps[:, mh, :], AF.Gelu,
                                        bias=gb2[:, n,
                                                 mcp * 2 + mh:mcp * 2 + mh + 1])
                            else:
                                nx.scalar.activation(
                                    gh2[:, n, mcp * 2:mcp * 2 + 2, :], ps[:],
                                    AF.Gelu)
                for tc_ in range(TCH):
                    for nn in range(4):
                        n = half * 4 + nn
                        pn = pp256.tile([128, O], F32, tag="pexp")
                        for kc in range(HCH):
                            if n < NS:
                                lhs = gh2[:, n, kc, bass.ts(tc_, 128)]
                            else:
                                lhs = eh[:, nn, kc, bass.ts(tc_, 128)]
                            nx.tensor.matmul(
                                pn[:], lhs, aWo[:, kc, n, :],
                                start=(kc == 0), stop=(kc == HCH - 1))
                        if have_bout:
                            nx.vector.tensor_tensor(
                                pn[:], pn[:], boutbc[:, bass.ts(n, O)],
                                op=ALU.add)
                        if n == 0:
                            nx.vector.tensor_scalar(
                                acq[:, tc_, :], pn[:], wst[:, tc_, i, 0:1],
                                None, op0=ALU.mult)
                        else:
                            nx.vector.scalar_tensor_tensor(
                                acq[:, tc_, :], pn[:], wst[:, tc_, i, n:n + 1],
                                acq[:, tc_, :], op0=ALU.mult, op1=ALU.add)
            nx.vector.tensor_copy(pst[:, :, i, :], acq[:])
            pm = pps.tile([1, O], F32, tag="ppsmall")
            for tc_ in range(TCH):
                nx.tensor.matmul(pm[:], onesb[:], pst[:, tc_, i, :],
                                 start=(tc_ == 0), stop=(tc_ == TCH - 1))
            nx.vector.tensor_copy(mlin[:, i, :], pm[:])
        # means -> feature-major [128, OCH, NT] via DRAM bounce
        nx.sync.dma_start(mean_bounce[:], mlin[0, :, :])
        mfm_f = acc.tile([128, OCH, NT], F32, tag="mfmf")
        for oc in range(OCH):
            nx.sync.dma_start(
                mfm_f[:, oc, :],
                mean_bounce[:, oc * 128:(oc + 1) * 128].transpose([1, 0]))
        nx.vector.tensor_copy(mfm[:], mfm_f[:])
        for mc in range(OCH):
            ps = pps.tile([128, NT], F32, tag="ppsmall")
            for kc in range(OCH):
                nx.tensor.matmul(ps[:], tew[:, kc, bass.ts(mc, 128)],
                                 mfm[:, kc, :],
                                 start=(kc == 0), stop=(kc == OCH - 1))
            nx.scalar.activation(reprs[:, mc, :], ps[:], AF.Gelu,
                                 bias=teb[:, mc:mc + 1])
        # concat [771 padded 896, BC]; rows s*256+o2 from reprs, 768+s from wm
        nx.vector.memset(ccat[:], 0.0)
        for s in range(S):
            for oc in range(OCH):
                nx.sync.dma_start(ccat[:, s * OCH + oc, :],
                                  reprs[:, oc, s * BC:(s + 1) * BC])
        nx.sync.dma_start(ccat[0:S, KCC - 1, :], dram["w_wmbc"][:])
        pw = pps.tile([S, BC], F32, tag="ppsmall")
        for kc in range(KCC):
            nx.tensor.matmul(pw[:], wcw[:, kc, :], ccat[:, kc, :],
                             start=(kc == 0), stop=(kc == KCC - 1))
        nx.scalar.activation(th_sb[:], pw[:], AF.Tanh, bias=wcb[0:S, 0:1])

        tc.no_sync_barrier()
        # ---------------- Phase G: softplus + scale softmax + entropy ------
        ee = sm.tile([S, BC], F32, tag="ee")
        nx.scalar.activation(ee[:], th_sb[:], AF.Exp)
        nx.vector.tensor_scalar(ee[:], ee[:], 1.0, None, op0=ALU.add)
        raw = sm.tile([S, BC], F32, tag="raw")
        nx.scalar.activation(raw[:], ee[:], AF.Ln)
        nx.sync.dma_start(tr_bounce[:], raw[:])
        rawt = sm.tile([BC, S], F32, tag="rawt")
        nx.sync.dma_start(rawt[:], tr_bounce[:].transpose([1, 0]))
        ex = sm.tile([BC, S], F32, tag="ex")
        nx.scalar.activation(ex[:], rawt[:], AF.Exp)
        ssum2 = sm.tile([BC, 1], F32, tag="ssum2")
        nx.vector.reduce_sum(ssum2[:], ex[:], axis=AX.X)
        rcp2 = sm.tile([BC, 1], F32, tag="rcp2")
        nx.vector.reciprocal(rcp2[:], ssum2[:])
        swt = sm.tile([BC, S], F32, tag="swt")
        nx.vector.tensor_scalar(swt[:], ex[:], rcp2[:], None, op0=ALU.mult)
        nx.sync.dma_start(sw_bounce[:], swt[:])
        nx.sync.dma_start(swb[:], sw_bounce[:].partition_broadcast(128))
        lns_all = sm.tile([128, TCH * NT], F32, tag="lns_all")
        nx.scalar.activation(lns_all[:], ssum_st[:], AF.Ln)
        ent_all = sm.tile([128, TCH * NT], F32, tag="ent_all")
        nx.vector.tensor_tensor(ent_all[:], lns_all[:], spl_st[:],
                                op=ALU.subtract)
        entv = sm.tile([128, 1], F32, tag="entv")
        nx.vector.reduce_sum(entv[:], ent_all[:], axis=AX.X)
        pe_ = pps.tile([1, 1], F32, tag="ppsmall")
        nx.tensor.matmul(pe_[:], onesf[:], entv[:], start=True, stop=True)
        esb = sm.tile([1, 1], F32, tag="esb")
        nx.vector.tensor_copy(esb[:], pe_[:])
        nx.sync.dma_start(ent_ext[:], esb[:])

        tc.no_sync_barrier()
        # ---------------- Phase OUT: scale-weighted sum + output MLP -------
        for b in range(BC):
            wacc = acc.tile([128, TCH, O], BF16, tag="wacc")
            for tc_ in range(TCH):
                tmp = sm.tile([128, O], F32, tag="wtmp")
                nx.vector.tensor_scalar(
                    tmp[:], pst[:, tc_, 0 * BC + b, :],
                    swb[:, b * S:b * S + 1], None, op0=ALU.mult)
                nx.vector.scalar_tensor_tensor(
                    tmp[:], pst[:, tc_, 1 * BC + b, :],
                    swb[:, b * S + 1:b * S + 2], tmp[:],
                    op0=ALU.mult, op1=ALU.add)
                nx.vector.scalar_tensor_tensor(
                    wacc[:, tc_, :], pst[:, tc_, 2 * BC + b, :],
                    swb[:, b * S + 2:b * S + 3], tmp[:],
                    op0=ALU.mult, op1=ALU.add)
            wfm = ghp.tile([128, OCH, T], BF16, tag="wfm")
            for tc_ in range(TCH):
                for oc in range(OCH):
                    pt = pp256.tile([128, 128], BF16, tag="pexp")
                    nx.tensor.transpose(pt[:], wacc[:, tc_, bass.ts(oc, 128)],
                                        ident[:])
                    nx.vector.tensor_copy(wfm[:, oc, bass.ts(tc_, 128)], pt[:])
            o1 = ghp.tile([128, OCH, T], BF16, tag="o1")
            for mc in range(OCH):
                ps = pp512.tile([128, T], F32, tag="ps512")
                for kc in range(OCH):
                    nx.tensor.matmul(ps[:], ow1[:, kc, bass.ts(mc, 128)],
                                     wfm[:, kc, :],
                                     start=(kc == 0), stop=(kc == OCH - 1))
                nx.scalar.activation(o1[:, mc, :], ps[:], AF.Gelu,
                                     bias=ob1[:, mc:mc + 1])
            for tc_ in range(TCH):
                pf = pp256.tile([128, O], F32, tag="pexp")
                for kc in range(OCH):
                    nx.tensor.matmul(pf[:], o1[:, kc, bass.ts(tc_, 128)],
                                     ow2[:, kc, :],
                                     start=(kc == 0), stop=(kc == OCH - 1))
                if have_ob2:
                    nx.vector.tensor_tensor(pf[:], pf[:], ob2bc[:], op=ALU.add)
                osb = sm.tile([128, O], F32, tag="osb")
                nx.vector.tensor_copy(osb[:], pf[:])
                nx.sync.dma_start(out_ext[b, bass.ts(tc_, 128), :], osb[:])

    nc.compile()
    return nc


def prep_weights(inp):
    h = {}
    w1 = np.asarray(inp["router_w1"], np.float32)
    w1hi = w1.astype(_bf).astype(np.float32)
    h["w_w1hi"] = _to_bf(_feat_major(w1hi))
    h["w_w1lo"] = _to_bf(_feat_major(w1 - w1hi))
    h["w_rw2"] = _feat_major(np.asarray(inp["router_w2"], np.float32) * 0.5)
    if np.any(np.asarray(inp["router_b1"]) != 0):
        h["w_rb1s"] = _bias_chunks(np.asarray(inp["router_b1"]) * RT2)
    aW1 = np.concatenate([inp["gW1"], inp["lW1"]], 0)
    h["w_aW1"] = _to_bf(np.stack([_feat_major(aW1[n]) for n in range(NE)], 2))
    ab1 = np.concatenate([inp["gb1"], inp["lb1"]], 0)
    if np.any(ab1 != 0):
        h["w_ab1"] = np.ascontiguousarray(
            np.stack([_bias_chunks(ab1[n]) for n in range(NE)], 1))
    h["w_gW2"] = _to_bf(np.stack(
        [_feat_major(np.asarray(inp["gW2"])[n]) for n in range(NS)], 2))
    if np.any(np.asarray(inp["gb2"]) != 0):
        h["w_gb2"] = np.ascontiguousarray(np.stack(
            [_bias_chunks(np.asarray(inp["gb2"])[n]) for n in range(NS)], 1))
    aWo = np.concatenate([inp["gW3"], inp["lW2"]], 0)
    h["w_aWo"] = _to_bf(np.stack([_feat_major(aWo[n]) for n in range(NE)], 2))
    h["w_tew"] = _to_bf(_feat_major(np.asarray(inp["te_w"], np.float32) / C))
    h["w_teb"] = _bias_chunks(inp["te_b"])
    wcw = np.zeros((896, S), np.float32)
    wcw[:768 + S] = np.asarray(inp["wc_w"], np.float32)
    h["w_wcw"] = _to_bf(_feat_major(wcw))
    wcb = np.zeros((128, 1), np.float32)
    wcb[:S, 0] = np.asarray(inp["wc_b"])
    h["w_wcb"] = wcb
    h["w_ow1"] = _to_bf(_feat_major(inp["out_w1"]))
    h["w_ob1"] = _bias_chunks(inp["out_b1"])
    h["w_ow2"] = _to_bf(_feat_major(inp["out_w2"]))
    h["w_ident"] = _to_bf(np.eye(128, dtype=np.float32))
    h["w_wmbc"] = _to_bf(np.broadcast_to(
        np.asarray(inp["weight_memory"], np.float32).reshape(S, 1), (S, BC)))
    if np.any(np.asarray(inp["router_b2"]) != 0):
        h["w_rb2bc"] = np.ascontiguousarray(np.broadcast_to(
            np.asarray(inp["router_b2"], np.float32), (128, NE)))
    bout = np.concatenate([inp["gb3"], inp["lb2"]], 0)
    if np.any(bout != 0):
        h["w_boutbc"] = np.ascontiguousarray(np.broadcast_to(
            np.asarray(bout, np.float32).reshape(1, NE * O), (128, NE * O)))
    if np.any(np.asarray(inp["out_b2"]) != 0):
        h["w_ob2bc"] = np.ascontiguousarray(np.broadcast_to(
            np.asarray(inp["out_b2"], np.float32), (128, O)))
    return h


def prep_x(xs):
    """xs [S,B,C,E] f32 -> per-core (xhi, xlo), each [NT, ECH, 128, T] bf16."""
    out = []
    for c in range(NCORES):
        blk = np.asarray(xs, np.float32)[:, c * BC:(c + 1) * BC]
        xt = blk.transpose(0, 1, 3, 2).reshape(NT, ECH, 128, T)
        hi = xt.astype(_bf)
        lo = (xt - hi.astype(np.float32)).astype(_bf)
        out.append((np.ascontiguousarray(hi), np.ascontiguousarray(lo)))
    return out


_CACHE = {}


def build_in_maps(inputs):
    host = prep_weights(inputs)
    key = "graph:" + ",".join(sorted(host))
    if key not in _CACHE:
        _CACHE[key] = build_graph(host)
    nc = _CACHE[key]
    xs_shards = prep_x(inputs["xs"])
    in_maps = []
    for c in range(NCORES):
        m = dict(host)
        m["xhi"], m["xlo"] = xs_shards[c]
        in_maps.append(m)
    return nc, in_maps


def assemble(results):
    outs = np.concatenate(
        [results[c]["out"][None] for c in range(NCORES)], 0).reshape(B, C, O)
    ent_sum = sum(float(results[c]["ent"][0, 0]) for c in range(NCORES))
    bl = np.float32(0.1 * ent_sum / (S * B * C))
    return outs, bl


def kernel(**inputs):
    nc, in_maps = build_in_maps(inputs)
    res = run_bass_kernel_spmd(nc, in_maps, core_ids=list(range(NCORES)))
    return assemble(res.results)


# revision 21
# speedup vs baseline: 1.3380x; 1.0279x over previous
"""AdaptiveScaleRoutingMoE block on 8 TRN2 NeuronCores.

Strategy: data-parallel over batch (B=32 -> 4 per core). All weights
replicated. Per (scale, batch) pair one 512-token tile, 12 tiles/core.

Precision: router L1 in split-bf16 (hi/lo, 3 cross terms, exact gelu via the
Erf LUT), router L2 in f32 (top-2 selection is rank-sensitive), experts bf16
with f32 accumulation. The top-2-of-6 local-expert gather is replaced by a
mask built from the second max; the weighted combine runs token-major via
scalar_tensor_tensor with per-partition router-weight columns. Balance-loss
entropy uses ln(sum_exp) - sum(p*logit) (log-softmax identity), partial sums
reduced on host.
"""
import numpy as np
import ml_dtypes
from contextlib import ExitStack

import concourse.bass as bass
import concourse.tile as tile
from concourse import bacc, mybir
from concourse.bass_utils import run_bass_kernel_spmd

F32 = mybir.dt.float32
BF16 = mybir.dt.bfloat16
AF = mybir.ActivationFunctionType
ALU = mybir.AluOpType
AX = mybir.AxisListType

S, B, C, E = 3, 32, 512, 256
H, O, NE, NS, NL = 512, 256, 8, 2, 6
NCORES = 8
BC = B // NCORES          # batches per core
NT = S * BC               # token tiles per core (one per (s,b)), each T tokens
T = C                     # 512 tokens per tile
TCH = T // 128            # 4 token chunks
ECH = E // 128            # 2
HCH = H // 128            # 4
OCH = O // 128            # 2
KCC = 896 // 128          # 7 concat chunks (771 padded to 896)
RT2 = float(1.0 / np.sqrt(2.0))

_bf = ml_dtypes.bfloat16


def _to_bf(a):
    return np.ascontiguousarray(np.asarray(a, np.float32).astype(_bf))


def _feat_major(w):
    """[K, M] weight -> SBUF lhsT layout [128, K/128, M]."""
    k, m = w.shape
    return np.ascontiguousarray(
        np.asarray(w, np.float32).reshape(k // 128, 128, m).transpose(1, 0, 2))


def _bias_chunks(b):
    """[F] bias -> [128, F/128] per-partition layout."""
    f = b.shape[0]
    return np.ascontiguousarray(np.asarray(b, np.float32).reshape(f // 128, 128).T)


def build_graph(host):
    """host: dict of prepped numpy weight arrays. Returns compiled nc."""
    nc = bacc.Bacc("TRN2", target_bir_lowering=False, debug=False,
                   num_devices=NCORES)

    dram = {}

    def din(name, shape, dt):
        dram[name] = nc.dram_tensor(name, list(shape), dt, kind="ExternalInput")
        return dram[name]

    din("xhi", [NT, ECH, 128, T], BF16)
    din("xlo", [NT, ECH, 128, T], BF16)
    for k, v in host.items():
        din(k, v.shape, BF16 if v.dtype == _bf else F32)

    out_ext = nc.dram_tensor("out", [BC, C, O], F32, kind="ExternalOutput")
    ent_ext = nc.dram_tensor("ent", [1, 1], F32, kind="ExternalOutput")

    have_rb2 = "w_rb2bc" in host
    have_bout = "w_boutbc" in host
    have_ob2 = "w_ob2bc" in host

    with tile.TileContext(nc) as tc, ExitStack() as ctx:
        nx = nc
        wp = ctx.enter_context(tc.tile_pool(name="weights", bufs=1))
        xp = ctx.enter_context(tc.tile_pool(name="xstash", bufs=1))
        stash = ctx.enter_context(tc.tile_pool(name="stash", bufs=1))
        hwork = ctx.enter_context(tc.tile_pool(name="hwork", bufs=2))
        ewp = ctx.enter_context(tc.tile_pool(name="ewp", bufs=1))
        ghp = ctx.enter_context(tc.tile_pool(name="ghp", bufs=1))
        sm = ctx.enter_context(tc.tile_pool(name="small", bufs=3))
        hsp = ctx.enter_context(tc.tile_pool(name="hsplit", bufs=1))
        acc = ctx.enter_context(tc.tile_pool(name="accp", bufs=2))
        drp = ctx.enter_context(tc.tile_pool(name="drbounce", bufs=1, space="DRAM"))
        pp512 = ctx.enter_context(tc.tile_pool(name="pp512", bufs=2, space="PSUM"))
        pp256 = ctx.enter_context(tc.tile_pool(name="pp256", bufs=3, space="PSUM"))
        pps = ctx.enter_context(tc.tile_pool(name="pps", bufs=1, space="PSUM"))

        def wt(name, dt=BF16):
            a = host[name]
            t_ = wp.tile(list(a.shape), dt, tag=name)
            nx.sync.dma_start(t_[:], dram[name][:])
            return t_

        # DMA priority order: what phase R needs first
        w1hi = wt("w_w1hi"); w1lo = wt("w_w1lo")
        rw2 = wt("w_rw2")
        rw2lo = wt("w_rw2lo")
        have_rb1 = "w_rb1s" in host
        have_ab1 = "w_ab1" in host
        have_gb2 = "w_gb2" in host
        rb1s = wt("w_rb1s", F32) if have_rb1 else None
        rb2bc = wt("w_rb2bc", F32) if have_rb2 else None
        xhi = xp.tile([128, ECH, NT, T], BF16, tag="xhi")
        for i in range(NT):
            for ec in range(ECH):
                nx.sync.dma_start(xhi[:, ec, i, :], dram["xhi"][i, ec])
        onesb = wp.tile([128, 1], BF16, tag="onesb")
        nx.vector.memset(onesb[:], 1.0)
        onesf = wp.tile([128, 1], F32, tag="onesf")
        nx.vector.memset(onesf[:], 1.0)

        logit_st = stash.tile([128, TCH, NT, NE], F32, tag="logit")
        wst = stash.tile([128, TCH, NT, NE], F32, tag="wst")
        ssum_st = stash.tile([128, TCH * NT], F32, tag="ssum_st")
        spl_st = stash.tile([128, TCH * NT], F32, tag="spl_st")
        pst = stash.tile([128, TCH, NT, O], BF16, tag="pst")
        mlin = stash.tile([1, NT, O], F32, tag="mlin")
        mfm = stash.tile([128, OCH, NT], BF16, tag="mfm")
        reprs = stash.tile([128, OCH, NT], BF16, tag="reprs")
        ccat = stash.tile([128, KCC, BC], BF16, tag="ccat")
        th_sb = stash.tile([S, BC], F32, tag="th")
        swb = stash.tile([128, BC * S], F32, tag="swb")
        sw_bounce = drp.tile([BC * S], F32, tag="swb_d")
        tr_bounce = drp.tile([S, BC], F32, tag="trb_d")
        mean_bounce = drp.tile([NT, O], F32, tag="meanb_d")

        # ---------------- Phase R: router L1 (Erf) + L2 (f32) --------------
        xlo_tiles = {}
        def fetch_xlo(i):
            t_ = hwork.tile([128, ECH, T], BF16, tag="xlo")
            for ec in range(ECH):
                nx.sync.dma_start(t_[:, ec, :], dram["xlo"][i, ec])
            xlo_tiles[i] = t_
        fetch_xlo(0)
        for i in range(NT):
            if i + 1 < NT:
                fetch_xlo(i + 1)
            xlo_t = xlo_tiles.pop(i)
            h1 = hwork.tile([128, HCH, T], F32, tag="h1")
            for mcp in range(HCH // 2):
                ps = pp512.tile([128, 2, T], F32, tag="ps512")
                for mh in range(2):
                    mc = mcp * 2 + mh
                    terms = [(w1hi, xhi[:, ec, i, :]) for ec in range(ECH)]
                    terms += [(w1lo, xhi[:, ec, i, :]) for ec in range(ECH)]
                    terms += [(w1hi, xlo_t[:, ec, :]) for ec in range(ECH)]
                    n_terms = len(terms)
                    for j, (lhs, r) in enumerate(terms):
                        ec = j % ECH
                        nx.tensor.matmul(
                            ps[:, mh, :], lhs[:, ec, bass.ts(mc, 128)], r,
                            start=(j == 0), stop=(j == n_terms - 1))
                erf = hwork.tile([128, 2, T], F32, tag="erf")
                # erf((z + b1)/sqrt(2)); z in psum; rb1s = b1/sqrt(2).
                if have_rb1:
                    for mh in range(2):
                        nx.scalar.activation(
                            erf[:, mh, :], ps[:, mh, :], AF.Erf,
                            bias=rb1s[:, mcp * 2 + mh:mcp * 2 + mh + 1],
                            scale=RT2)
                else:
                    nx.scalar.activation(erf[:], ps[:], AF.Erf, scale=RT2)
                # h1 = (erf + 1) * z = 2*gelu(z)  (0.5 folded into rw2)
                nx.vector.scalar_tensor_tensor(
                    h1[:, mcp * 2:mcp * 2 + 2, :], erf[:], 1.0, ps[:],
                    op0=ALU.add, op1=ALU.mult)
            h1hi = hsp.tile([128, HCH, T], BF16, tag="h1hi")
            nx.vector.tensor_copy(h1hi[:], h1[:])
            h1lo = hsp.tile([128, HCH, T], BF16, tag="h1lo")
            nx.vector.tensor_tensor(h1lo[:], h1[:], h1hi[:], op=ALU.subtract)
            for tc_ in range(TCH):
                pl = pps.tile([128, NE], F32, tag="ppsmall")
                nmm = 3 * HCH
                j = 0
                for kc in range(HCH):
                    for lhs, r in ((h1hi, rw2), (h1lo, rw2), (h1hi, rw2lo)):
                        nx.tensor.matmul(
                            pl[:], lhs[:, kc, bass.ts(tc_, 128)], r[:, kc, :],
                            start=(j == 0), stop=(j == nmm - 1))
                        j += 1
                if have_rb2:
                    nx.vector.tensor_tensor(pl[:], pl[:], rb2bc[:, :NE],
                                            op=ALU.add)
                nx.vector.tensor_copy(logit_st[:, tc_, i, :], pl[:])

        # bulk weights: issued after R's x DMAs so they don't starve phase R
        aW1 = wt("w_aW1")
        ab1 = wt("w_ab1", F32) if have_ab1 else None
        gW2 = wt("w_gW2")
        gb2 = wt("w_gb2", F32) if have_gb2 else None
        aWo = wt("w_aWo")
        tew = wt("w_tew"); teb = wt("w_teb", F32)
        wcw = wt("w_wcw"); wcb = wt("w_wcb", F32)
        ow1 = wt("w_ow1"); ob1 = wt("w_ob1", F32)
        ow2 = wt("w_ow2")
        ident = wt("w_ident")
        boutbc = wt("w_boutbc", F32) if have_bout else None
        ob2bc = wt("w_ob2bc", F32) if have_ob2 else None

        tc.no_sync_barrier()
        # ---------------- Phase W: batched softmax + entropy + top-2 -------
        # All 48 chunks at once on [128, TCH, NT, *] views of the stashes.
        ssum3 = ssum_st[:].rearrange("p (a b) -> p a b", a=TCH)
        spl3 = spl_st[:].rearrange("p (a b) -> p a b", a=TCH)
        nx.scalar.activation(wst[:], logit_st[:], AF.Exp)
        nx.vector.reduce_sum(ssum3, wst[:], axis=AX.X)
        rcpa = sm.tile([128, TCH, NT, 1], F32, tag="rcpa")
        nx.vector.reciprocal(rcpa[:], ssum3)
        nx.vector.tensor_tensor(
            wst[:], wst[:], rcpa[:].broadcast_to([128, TCH, NT, NE]),
            op=ALU.mult)
        pl8a = sm.tile([128, TCH, NT, NE], F32, tag="pl8a")
        nx.vector.tensor_tensor(pl8a[:], wst[:], logit_st[:], op=ALU.mult)
        nx.vector.reduce_sum(spl3, pl8a[:], axis=AX.X)
        wl = wst[:, :, :, NS:NE]
        m1a = sm.tile([128, TCH, NT, 1], F32, tag="rcpa")
        nx.vector.reduce_max(m1a[:], wl, axis=AX.X)
        eqa = sm.tile([128, TCH, NT, NL], F32, tag="pl8a")
        nx.vector.tensor_tensor(
            eqa[:], wl, m1a[:].broadcast_to([128, TCH, NT, NL]),
            op=ALU.is_equal)
        w2a = sm.tile([128, TCH, NT, NL], F32, tag="w2a")
        nx.vector.scalar_tensor_tensor(
            w2a[:], eqa[:], -1e30, wl, op0=ALU.mult, op1=ALU.add)
        m2a = sm.tile([128, TCH, NT, 1], F32, tag="rcpa")
        nx.vector.reduce_max(m2a[:], w2a[:], axis=AX.X)
        sela = sm.tile([128, TCH, NT, NL], F32, tag="pl8a")
        nx.vector.tensor_tensor(
            sela[:], wl, m2a[:].broadcast_to([128, TCH, NT, NL]),
            op=ALU.is_ge)
        nx.vector.tensor_tensor(wl, wl, sela[:], op=ALU.mult)

        tc.no_sync_barrier()
        # ---------------- Phase E: experts (Gelu) + combine + means --------
        # Experts run in two half-groups of 4 so the eh buffer double-buffers
        # across tiles (cross-tile pipelining) at no extra SBUF cost.
        for i in range(NT):
            acq = acc.tile([128, TCH, O], F32, tag="acc4")
            for half in range(2):
                eh = ewp.tile([128, 4, HCH, T], BF16, tag="eh")
                for nn in range(4):
                    n = half * 4 + nn
                    for mcp in range(HCH // 2):
                        ps = pp512.tile([128, 2, T], F32, tag="ps512")
                        for mh in range(2):
                            mc = mcp * 2 + mh
                            for ec in range(ECH):
                                nx.tensor.matmul(
                                    ps[:, mh, :],
                                    aW1[:, ec, n, bass.ts(mc, 128)],
                                    xhi[:, ec, i, :],
                                    start=(ec == 0), stop=(ec == ECH - 1))
                        if have_ab1:
                            for mh in range(2):
                                nx.scalar.activation(
                                    eh[:, nn, mcp * 2 + mh, :], ps[:, mh, :],
                                    AF.Gelu,
                                    bias=ab1[:, n,
                                             mcp * 2 + mh:mcp * 2 + mh + 1])
                        else:
                            nx.scalar.activation(
                                eh[:, nn, mcp * 2:mcp * 2 + 2, :], ps[:],
                                AF.Gelu)
                if half == 0:
                    gh2 = ghp.tile([128, NS, HCH, T], BF16, tag="gh2")
                    for n in range(NS):
                        for mcp in range(HCH // 2):
                            ps = pp512.tile([128, 2, T], F32, tag="ps512")
                            for mh in range(2):
                                mc = mcp * 2 + mh
                                for kc in range(HCH):
                                    nx.tensor.matmul(
                                        ps[:, mh, :],
                                        gW2[:, kc, n, bass.ts(mc, 128)],
                                        eh[:, n, kc, :],
                                        start=(kc == 0), stop=(kc == HCH - 1))
                            if have_gb2:
                                for mh in range(2):
                                    nx.scalar.activation(
                                        gh2[:, n, mcp * 2 + mh, :],
                                        ps[:, mh, :], AF.Gelu,
                                        bias=gb2[:, n,
                                                 mcp * 2 + mh:mcp * 2 + mh + 1])
                            else:
                                nx.scalar.activation(
                                    gh2[:, n, mcp * 2:mcp * 2 + 2, :], ps[:],
                                    AF.Gelu)
                for tc_ in range(TCH):
                    for nn in range(4):
                        n = half * 4 + nn
                        pn = pp256.tile([128, O], F32, tag="pexp")
                        for kc in range(HCH):
                            if n < NS:
                                lhs = gh2[:, n, kc, bass.ts(tc_, 128)]
                            else:
                                lhs = eh[:, nn, kc, bass.ts(tc_, 128)]
                            nx.tensor.matmul(
                                pn[:], lhs, aWo[:, kc, n, :],
                                start=(kc == 0), stop=(kc == HCH - 1))
                        if have_bout:
                            nx.vector.tensor_tensor(
                                pn[:], pn[:], boutbc[:, bass.ts(n, O)],
                                op=ALU.add)
                        if n == 0:
                            nx.vector.tensor_scalar(
                                acq[:, tc_, :], pn[:], wst[:, tc_, i, 0:1],
                                None, op0=ALU.mult)
                        else:
                            nx.vector.scalar_tensor_tensor(
                                acq[:, tc_, :], pn[:], wst[:, tc_, i, n:n + 1],
                                acq[:, tc_, :], op0=ALU.mult, op1=ALU.add)
            nx.vector.tensor_copy(pst[:, :, i, :], acq[:])
            pm = pps.tile([1, O], F32, tag="ppsmall")
            for tc_ in range(TCH):
                nx.tensor.matmul(pm[:], onesb[:], pst[:, tc_, i, :],
                                 start=(tc_ == 0), stop=(tc_ == TCH - 1))
            nx.vector.tensor_copy(mlin[:, i, :], pm[:])
        # means -> feature-major [128, OCH, NT] via DRAM bounce
        nx.sync.dma_start(mean_bounce[:], mlin[0, :, :])
        mfm_f = acc.tile([128, OCH, NT], F32, tag="mfmf")
        for oc in range(OCH):
            nx.sync.dma_start(
                mfm_f[:, oc, :],
                mean_bounce[:, oc * 128:(oc + 1) * 128].transpose([1, 0]))
        nx.vector.tensor_copy(mfm[:], mfm_f[:])
        for mc in range(OCH):
            ps = pps.tile([128, NT], F32, tag="ppsmall")
            for kc in range(OCH):
                nx.tensor.matmul(ps[:], tew[:, kc, bass.ts(mc, 128)],
                                 mfm[:, kc, :],
                                 start=(kc == 0), stop=(kc == OCH - 1))
            nx.scalar.activation(reprs[:, mc, :], ps[:], AF.Gelu,
                                 bias=teb[:, mc:mc + 1])
        # concat [771 padded 896, BC]; rows s*256+o2 from reprs, 768+s from wm
        nx.vector.memset(ccat[:], 0.0)
        for s in range(S):
            for oc in range(OCH):
                nx.sync.dma_start(ccat[:, s * OCH + oc, :],
                                  reprs[:, oc, s * BC:(s + 1) * BC])
        nx.sync.dma_start(ccat[0:S, KCC - 1, :], dram["w_wmbc"][:])
        pw = pps.tile([S, BC], F32, tag="ppsmall")
        for kc in range(KCC):
            nx.tensor.matmul(pw[:], wcw[:, kc, :], ccat[:, kc, :],
                             start=(kc == 0), stop=(kc == KCC - 1))
        nx.scalar.activation(th_sb[:], pw[:], AF.Tanh, bias=wcb[0:S, 0:1])

        tc.no_sync_barrier()
        # ---------------- Phase G: softplus + scale softmax + entropy ------
        ee = sm.tile([S, BC], F32, tag="ee")
        nx.scalar.activation(ee[:], th_sb[:], AF.Exp)
        nx.vector.tensor_scalar(ee[:], ee[:], 1.0, None, op0=ALU.add)
        raw = sm.tile([S, BC], F32, tag="raw")
        nx.scalar.activation(raw[:], ee[:], AF.Ln)
        nx.sync.dma_start(tr_bounce[:], raw[:])
        rawt = sm.tile([BC, S], F32, tag="rawt")
        nx.sync.dma_start(rawt[:], tr_bounce[:].transpose([1, 0]))
        ex = sm.tile([BC, S], F32, tag="ex")
        nx.scalar.activation(ex[:], rawt[:], AF.Exp)
        ssum2 = sm.tile([BC, 1], F32, tag="ssum2")
        nx.vector.reduce_sum(ssum2[:], ex[:], axis=AX.X)
        rcp2 = sm.tile([BC, 1], F32, tag="rcp2")
        nx.vector.reciprocal(rcp2[:], ssum2[:])
        swt = sm.tile([BC, S], F32, tag="swt")
        nx.vector.tensor_scalar(swt[:], ex[:], rcp2[:], None, op0=ALU.mult)
        nx.sync.dma_start(sw_bounce[:], swt[:])
        nx.sync.dma_start(swb[:], sw_bounce[:].partition_broadcast(128))
        lns_all = sm.tile([128, TCH * NT], F32, tag="lns_all")
        nx.scalar.activation(lns_all[:], ssum_st[:], AF.Ln)
        ent_all = sm.tile([128, TCH * NT], F32, tag="ent_all")
        nx.vector.tensor_tensor(ent_all[:], lns_all[:], spl_st[:],
                                op=ALU.subtract)
        entv = sm.tile([128, 1], F32, tag="entv")
        nx.vector.reduce_sum(entv[:], ent_all[:], axis=AX.X)
        pe_ = pps.tile([1, 1], F32, tag="ppsmall")
        nx.tensor.matmul(pe_[:], onesf[:], entv[:], start=True, stop=True)
        esb = sm.tile([1, 1], F32, tag="esb")
        nx.vector.tensor_copy(esb[:], pe_[:])
        nx.sync.dma_start(ent_ext[:], esb[:])

        tc.no_sync_barrier()
        # ---------------- Phase OUT: scale-weighted sum + output MLP -------
        for b in range(BC):
            wacc = acc.tile([128, TCH, O], BF16, tag="wacc")
            for tc_ in range(TCH):
                tmp = sm.tile([128, O], F32, tag="wtmp")
                nx.vector.tensor_scalar(
                    tmp[:], pst[:, tc_, 0 * BC + b, :],
                    swb[:, b * S:b * S + 1], None, op0=ALU.mult)
                nx.vector.scalar_tensor_tensor(
                    tmp[:], pst[:, tc_, 1 * BC + b, :],
                    swb[:, b * S + 1:b * S + 2], tmp[:],
                    op0=ALU.mult, op1=ALU.add)
                nx.vector.scalar_tensor_tensor(
                    wacc[:, tc_, :], pst[:, tc_, 2 * BC + b, :],
                    swb[:, b * S + 2:b * S + 3], tmp[:],
                    op0=ALU.mult, op1=ALU.add)
            wfm = ghp.tile([128, OCH, T], BF16, tag="wfm")
            for tc_ in range(TCH):
                for oc in range(OCH):
                    pt = pp256.tile([128, 128], BF16, tag="pexp")
                    nx.tensor.transpose(pt[:], wacc[:, tc_, bass.ts(oc, 128)],
                                        ident[:])
                    nx.vector.tensor_copy(wfm[:, oc, bass.ts(tc_, 128)], pt[:])
            o1 = ghp.tile([128, OCH, T], BF16, tag="o1")
            for mc in range(OCH):
                ps = pp512.tile([128, T], F32, tag="ps512")
                for kc in range(OCH):
                    nx.tensor.matmul(ps[:], ow1[:, kc, bass.ts(mc, 128)],
                                     wfm[:, kc, :],
                                     start=(kc == 0), stop=(kc == OCH - 1))
                nx.scalar.activation(o1[:, mc, :], ps[:], AF.Gelu,
                                     bias=ob1[:, mc:mc + 1])
            for tc_ in range(TCH):
                pf = pp256.tile([128, O], F32, tag="pexp")
                for kc in range(OCH):
                    nx.tensor.matmul(pf[:], o1[:, kc, bass.ts(tc_, 128)],
                                     ow2[:, kc, :],
                                     start=(kc == 0), stop=(kc == OCH - 1))
                if have_ob2:
                    nx.vector.tensor_tensor(pf[:], pf[:], ob2bc[:], op=ALU.add)
                osb = sm.tile([128, O], F32, tag="osb")
                nx.vector.tensor_copy(osb[:], pf[:])
                nx.sync.dma_start(out_ext[b, bass.ts(tc_, 128), :], osb[:])

    nc.compile()
    return nc


def prep_weights(inp):
    h = {}
    w1 = np.asarray(inp["router_w1"], np.float32)
    w1hi = w1.astype(_bf).astype(np.float32)
    h["w_w1hi"] = _to_bf(_feat_major(w1hi))
    h["w_w1lo"] = _to_bf(_feat_major(w1 - w1hi))
    h["w_rw2"] = _to_bf(_feat_major(np.asarray(inp["router_w2"], np.float32) * 0.5))
    rw2f = _feat_major(np.asarray(inp["router_w2"], np.float32) * 0.5)
    h["w_rw2lo"] = _to_bf(rw2f - rw2f.astype(_bf).astype(np.float32))
    if np.any(np.asarray(inp["router_b1"]) != 0):
        h["w_rb1s"] = _bias_chunks(np.asarray(inp["router_b1"]) * RT2)
    aW1 = np.concatenate([inp["gW1"], inp["lW1"]], 0)
    h["w_aW1"] = _to_bf(np.stack([_feat_major(aW1[n]) for n in range(NE)], 2))
    ab1 = np.concatenate([inp["gb1"], inp["lb1"]], 0)
    if np.any(ab1 != 0):
        h["w_ab1"] = np.ascontiguousarray(
            np.stack([_bias_chunks(ab1[n]) for n in range(NE)], 1))
    h["w_gW2"] = _to_bf(np.stack(
        [_feat_major(np.asarray(inp["gW2"])[n]) for n in range(NS)], 2))
    if np.any(np.asarray(inp["gb2"]) != 0):
        h["w_gb2"] = np.ascontiguousarray(np.stack(
            [_bias_chunks(np.asarray(inp["gb2"])[n]) for n in range(NS)], 1))
    aWo = np.concatenate([inp["gW3"], inp["lW2"]], 0)
    h["w_aWo"] = _to_bf(np.stack([_feat_major(aWo[n]) for n in range(NE)], 2))
    h["w_tew"] = _to_bf(_feat_major(np.asarray(inp["te_w"], np.float32) / C))
    h["w_teb"] = _bias_chunks(inp["te_b"])
    wcw = np.zeros((896, S), np.float32)
    wcw[:768 + S] = np.asarray(inp["wc_w"], np.float32)
    h["w_wcw"] = _to_bf(_feat_major(wcw))
    wcb = np.zeros((128, 1), np.float32)
    wcb[:S, 0] = np.asarray(inp["wc_b"])
    h["w_wcb"] = wcb
    h["w_ow1"] = _to_bf(_feat_major(inp["out_w1"]))
    h["w_ob1"] = _bias_chunks(inp["out_b1"])
    h["w_ow2"] = _to_bf(_feat_major(inp["out_w2"]))
    h["w_ident"] = _to_bf(np.eye(128, dtype=np.float32))
    h["w_wmbc"] = _to_bf(np.broadcast_to(
        np.asarray(inp["weight_memory"], np.float32).reshape(S, 1), (S, BC)))
    if np.any(np.asarray(inp["router_b2"]) != 0):
        h["w_rb2bc"] = np.ascontiguousarray(np.broadcast_to(
            np.asarray(inp["router_b2"], np.float32), (128, NE)))
    bout = np.concatenate([inp["gb3"], inp["lb2"]], 0)
    if np.any(bout != 0):
        h["w_boutbc"] = np.ascontiguousarray(np.broadcast_to(
            np.asarray(bout, np.float32).reshape(1, NE * O), (128, NE * O)))
    if np.any(np.asarray(inp["out_b2"]) != 0):
        h["w_ob2bc"] = np.ascontiguousarray(np.broadcast_to(
            np.asarray(inp["out_b2"], np.float32), (128, O)))
    return h


def prep_x(xs):
    """xs [S,B,C,E] f32 -> per-core (xhi, xlo), each [NT, ECH, 128, T] bf16."""
    out = []
    for c in range(NCORES):
        blk = np.asarray(xs, np.float32)[:, c * BC:(c + 1) * BC]
        xt = blk.transpose(0, 1, 3, 2).reshape(NT, ECH, 128, T)
        hi = xt.astype(_bf)
        lo = (xt - hi.astype(np.float32)).astype(_bf)
        out.append((np.ascontiguousarray(hi), np.ascontiguousarray(lo)))
    return out


_CACHE = {}


def build_in_maps(inputs):
    host = prep_weights(inputs)
    key = "graph:" + ",".join(sorted(host))
    if key not in _CACHE:
        _CACHE[key] = build_graph(host)
    nc = _CACHE[key]
    xs_shards = prep_x(inputs["xs"])
    in_maps = []
    for c in range(NCORES):
        m = dict(host)
        m["xhi"], m["xlo"] = xs_shards[c]
        in_maps.append(m)
    return nc, in_maps


def assemble(results):
    outs = np.concatenate(
        [results[c]["out"][None] for c in range(NCORES)], 0).reshape(B, C, O)
    ent_sum = sum(float(results[c]["ent"][0, 0]) for c in range(NCORES))
    bl = np.float32(0.1 * ent_sum / (S * B * C))
    return outs, bl


def kernel(**inputs):
    nc, in_maps = build_in_maps(inputs)
    res = run_bass_kernel_spmd(nc, in_maps, core_ids=list(range(NCORES)))
    return assemble(res.results)


# revision 25
# speedup vs baseline: 1.3391x; 1.0008x over previous
"""AdaptiveScaleRoutingMoE block on 8 TRN2 NeuronCores.

Strategy: data-parallel over batch (B=32 -> 4 per core). All weights
replicated. Per (scale, batch) pair one 512-token tile, 12 tiles/core.

Precision: router L1 in split-bf16 (hi/lo, 3 cross terms, exact gelu via the
Erf LUT), router L2 in f32 (top-2 selection is rank-sensitive), experts bf16
with f32 accumulation. The top-2-of-6 local-expert gather is replaced by a
mask built from the second max; the weighted combine runs token-major via
scalar_tensor_tensor with per-partition router-weight columns. Balance-loss
entropy uses ln(sum_exp) - sum(p*logit) (log-softmax identity), partial sums
reduced on host.
"""
import numpy as np
import ml_dtypes
from contextlib import ExitStack

import concourse.bass as bass
import concourse.tile as tile
from concourse import bacc, mybir
from concourse.bass_utils import run_bass_kernel_spmd

F32 = mybir.dt.float32
BF16 = mybir.dt.bfloat16
AF = mybir.ActivationFunctionType
ALU = mybir.AluOpType
AX = mybir.AxisListType

S, B, C, E = 3, 32, 512, 256
H, O, NE, NS, NL = 512, 256, 8, 2, 6
NCORES = 8
BC = B // NCORES          # batches per core
NT = S * BC               # token tiles per core (one per (s,b)), each T tokens
T = C                     # 512 tokens per tile
TCH = T // 128            # 4 token chunks
ECH = E // 128            # 2
HCH = H // 128            # 4
OCH = O // 128            # 2
KCC = 896 // 128          # 7 concat chunks (771 padded to 896)
RT2 = float(1.0 / np.sqrt(2.0))

_bf = ml_dtypes.bfloat16


def _to_bf(a):
    return np.ascontiguousarray(np.asarray(a, np.float32).astype(_bf))


def _feat_major(w):
    """[K, M] weight -> SBUF lhsT layout [128, K/128, M]."""
    k, m = w.shape
    return np.ascontiguousarray(
        np.asarray(w, np.float32).reshape(k // 128, 128, m).transpose(1, 0, 2))


def _bias_chunks(b):
    """[F] bias -> [128, F/128] per-partition layout."""
    f = b.shape[0]
    return np.ascontiguousarray(np.asarray(b, np.float32).reshape(f // 128, 128).T)


def build_graph(host):
    """host: dict of prepped numpy weight arrays. Returns compiled nc."""
    nc = bacc.Bacc("TRN2", target_bir_lowering=False, debug=False,
                   num_devices=NCORES)

    dram = {}

    def din(name, shape, dt):
        dram[name] = nc.dram_tensor(name, list(shape), dt, kind="ExternalInput")
        return dram[name]

    din("xhi", [NT, ECH, 128, T], BF16)
    din("xlo", [NT, ECH, 128, T], BF16)
    for k, v in host.items():
        din(k, v.shape, BF16 if v.dtype == _bf else F32)

    out_ext = nc.dram_tensor("out", [BC, C, O], F32, kind="ExternalOutput")
    ent_ext = nc.dram_tensor("ent", [1, 1], F32, kind="ExternalOutput")

    have_rb2 = "w_rb2bc" in host
    have_bout = "w_boutbc" in host
    have_ob2 = "w_ob2bc" in host

    with tile.TileContext(nc) as tc, ExitStack() as ctx:
        nx = nc
        wp = ctx.enter_context(tc.tile_pool(name="weights", bufs=1))
        xp = ctx.enter_context(tc.tile_pool(name="xstash", bufs=1))
        stash = ctx.enter_context(tc.tile_pool(name="stash", bufs=1))
        hwork = ctx.enter_context(tc.tile_pool(name="hwork", bufs=2))
        ewp = ctx.enter_context(tc.tile_pool(name="ewp", bufs=1))
        ghp = ctx.enter_context(tc.tile_pool(name="ghp", bufs=1))
        sm = ctx.enter_context(tc.tile_pool(name="small", bufs=3))
        hsp = ctx.enter_context(tc.tile_pool(name="hsplit", bufs=1))
        acc = ctx.enter_context(tc.tile_pool(name="accp", bufs=2))
        drp = ctx.enter_context(tc.tile_pool(name="drbounce", bufs=1, space="DRAM"))
        pp512 = ctx.enter_context(tc.tile_pool(name="pp512", bufs=2, space="PSUM"))
        pp256 = ctx.enter_context(tc.tile_pool(name="pp256", bufs=3, space="PSUM"))
        pps = ctx.enter_context(tc.tile_pool(name="pps", bufs=1, space="PSUM"))

        def wt(name, dt=BF16):
            a = host[name]
            t_ = wp.tile(list(a.shape), dt, tag=name)
            nx.sync.dma_start(t_[:], dram[name][:])
            return t_

        # DMA priority order: what phase R needs first
        w1hi = wt("w_w1hi"); w1lo = wt("w_w1lo")
        rw2 = wt("w_rw2")
        rw2lo = wt("w_rw2lo")
        have_rb1 = "w_rb1s" in host
        have_ab1 = "w_ab1" in host
        have_gb2 = "w_gb2" in host
        rb1s = wt("w_rb1s", F32) if have_rb1 else None
        rb2bc = wt("w_rb2bc", F32) if have_rb2 else None
        xhi = xp.tile([128, ECH, NT, T], BF16, tag="xhi")
        for i in range(NT):
            for ec in range(ECH):
                nx.sync.dma_start(xhi[:, ec, i, :], dram["xhi"][i, ec])
        onesb = wp.tile([128, 1], BF16, tag="onesb")
        nx.vector.memset(onesb[:], 1.0)
        onesf = wp.tile([128, 1], F32, tag="onesf")
        nx.vector.memset(onesf[:], 1.0)

        logit_st = stash.tile([128, TCH, NT, NE], F32, tag="logit")
        wst = stash.tile([128, TCH, NT, NE], F32, tag="wst")
        ssum_st = stash.tile([128, TCH * NT], F32, tag="ssum_st")
        spl_st = stash.tile([128, TCH * NT], F32, tag="spl_st")
        pst = stash.tile([128, TCH, NT, O], BF16, tag="pst")
        mlin = stash.tile([1, NT, O], F32, tag="mlin")
        mfm = stash.tile([128, OCH, NT], BF16, tag="mfm")
        reprs = stash.tile([128, OCH, NT], BF16, tag="reprs")
        ccat = stash.tile([128, KCC, BC], BF16, tag="ccat")
        th_sb = stash.tile([S, BC], F32, tag="th")
        swb = stash.tile([128, BC * S], F32, tag="swb")
        sw_bounce = drp.tile([BC * S], F32, tag="swb_d")
        tr_bounce = drp.tile([S, BC], F32, tag="trb_d")
        mean_bounce = drp.tile([NT, O], F32, tag="meanb_d")

        # ---------------- Phase R: router L1 (Erf) + L2 (f32) --------------
        xlo_tiles = {}
        def fetch_xlo(i):
            t_ = hwork.tile([128, ECH, T], BF16, tag="xlo")
            for ec in range(ECH):
                nx.sync.dma_start(t_[:, ec, :], dram["xlo"][i, ec])
            xlo_tiles[i] = t_
        fetch_xlo(0)
        for i in range(NT):
            if i + 1 < NT:
                fetch_xlo(i + 1)
            xlo_t = xlo_tiles.pop(i)
            h1 = hwork.tile([128, HCH, T], F32, tag="h1")
            for mcp in range(HCH // 2):
                ps = pp512.tile([128, 2, T], F32, tag="ps512")
                for mh in range(2):
                    mc = mcp * 2 + mh
                    terms = [(w1hi, xhi[:, ec, i, :]) for ec in range(ECH)]
                    terms += [(w1lo, xhi[:, ec, i, :]) for ec in range(ECH)]
                    terms += [(w1hi, xlo_t[:, ec, :]) for ec in range(ECH)]
                    n_terms = len(terms)
                    for j, (lhs, r) in enumerate(terms):
                        ec = j % ECH
                        nx.tensor.matmul(
                            ps[:, mh, :], lhs[:, ec, bass.ts(mc, 128)], r,
                            start=(j == 0), stop=(j == n_terms - 1))
                erf = hwork.tile([128, 2, T], F32, tag="erf")
                # erf((z + b1)/sqrt(2)); z in psum; rb1s = b1/sqrt(2).
                if have_rb1:
                    for mh in range(2):
                        nx.scalar.activation(
                            erf[:, mh, :], ps[:, mh, :], AF.Erf,
                            bias=rb1s[:, mcp * 2 + mh:mcp * 2 + mh + 1],
                            scale=RT2)
                else:
                    nx.scalar.activation(erf[:], ps[:], AF.Erf, scale=RT2)
                # h1 = (erf + 1) * z = 2*gelu(z)  (0.5 folded into rw2)
                nx.vector.scalar_tensor_tensor(
                    h1[:, mcp * 2:mcp * 2 + 2, :], erf[:], 1.0, ps[:],
                    op0=ALU.add, op1=ALU.mult)
            h1hi = hsp.tile([128, HCH, T], BF16, tag="h1hi")
            nx.vector.tensor_copy(h1hi[:], h1[:])
            h1lo = hsp.tile([128, HCH, T], BF16, tag="h1lo")
            nx.vector.tensor_tensor(h1lo[:], h1[:], h1hi[:], op=ALU.subtract)
            for tc_ in range(TCH):
                pl = pps.tile([128, NE], F32, tag="ppsmall")
                nmm = 3 * HCH
                j = 0
                for kc in range(HCH):
                    for lhs, r in ((h1hi, rw2), (h1lo, rw2), (h1hi, rw2lo)):
                        nx.tensor.matmul(
                            pl[:], lhs[:, kc, bass.ts(tc_, 128)], r[:, kc, :],
                            start=(j == 0), stop=(j == nmm - 1))
                        j += 1
                if have_rb2:
                    nx.vector.tensor_tensor(pl[:], pl[:], rb2bc[:, :NE],
                                            op=ALU.add)
                nx.vector.tensor_copy(logit_st[:, tc_, i, :], pl[:])

        # bulk weights: issued after R's x DMAs so they don't starve phase R
        aW1 = wt("w_aW1")
        ab1 = wt("w_ab1", F32) if have_ab1 else None
        gW2 = wt("w_gW2")
        gb2 = wt("w_gb2", F32) if have_gb2 else None
        aWo = wt("w_aWo")
        tew = wt("w_tew"); teb = wt("w_teb", F32)
        wcw = wt("w_wcw"); wcb = wt("w_wcb", F32)
        ow1 = wt("w_ow1"); ob1 = wt("w_ob1", F32)
        ow2 = wt("w_ow2")
        ident = wt("w_ident")
        boutbc = wt("w_boutbc", F32) if have_bout else None
        ob2bc = wt("w_ob2bc", F32) if have_ob2 else None

        tc.no_sync_barrier()
        # ---------------- Phase W: batched softmax + entropy + top-2 -------
        # All 48 chunks at once on [128, TCH, NT, *] views of the stashes.
        ssum3 = ssum_st[:].rearrange("p (a b) -> p a b", a=TCH)
        spl3 = spl_st[:].rearrange("p (a b) -> p a b", a=TCH)
        nx.scalar.activation(wst[:], logit_st[:], AF.Exp)
        nx.vector.reduce_sum(ssum3, wst[:], axis=AX.X)
        rcpa = sm.tile([128, TCH, NT, 1], F32, tag="rcpa")
        nx.vector.reciprocal(rcpa[:], ssum3)
        nx.vector.tensor_tensor(
            wst[:], wst[:], rcpa[:].broadcast_to([128, TCH, NT, NE]),
            op=ALU.mult)
        pl8a = sm.tile([128, TCH, NT, NE], F32, tag="pl8a")
        nx.vector.tensor_tensor(pl8a[:], wst[:], logit_st[:], op=ALU.mult)
        nx.vector.reduce_sum(spl3, pl8a[:], axis=AX.X)
        wl = wst[:, :, :, NS:NE]
        m1a = sm.tile([128, TCH, NT, 1], F32, tag="rcpa")
        nx.vector.reduce_max(m1a[:], wl, axis=AX.X)
        eqa = sm.tile([128, TCH, NT, NL], F32, tag="pl8a")
        nx.vector.tensor_tensor(
            eqa[:], wl, m1a[:].broadcast_to([128, TCH, NT, NL]),
            op=ALU.is_equal)
        w2a = sm.tile([128, TCH, NT, NL], F32, tag="w2a")
        nx.vector.scalar_tensor_tensor(
            w2a[:], eqa[:], -1e30, wl, op0=ALU.mult, op1=ALU.add)
        m2a = sm.tile([128, TCH, NT, 1], F32, tag="rcpa")
        nx.vector.reduce_max(m2a[:], w2a[:], axis=AX.X)
        sela = sm.tile([128, TCH, NT, NL], F32, tag="pl8a")
        nx.vector.tensor_tensor(
            sela[:], wl, m2a[:].broadcast_to([128, TCH, NT, NL]),
            op=ALU.is_ge)
        nx.vector.tensor_tensor(wl, wl, sela[:], op=ALU.mult)

        tc.no_sync_barrier()
        # ---------------- Phase E: experts (Gelu) + combine + means --------
        # Experts run in two half-groups of 4 so the eh buffer double-buffers
        # across tiles (cross-tile pipelining) at no extra SBUF cost.
        for i in range(NT):
            acq = acc.tile([128, TCH, O], F32, tag="acc4")
            for half in range(2):
                eh = ewp.tile([128, 4, HCH, T], BF16, tag="eh")
                for nn in range(4):
                    n = half * 4 + nn
                    for mcp in range(HCH // 2):
                        ps = pp512.tile([128, 2, T], F32, tag="ps512")
                        for mh in range(2):
                            mc = mcp * 2 + mh
                            for ec in range(ECH):
                                nx.tensor.matmul(
                                    ps[:, mh, :],
                                    aW1[:, ec, n, bass.ts(mc, 128)],
                                    xhi[:, ec, i, :],
                                    start=(ec == 0), stop=(ec == ECH - 1))
                        if have_ab1:
                            for mh in range(2):
                                nx.scalar.activation(
                                    eh[:, nn, mcp * 2 + mh, :], ps[:, mh, :],
                                    AF.Gelu,
                                    bias=ab1[:, n,
                                             mcp * 2 + mh:mcp * 2 + mh + 1])
                        else:
                            nx.scalar.activation(
                                eh[:, nn, mcp * 2:mcp * 2 + 2, :], ps[:],
                                AF.Gelu)
                if half == 0:
                    gh2 = ghp.tile([128, NS, HCH, T], BF16, tag="gh2")
                    for n in range(NS):
                        for mcp in range(HCH // 2):
                            ps = pp512.tile([128, 2, T], F32, tag="ps512")
                            for mh in range(2):
                                mc = mcp * 2 + mh
                                for kc in range(HCH):
                                    nx.tensor.matmul(
                                        ps[:, mh, :],
                                        gW2[:, kc, n, bass.ts(mc, 128)],
                                        eh[:, n, kc, :],
                                        start=(kc == 0), stop=(kc == HCH - 1))
                            if have_gb2:
                                for mh in range(2):
                                    nx.scalar.activation(
                                        gh2[:, n, mcp * 2 + mh, :],
                                        ps[:, mh, :], AF.Gelu,
                                        bias=gb2[:, n,
                                                 mcp * 2 + mh:mcp * 2 + mh + 1])
                            else:
                                nx.scalar.activation(
                                    gh2[:, n, mcp * 2:mcp * 2 + 2, :], ps[:],
                                    AF.Gelu)
                for tc_ in range(TCH):
                    for nn in range(4):
                        n = half * 4 + nn
                        pn = pp256.tile([128, O], F32, tag="pexp")
                        for kc in range(HCH):
                            if n < NS:
                                lhs = gh2[:, n, kc, bass.ts(tc_, 128)]
                            else:
                                lhs = eh[:, nn, kc, bass.ts(tc_, 128)]
                            nx.tensor.matmul(
                                pn[:], lhs, aWo[:, kc, n, :],
                                start=(kc == 0), stop=(kc == HCH - 1))
                        if have_bout:
                            nx.vector.tensor_tensor(
                                pn[:], pn[:], boutbc[:, bass.ts(n, O)],
                                op=ALU.add)
                        if n == 0:
                            nx.vector.tensor_scalar(
                                acq[:, tc_, :], pn[:], wst[:, tc_, i, 0:1],
                                None, op0=ALU.mult)
                        else:
                            nx.vector.scalar_tensor_tensor(
                                acq[:, tc_, :], pn[:], wst[:, tc_, i, n:n + 1],
                                acq[:, tc_, :], op0=ALU.mult, op1=ALU.add)
            nx.vector.tensor_copy(pst[:, :, i, :], acq[:])
            pm = pps.tile([1, O], F32, tag="ppsmall")
            for tc_ in range(TCH):
                nx.tensor.matmul(pm[:], onesb[:], pst[:, tc_, i, :],
                                 start=(tc_ == 0), stop=(tc_ == TCH - 1))
            nx.vector.tensor_copy(mlin[:, i, :], pm[:])
        # means -> feature-major [128, OCH, NT] via DRAM bounce
        nx.sync.dma_start(mean_bounce[:], mlin[0, :, :])
        mfm_f = acc.tile([128, OCH, NT], F32, tag="mfmf")
        for oc in range(OCH):
            nx.sync.dma_start(
                mfm_f[:, oc, :],
                mean_bounce[:, oc * 128:(oc + 1) * 128].transpose([1, 0]))
        nx.vector.tensor_copy(mfm[:], mfm_f[:])
        for mc in range(OCH):
            ps = pps.tile([128, NT], F32, tag="ppsmall")
            for kc in range(OCH):
                nx.tensor.matmul(ps[:], tew[:, kc, bass.ts(mc, 128)],
                                 mfm[:, kc, :],
                                 start=(kc == 0), stop=(kc == OCH - 1))
            nx.scalar.activation(reprs[:, mc, :], ps[:], AF.Gelu,
                                 bias=teb[:, mc:mc + 1])
        nx.vector.memset(ccat[:], 0.0)
        for s in range(S):
            for oc in range(OCH):
                nx.sync.dma_start(ccat[:, s * OCH + oc, :],
                                  reprs[:, oc, s * BC:(s + 1) * BC])
        nx.sync.dma_start(ccat[0:S, KCC - 1, :], dram["w_wmbc"][:])
        pw = pps.tile([S, BC], F32, tag="ppsmall")
        for kc in range(KCC):
            nx.tensor.matmul(pw[:], wcw[:, kc, :], ccat[:, kc, :],
                             start=(kc == 0), stop=(kc == KCC - 1))
        nx.scalar.activation(th_sb[:], pw[:], AF.Tanh, bias=wcb[0:S, 0:1])

        tc.no_sync_barrier()
        # ---------------- Phase G: softplus + scale softmax ---------------
        ee = sm.tile([S, BC], F32, tag="ee")
        nx.scalar.activation(ee[:], th_sb[:], AF.Exp)
        nx.vector.tensor_scalar(ee[:], ee[:], 1.0, None, op0=ALU.add)
        raw = sm.tile([S, BC], F32, tag="raw")
        nx.scalar.activation(raw[:], ee[:], AF.Ln)
        nx.sync.dma_start(tr_bounce[:], raw[:])
        rawt = sm.tile([BC, S], F32, tag="rawt")
        nx.sync.dma_start(rawt[:], tr_bounce[:].transpose([1, 0]))
        ex = sm.tile([BC, S], F32, tag="ex")
        nx.scalar.activation(ex[:], rawt[:], AF.Exp)
        ssum2 = sm.tile([BC, 1], F32, tag="ssum2")
        nx.vector.reduce_sum(ssum2[:], ex[:], axis=AX.X)
        rcp2 = sm.tile([BC, 1], F32, tag="rcp2")
        nx.vector.reciprocal(rcp2[:], ssum2[:])
        swt = sm.tile([BC, S], F32, tag="swt")
        nx.vector.tensor_scalar(swt[:], ex[:], rcp2[:], None, op0=ALU.mult)
        nx.sync.dma_start(sw_bounce[:], swt[:])
        nx.sync.dma_start(swb[:], sw_bounce[:].partition_broadcast(128))
        lns_all = sm.tile([128, TCH * NT], F32, tag="lns_all")
        nx.scalar.activation(lns_all[:], ssum_st[:], AF.Ln)
        ent_all = sm.tile([128, TCH * NT], F32, tag="ent_all")
        nx.vector.tensor_tensor(ent_all[:], lns_all[:], spl_st[:],
                                op=ALU.subtract)
        entv = sm.tile([128, 1], F32, tag="entv")
        nx.vector.reduce_sum(entv[:], ent_all[:], axis=AX.X)
        pe_ = pps.tile([1, 1], F32, tag="ppsmall")
        nx.tensor.matmul(pe_[:], onesf[:], entv[:], start=True, stop=True)
        esb = sm.tile([1, 1], F32, tag="esb")
        nx.vector.tensor_copy(esb[:], pe_[:])
        nx.sync.dma_start(ent_ext[:], esb[:])

        tc.no_sync_barrier()
        # ---------------- Phase OUT: scale-weighted sum + output MLP -------
        for b in range(BC):
            wacc = acc.tile([128, TCH, O], BF16, tag="wacc")
            for tc_ in range(TCH):
                tmp = sm.tile([128, O], F32, tag="wtmp")
                nx.vector.tensor_scalar(
                    tmp[:], pst[:, tc_, 0 * BC + b, :],
                    swb[:, b * S:b * S + 1], None, op0=ALU.mult)
                nx.vector.scalar_tensor_tensor(
                    tmp[:], pst[:, tc_, 1 * BC + b, :],
                    swb[:, b * S + 1:b * S + 2], tmp[:],
                    op0=ALU.mult, op1=ALU.add)
                nx.vector.scalar_tensor_tensor(
                    wacc[:, tc_, :], pst[:, tc_, 2 * BC + b, :],
                    swb[:, b * S + 2:b * S + 3], tmp[:],
                    op0=ALU.mult, op1=ALU.add)
            wfm = ghp.tile([128, OCH, T], BF16, tag="wfm")
            for tc_ in range(TCH):
                for oc in range(OCH):
                    pt = pp256.tile([128, 128], BF16, tag="pexp")
                    nx.tensor.transpose(pt[:], wacc[:, tc_, bass.ts(oc, 128)],
                                        ident[:])
                    nx.vector.tensor_copy(wfm[:, oc, bass.ts(tc_, 128)], pt[:])
            o1 = ghp.tile([128, OCH, T], BF16, tag="o1")
            for mc in range(OCH):
                ps = pp512.tile([128, T], F32, tag="ps512")
                for kc in range(OCH):
                    nx.tensor.matmul(ps[:], ow1[:, kc, bass.ts(mc, 128)],
                                     wfm[:, kc, :],
                                     start=(kc == 0), stop=(kc == OCH - 1))
                nx.scalar.activation(o1[:, mc, :], ps[:], AF.Gelu,
                                     bias=ob1[:, mc:mc + 1])
            for tc_ in range(TCH):
                pf = pp256.tile([128, O], F32, tag="pexp")
                for kc in range(OCH):
                    nx.tensor.matmul(pf[:], o1[:, kc, bass.ts(tc_, 128)],
                                     ow2[:, kc, :],
                                     start=(kc == 0), stop=(kc == OCH - 1))
                if have_ob2:
                    nx.vector.tensor_tensor(pf[:], pf[:], ob2bc[:], op=ALU.add)
                osb = sm.tile([128, O], F32, tag="osb")
                nx.vector.tensor_copy(osb[:], pf[:])
                nx.sync.dma_start(out_ext[b, bass.ts(tc_, 128), :], osb[:])

    nc.compile()
    return nc


def prep_weights(inp):
    h = {}
    w1 = np.asarray(inp["router_w1"], np.float32)
    w1hi = w1.astype(_bf).astype(np.float32)
    h["w_w1hi"] = _to_bf(_feat_major(w1hi))
    h["w_w1lo"] = _to_bf(_feat_major(w1 - w1hi))
    h["w_rw2"] = _to_bf(_feat_major(np.asarray(inp["router_w2"], np.float32) * 0.5))
    rw2f = _feat_major(np.asarray(inp["router_w2"], np.float32) * 0.5)
    h["w_rw2lo"] = _to_bf(rw2f - rw2f.astype(_bf).astype(np.float32))
    if np.any(np.asarray(inp["router_b1"]) != 0):
        h["w_rb1s"] = _bias_chunks(np.asarray(inp["router_b1"]) * RT2)
    aW1 = np.concatenate([inp["gW1"], inp["lW1"]], 0)
    h["w_aW1"] = _to_bf(np.stack([_feat_major(aW1[n]) for n in range(NE)], 2))
    ab1 = np.concatenate([inp["gb1"], inp["lb1"]], 0)
    if np.any(ab1 != 0):
        h["w_ab1"] = np.ascontiguousarray(
            np.stack([_bias_chunks(ab1[n]) for n in range(NE)], 1))
    h["w_gW2"] = _to_bf(np.stack(
        [_feat_major(np.asarray(inp["gW2"])[n]) for n in range(NS)], 2))
    if np.any(np.asarray(inp["gb2"]) != 0):
        h["w_gb2"] = np.ascontiguousarray(np.stack(
            [_bias_chunks(np.asarray(inp["gb2"])[n]) for n in range(NS)], 1))
    aWo = np.concatenate([inp["gW3"], inp["lW2"]], 0)
    h["w_aWo"] = _to_bf(np.stack([_feat_major(aWo[n]) for n in range(NE)], 2))
    h["w_tew"] = _to_bf(_feat_major(np.asarray(inp["te_w"], np.float32) / C))
    h["w_teb"] = _bias_chunks(inp["te_b"])
    wcw = np.zeros((896, S), np.float32)
    wcw[:768 + S] = np.asarray(inp["wc_w"], np.float32)
    h["w_wcw"] = _to_bf(_feat_major(wcw))
    wcb = np.zeros((128, 1), np.float32)
    wcb[:S, 0] = np.asarray(inp["wc_b"])
    h["w_wcb"] = wcb
    h["w_ow1"] = _to_bf(_feat_major(inp["out_w1"]))
    h["w_ob1"] = _bias_chunks(inp["out_b1"])
    h["w_ow2"] = _to_bf(_feat_major(inp["out_w2"]))
    h["w_ident"] = _to_bf(np.eye(128, dtype=np.float32))
    h["w_wmbc"] = _to_bf(np.broadcast_to(
        np.asarray(inp["weight_memory"], np.float32).reshape(S, 1), (S, BC)))
    if np.any(np.asarray(inp["router_b2"]) != 0):
        h["w_rb2bc"] = np.ascontiguousarray(np.broadcast_to(
            np.asarray(inp["router_b2"], np.float32), (128, NE)))
    bout = np.concatenate([inp["gb3"], inp["lb2"]], 0)
    if np.any(bout != 0):
        h["w_boutbc"] = np.ascontiguousarray(np.broadcast_to(
            np.asarray(bout, np.float32).reshape(1, NE * O), (128, NE * O)))
    if np.any(np.asarray(inp["out_b2"]) != 0):
        h["w_ob2bc"] = np.ascontiguousarray(np.broadcast_to(
            np.asarray(inp["out_b2"], np.float32), (128, O)))
    return h


def prep_x(xs):
    """xs [S,B,C,E] f32 -> per-core (xhi, xlo), each [NT, ECH, 128, T] bf16."""
    out = []
    for c in range(NCORES):
        blk = np.asarray(xs, np.float32)[:, c * BC:(c + 1) * BC]
        xt = blk.transpose(0, 1, 3, 2).reshape(NT, ECH, 128, T)
        hi = xt.astype(_bf)
        lo = (xt - hi.astype(np.float32)).astype(_bf)
        out.append((np.ascontiguousarray(hi), np.ascontiguousarray(lo)))
    return out


_CACHE = {}


def build_in_maps(inputs):
    host = prep_weights(inputs)
    key = "graph:" + ",".join(sorted(host))
    if key not in _CACHE:
        _CACHE[key] = build_graph(host)
    nc = _CACHE[key]
    xs_shards = prep_x(inputs["xs"])
    in_maps = []
    for c in range(NCORES):
        m = dict(host)
        m["xhi"], m["xlo"] = xs_shards[c]
        in_maps.append(m)
    return nc, in_maps


def assemble(results):
    outs = np.concatenate(
        [results[c]["out"][None] for c in range(NCORES)], 0).reshape(B, C, O)
    ent_sum = sum(float(results[c]["ent"][0, 0]) for c in range(NCORES))
    bl = np.float32(0.1 * ent_sum / (S * B * C))
    return outs, bl


def kernel(**inputs):
    nc, in_maps = build_in_maps(inputs)
    res = run_bass_kernel_spmd(nc, in_maps, core_ids=list(range(NCORES)))
    return assemble(res.results)


# revision 26
# speedup vs baseline: 1.3398x; 1.0006x over previous
"""AdaptiveScaleRoutingMoE block on 8 TRN2 NeuronCores.

Strategy: data-parallel over batch (B=32 -> 4 per core). All weights
replicated. Per (scale, batch) pair one 512-token tile, 12 tiles/core.

Precision: router L1 in split-bf16 (hi/lo, 3 cross terms, exact gelu via the
Erf LUT), router L2 in f32 (top-2 selection is rank-sensitive), experts bf16
with f32 accumulation. The top-2-of-6 local-expert gather is replaced by a
mask built from the second max; the weighted combine runs token-major via
scalar_tensor_tensor with per-partition router-weight columns. Balance-loss
entropy uses ln(sum_exp) - sum(p*logit) (log-softmax identity), partial sums
reduced on host.
"""
import numpy as np
import ml_dtypes
from contextlib import ExitStack

import concourse.bass as bass
import concourse.tile as tile
from concourse import bacc, mybir
from concourse.bass_utils import run_bass_kernel_spmd

F32 = mybir.dt.float32
BF16 = mybir.dt.bfloat16
AF = mybir.ActivationFunctionType
ALU = mybir.AluOpType
AX = mybir.AxisListType

S, B, C, E = 3, 32, 512, 256
H, O, NE, NS, NL = 512, 256, 8, 2, 6
NCORES = 8
BC = B // NCORES          # batches per core
NT = S * BC               # token tiles per core (one per (s,b)), each T tokens
T = C                     # 512 tokens per tile
TCH = T // 128            # 4 token chunks
ECH = E // 128            # 2
HCH = H // 128            # 4
OCH = O // 128            # 2
KCC = 896 // 128          # 7 concat chunks (771 padded to 896)
RT2 = float(1.0 / np.sqrt(2.0))

_bf = ml_dtypes.bfloat16


def _to_bf(a):
    return np.ascontiguousarray(np.asarray(a, np.float32).astype(_bf))


def _feat_major(w):
    """[K, M] weight -> SBUF lhsT layout [128, K/128, M]."""
    k, m = w.shape
    return np.ascontiguousarray(
        np.asarray(w, np.float32).reshape(k // 128, 128, m).transpose(1, 0, 2))


def _bias_chunks(b):
    """[F] bias -> [128, F/128] per-partition layout."""
    f = b.shape[0]
    return np.ascontiguousarray(np.asarray(b, np.float32).reshape(f // 128, 128).T)


def build_graph(host):
    """host: dict of prepped numpy weight arrays. Returns compiled nc."""
    nc = bacc.Bacc("TRN2", target_bir_lowering=False, debug=False,
                   num_devices=NCORES)

    dram = {}

    def din(name, shape, dt):
        dram[name] = nc.dram_tensor(name, list(shape), dt, kind="ExternalInput")
        return dram[name]

    din("xhi", [NT, ECH, 128, T], BF16)
    din("xlo", [NT, ECH, 128, T], BF16)
    for k, v in host.items():
        din(k, v.shape, BF16 if v.dtype == _bf else F32)

    out_ext = nc.dram_tensor("out", [BC, C, O], F32, kind="ExternalOutput")
    ent_ext = nc.dram_tensor("ent", [1, 1], F32, kind="ExternalOutput")

    have_rb2 = "w_rb2bc" in host
    have_bout = "w_boutbc" in host
    have_ob2 = "w_ob2bc" in host

    with tile.TileContext(nc) as tc, ExitStack() as ctx:
        nx = nc
        wp = ctx.enter_context(tc.tile_pool(name="weights", bufs=1))
        xp = ctx.enter_context(tc.tile_pool(name="xstash", bufs=1))
        stash = ctx.enter_context(tc.tile_pool(name="stash", bufs=1))
        hwork = ctx.enter_context(tc.tile_pool(name="hwork", bufs=2))
        ewp = ctx.enter_context(tc.tile_pool(name="ewp", bufs=1))
        ghp = ctx.enter_context(tc.tile_pool(name="ghp", bufs=1))
        sm = ctx.enter_context(tc.tile_pool(name="small", bufs=3))
        hsp = ctx.enter_context(tc.tile_pool(name="hsplit", bufs=1))
        acc = ctx.enter_context(tc.tile_pool(name="accp", bufs=2))
        drp = ctx.enter_context(tc.tile_pool(name="drbounce", bufs=1, space="DRAM"))
        pp512 = ctx.enter_context(tc.tile_pool(name="pp512", bufs=2, space="PSUM"))
        pp256 = ctx.enter_context(tc.tile_pool(name="pp256", bufs=3, space="PSUM"))
        pps = ctx.enter_context(tc.tile_pool(name="pps", bufs=1, space="PSUM"))

        def wt(name, dt=BF16):
            a = host[name]
            t_ = wp.tile(list(a.shape), dt, tag=name)
            nx.sync.dma_start(t_[:], dram[name][:])
            return t_

        # DMA priority order: what phase R needs first
        w1hi = wt("w_w1hi"); w1lo = wt("w_w1lo")
        rw2 = wt("w_rw2")
        rw2lo = wt("w_rw2lo")
        have_rb1 = "w_rb1s" in host
        have_ab1 = "w_ab1" in host
        have_gb2 = "w_gb2" in host
        rb1s = wt("w_rb1s", F32) if have_rb1 else None
        rb2bc = wt("w_rb2bc", F32) if have_rb2 else None
        xhi = xp.tile([128, ECH, NT, T], BF16, tag="xhi")
        for i in range(2):
            for ec in range(ECH):
                nx.sync.dma_start(xhi[:, ec, i, :], dram["xhi"][i, ec])
        onesb = wp.tile([128, 1], BF16, tag="onesb")
        nx.vector.memset(onesb[:], 1.0)
        onesf = wp.tile([128, 1], F32, tag="onesf")
        nx.vector.memset(onesf[:], 1.0)

        logit_st = stash.tile([128, TCH, NT, NE], F32, tag="logit")
        wst = stash.tile([128, TCH, NT, NE], F32, tag="wst")
        ssum_st = stash.tile([128, TCH * NT], F32, tag="ssum_st")
        spl_st = stash.tile([128, TCH * NT], F32, tag="spl_st")
        pst = stash.tile([128, TCH, NT, O], BF16, tag="pst")
        mlin = stash.tile([1, NT, O], F32, tag="mlin")
        mfm = stash.tile([128, OCH, NT], BF16, tag="mfm")
        reprs = stash.tile([128, OCH, NT], BF16, tag="reprs")
        ccat = stash.tile([128, KCC, BC], BF16, tag="ccat")
        th_sb = stash.tile([S, BC], F32, tag="th")
        swb = stash.tile([128, BC * S], F32, tag="swb")
        sw_bounce = drp.tile([BC * S], F32, tag="swb_d")
        tr_bounce = drp.tile([S, BC], F32, tag="trb_d")
        mean_bounce = drp.tile([NT, O], F32, tag="meanb_d")

        # ---------------- Phase R: router L1 (Erf) + L2 (f32) --------------
        xlo_tiles = {}
        def fetch_xlo(i):
            t_ = hwork.tile([128, ECH, T], BF16, tag="xlo")
            for ec in range(ECH):
                nx.sync.dma_start(t_[:, ec, :], dram["xlo"][i, ec])
            xlo_tiles[i] = t_
        fetch_xlo(0)
        fetch_xlo(1)
        for i in range(2, NT):
            for ec in range(ECH):
                nx.sync.dma_start(xhi[:, ec, i, :], dram["xhi"][i, ec])
        for i in range(NT):
            if i + 2 < NT:
                fetch_xlo(i + 2)
            xlo_t = xlo_tiles.pop(i)
            h1 = hwork.tile([128, HCH, T], F32, tag="h1")
            for mcp in range(HCH // 2):
                ps = pp512.tile([128, 2, T], F32, tag="ps512")
                for mh in range(2):
                    mc = mcp * 2 + mh
                    terms = [(w1hi, xhi[:, ec, i, :]) for ec in range(ECH)]
                    terms += [(w1lo, xhi[:, ec, i, :]) for ec in range(ECH)]
                    terms += [(w1hi, xlo_t[:, ec, :]) for ec in range(ECH)]
                    n_terms = len(terms)
                    for j, (lhs, r) in enumerate(terms):
                        ec = j % ECH
                        nx.tensor.matmul(
                            ps[:, mh, :], lhs[:, ec, bass.ts(mc, 128)], r,
                            start=(j == 0), stop=(j == n_terms - 1))
                erf = hwork.tile([128, 2, T], F32, tag="erf")
                # erf((z + b1)/sqrt(2)); z in psum; rb1s = b1/sqrt(2).
                if have_rb1:
                    for mh in range(2):
                        nx.scalar.activation(
                            erf[:, mh, :], ps[:, mh, :], AF.Erf,
                            bias=rb1s[:, mcp * 2 + mh:mcp * 2 + mh + 1],
                            scale=RT2)
                else:
                    nx.scalar.activation(erf[:], ps[:], AF.Erf, scale=RT2)
                # h1 = (erf + 1) * z = 2*gelu(z)  (0.5 folded into rw2)
                nx.vector.scalar_tensor_tensor(
                    h1[:, mcp * 2:mcp * 2 + 2, :], erf[:], 1.0, ps[:],
                    op0=ALU.add, op1=ALU.mult)
            h1hi = hsp.tile([128, HCH, T], BF16, tag="h1hi")
            nx.vector.tensor_copy(h1hi[:], h1[:])
            h1lo = hsp.tile([128, HCH, T], BF16, tag="h1lo")
            nx.vector.tensor_tensor(h1lo[:], h1[:], h1hi[:], op=ALU.subtract)
            for tc_ in range(TCH):
                pl = pps.tile([128, NE], F32, tag="ppsmall")
                nmm = 3 * HCH
                j = 0
                for kc in range(HCH):
                    for lhs, r in ((h1hi, rw2), (h1lo, rw2), (h1hi, rw2lo)):
                        nx.tensor.matmul(
                            pl[:], lhs[:, kc, bass.ts(tc_, 128)], r[:, kc, :],
                            start=(j == 0), stop=(j == nmm - 1))
                        j += 1
                if have_rb2:
                    nx.vector.tensor_tensor(pl[:], pl[:], rb2bc[:, :NE],
                                            op=ALU.add)
                nx.vector.tensor_copy(logit_st[:, tc_, i, :], pl[:])

        # bulk weights: issued after R's x DMAs so they don't starve phase R
        aW1 = wt("w_aW1")
        ab1 = wt("w_ab1", F32) if have_ab1 else None
        gW2 = wt("w_gW2")
        gb2 = wt("w_gb2", F32) if have_gb2 else None
        aWo = wt("w_aWo")
        tew = wt("w_tew"); teb = wt("w_teb", F32)
        wcw = wt("w_wcw"); wcb = wt("w_wcb", F32)
        ow1 = wt("w_ow1"); ob1 = wt("w_ob1", F32)
        ow2 = wt("w_ow2")
        ident = wt("w_ident")
        boutbc = wt("w_boutbc", F32) if have_bout else None
        ob2bc = wt("w_ob2bc", F32) if have_ob2 else None

        tc.no_sync_barrier()
        # ---------------- Phase W: batched softmax + entropy + top-2 -------
        # All 48 chunks at once on [128, TCH, NT, *] views of the stashes.
        ssum3 = ssum_st[:].rearrange("p (a b) -> p a b", a=TCH)
        spl3 = spl_st[:].rearrange("p (a b) -> p a b", a=TCH)
        nx.scalar.activation(wst[:], logit_st[:], AF.Exp)
        nx.vector.reduce_sum(ssum3, wst[:], axis=AX.X)
        rcpa = sm.tile([128, TCH, NT, 1], F32, tag="rcpa")
        nx.vector.reciprocal(rcpa[:], ssum3)
        nx.vector.tensor_tensor(
            wst[:], wst[:], rcpa[:].broadcast_to([128, TCH, NT, NE]),
            op=ALU.mult)
        pl8a = sm.tile([128, TCH, NT, NE], F32, tag="pl8a")
        nx.vector.tensor_tensor(pl8a[:], wst[:], logit_st[:], op=ALU.mult)
        nx.vector.reduce_sum(spl3, pl8a[:], axis=AX.X)
        wl = wst[:, :, :, NS:NE]
        m1a = sm.tile([128, TCH, NT, 1], F32, tag="rcpa")
        nx.vector.reduce_max(m1a[:], wl, axis=AX.X)
        eqa = sm.tile([128, TCH, NT, NL], F32, tag="pl8a")
        nx.vector.tensor_tensor(
            eqa[:], wl, m1a[:].broadcast_to([128, TCH, NT, NL]),
            op=ALU.is_equal)
        w2a = sm.tile([128, TCH, NT, NL], F32, tag="w2a")
        nx.vector.scalar_tensor_tensor(
            w2a[:], eqa[:], -1e30, wl, op0=ALU.mult, op1=ALU.add)
        m2a = sm.tile([128, TCH, NT, 1], F32, tag="rcpa")
        nx.vector.reduce_max(m2a[:], w2a[:], axis=AX.X)
        sela = sm.tile([128, TCH, NT, NL], F32, tag="pl8a")
        nx.vector.tensor_tensor(
            sela[:], wl, m2a[:].broadcast_to([128, TCH, NT, NL]),
            op=ALU.is_ge)
        nx.vector.tensor_tensor(wl, wl, sela[:], op=ALU.mult)

        tc.no_sync_barrier()
        # ---------------- Phase E: experts (Gelu) + combine + means --------
        # Experts run in two half-groups of 4 so the eh buffer double-buffers
        # across tiles (cross-tile pipelining) at no extra SBUF cost.
        for i in range(NT):
            acq = acc.tile([128, TCH, O], F32, tag="acc4")
            for half in range(2):
                eh = ewp.tile([128, 4, HCH, T], BF16, tag="eh")
                for nn in range(4):
                    n = half * 4 + nn
                    for mcp in range(HCH // 2):
                        ps = pp512.tile([128, 2, T], F32, tag="ps512")
                        for mh in range(2):
                            mc = mcp * 2 + mh
                            for ec in range(ECH):
                                nx.tensor.matmul(
                                    ps[:, mh, :],
                                    aW1[:, ec, n, bass.ts(mc, 128)],
                                    xhi[:, ec, i, :],
                                    start=(ec == 0), stop=(ec == ECH - 1))
                        if have_ab1:
                            for mh in range(2):
                                nx.scalar.activation(
                                    eh[:, nn, mcp * 2 + mh, :], ps[:, mh, :],
                                    AF.Gelu,
                                    bias=ab1[:, n,
                                             mcp * 2 + mh:mcp * 2 + mh + 1])
                        else:
                            nx.scalar.activation(
                                eh[:, nn, mcp * 2:mcp * 2 + 2, :], ps[:],
                                AF.Gelu)
                if half == 0:
                    gh2 = ghp.tile([128, NS, HCH, T], BF16, tag="gh2")
                    for n in range(NS):
                        for mcp in range(HCH // 2):
                            ps = pp512.tile([128, 2, T], F32, tag="ps512")
                            for mh in range(2):
                                mc = mcp * 2 + mh
                                for kc in range(HCH):
                                    nx.tensor.matmul(
                                        ps[:, mh, :],
                                        gW2[:, kc, n, bass.ts(mc, 128)],
                                        eh[:, n, kc, :],
                                        start=(kc == 0), stop=(kc == HCH - 1))
                            if have_gb2:
                                for mh in range(2):
                                    nx.scalar.activation(
                                        gh2[:, n, mcp * 2 + mh, :],
                                        ps[:, mh, :], AF.Gelu,
                                        bias=gb2[:, n,
                                                 mcp * 2 + mh:mcp * 2 + mh + 1])
                            else:
                                nx.scalar.activation(
                                    gh2[:, n, mcp * 2:mcp * 2 + 2, :], ps[:],
                                    AF.Gelu)
                for tc_ in range(TCH):
                    for nn in range(4):
                        n = half * 4 + nn
                        pn = pp256.tile([128, O], F32, tag="pexp")
                        for kc in range(HCH):
                            if n < NS:
                                lhs = gh2[:, n, kc, bass.ts(tc_, 128)]
                            else:
                                lhs = eh[:, nn, kc, bass.ts(tc_, 128)]
                            nx.tensor.matmul(
                                pn[:], lhs, aWo[:, kc, n, :],
                                start=(kc == 0), stop=(kc == HCH - 1))
                        if have_bout:
                            nx.vector.tensor_tensor(
                                pn[:], pn[:], boutbc[:, bass.ts(n, O)],
                                op=ALU.add)
                        if n == 0:
                            nx.vector.tensor_scalar(
                                acq[:, tc_, :], pn[:], wst[:, tc_, i, 0:1],
                                None, op0=ALU.mult)
                        else:
                            nx.vector.scalar_tensor_tensor(
                                acq[:, tc_, :], pn[:], wst[:, tc_, i, n:n + 1],
                                acq[:, tc_, :], op0=ALU.mult, op1=ALU.add)
            nx.vector.tensor_copy(pst[:, :, i, :], acq[:])
            pm = pps.tile([1, O], F32, tag="ppsmall")
            for tc_ in range(TCH):
                nx.tensor.matmul(pm[:], onesb[:], pst[:, tc_, i, :],
                                 start=(tc_ == 0), stop=(tc_ == TCH - 1))
            nx.vector.tensor_copy(mlin[:, i, :], pm[:])
        # means -> feature-major [128, OCH, NT] via DRAM bounce
        nx.sync.dma_start(mean_bounce[:], mlin[0, :, :])
        mfm_f = acc.tile([128, OCH, NT], F32, tag="mfmf")
        for oc in range(OCH):
            nx.sync.dma_start(
                mfm_f[:, oc, :],
                mean_bounce[:, oc * 128:(oc + 1) * 128].transpose([1, 0]))
        nx.vector.tensor_copy(mfm[:], mfm_f[:])
        for mc in range(OCH):
            ps = pps.tile([128, NT], F32, tag="ppsmall")
            for kc in range(OCH):
                nx.tensor.matmul(ps[:], tew[:, kc, bass.ts(mc, 128)],
                                 mfm[:, kc, :],
                                 start=(kc == 0), stop=(kc == OCH - 1))
            nx.scalar.activation(reprs[:, mc, :], ps[:], AF.Gelu,
                                 bias=teb[:, mc:mc + 1])
        nx.vector.memset(ccat[:], 0.0)
        for s in range(S):
            for oc in range(OCH):
                nx.sync.dma_start(ccat[:, s * OCH + oc, :],
                                  reprs[:, oc, s * BC:(s + 1) * BC])
        nx.sync.dma_start(ccat[0:S, KCC - 1, :], dram["w_wmbc"][:])
        pw = pps.tile([S, BC], F32, tag="ppsmall")
        for kc in range(KCC):
            nx.tensor.matmul(pw[:], wcw[:, kc, :], ccat[:, kc, :],
                             start=(kc == 0), stop=(kc == KCC - 1))
        nx.scalar.activation(th_sb[:], pw[:], AF.Tanh, bias=wcb[0:S, 0:1])

        tc.no_sync_barrier()
        # ---------------- Phase G: softplus + scale softmax ---------------
        ee = sm.tile([S, BC], F32, tag="ee")
        nx.scalar.activation(ee[:], th_sb[:], AF.Exp)
        nx.vector.tensor_scalar(ee[:], ee[:], 1.0, None, op0=ALU.add)
        raw = sm.tile([S, BC], F32, tag="raw")
        nx.scalar.activation(raw[:], ee[:], AF.Ln)
        nx.sync.dma_start(tr_bounce[:], raw[:])
        rawt = sm.tile([BC, S], F32, tag="rawt")
        nx.sync.dma_start(rawt[:], tr_bounce[:].transpose([1, 0]))
        ex = sm.tile([BC, S], F32, tag="ex")
        nx.scalar.activation(ex[:], rawt[:], AF.Exp)
        ssum2 = sm.tile([BC, 1], F32, tag="ssum2")
        nx.vector.reduce_sum(ssum2[:], ex[:], axis=AX.X)
        rcp2 = sm.tile([BC, 1], F32, tag="rcp2")
        nx.vector.reciprocal(rcp2[:], ssum2[:])
        swt = sm.tile([BC, S], F32, tag="swt")
        nx.vector.tensor_scalar(swt[:], ex[:], rcp2[:], None, op0=ALU.mult)
        nx.sync.dma_start(sw_bounce[:], swt[:])
        nx.sync.dma_start(swb[:], sw_bounce[:].partition_broadcast(128))
        lns_all = sm.tile([128, TCH * NT], F32, tag="lns_all")
        nx.scalar.activation(lns_all[:], ssum_st[:], AF.Ln)
        ent_all = sm.tile([128, TCH * NT], F32, tag="ent_all")
        nx.vector.tensor_tensor(ent_all[:], lns_all[:], spl_st[:],
                                op=ALU.subtract)
        entv = sm.tile([128, 1], F32, tag="entv")
        nx.vector.reduce_sum(entv[:], ent_all[:], axis=AX.X)
        pe_ = pps.tile([1, 1], F32, tag="ppsmall")
        nx.tensor.matmul(pe_[:], onesf[:], entv[:], start=True, stop=True)
        esb = sm.tile([1, 1], F32, tag="esb")
        nx.vector.tensor_copy(esb[:], pe_[:])
        nx.sync.dma_start(ent_ext[:], esb[:])

        tc.no_sync_barrier()
        # ---------------- Phase OUT: scale-weighted sum + output MLP -------
        for b in range(BC):
            wacc = acc.tile([128, TCH, O], BF16, tag="wacc")
            for tc_ in range(TCH):
                tmp = sm.tile([128, O], F32, tag="wtmp")
                nx.vector.tensor_scalar(
                    tmp[:], pst[:, tc_, 0 * BC + b, :],
                    swb[:, b * S:b * S + 1], None, op0=ALU.mult)
                nx.vector.scalar_tensor_tensor(
                    tmp[:], pst[:, tc_, 1 * BC + b, :],
                    swb[:, b * S + 1:b * S + 2], tmp[:],
                    op0=ALU.mult, op1=ALU.add)
                nx.vector.scalar_tensor_tensor(
                    wacc[:, tc_, :], pst[:, tc_, 2 * BC + b, :],
                    swb[:, b * S + 2:b * S + 3], tmp[:],
                    op0=ALU.mult, op1=ALU.add)
            wfm = ghp.tile([128, OCH, T], BF16, tag="wfm")
            for tc_ in range(TCH):
                for oc in range(OCH):
                    pt = pp256.tile([128, 128], BF16, tag="pexp")
                    nx.tensor.transpose(pt[:], wacc[:, tc_, bass.ts(oc, 128)],
                                        ident[:])
                    nx.vector.tensor_copy(wfm[:, oc, bass.ts(tc_, 128)], pt[:])
            o1 = ghp.tile([128, OCH, T], BF16, tag="o1")
            for mc in range(OCH):
                ps = pp512.tile([128, T], F32, tag="ps512")
                for kc in range(OCH):
                    nx.tensor.matmul(ps[:], ow1[:, kc, bass.ts(mc, 128)],
                                     wfm[:, kc, :],
                                     start=(kc == 0), stop=(kc == OCH - 1))
                nx.scalar.activation(o1[:, mc, :], ps[:], AF.Gelu,
                                     bias=ob1[:, mc:mc + 1])
            for tc_ in range(TCH):
                pf = pp256.tile([128, O], F32, tag="pexp")
                for kc in range(OCH):
                    nx.tensor.matmul(pf[:], o1[:, kc, bass.ts(tc_, 128)],
                                     ow2[:, kc, :],
                                     start=(kc == 0), stop=(kc == OCH - 1))
                if have_ob2:
                    nx.vector.tensor_tensor(pf[:], pf[:], ob2bc[:], op=ALU.add)
                osb = sm.tile([128, O], F32, tag="osb")
                nx.vector.tensor_copy(osb[:], pf[:])
                nx.sync.dma_start(out_ext[b, bass.ts(tc_, 128), :], osb[:])

    nc.compile()
    return nc


def prep_weights(inp):
    h = {}
    w1 = np.asarray(inp["router_w1"], np.float32)
    w1hi = w1.astype(_bf).astype(np.float32)
    h["w_w1hi"] = _to_bf(_feat_major(w1hi))
    h["w_w1lo"] = _to_bf(_feat_major(w1 - w1hi))
    h["w_rw2"] = _to_bf(_feat_major(np.asarray(inp["router_w2"], np.float32) * 0.5))
    rw2f = _feat_major(np.asarray(inp["router_w2"], np.float32) * 0.5)
    h["w_rw2lo"] = _to_bf(rw2f - rw2f.astype(_bf).astype(np.float32))
    if np.any(np.asarray(inp["router_b1"]) != 0):
        h["w_rb1s"] = _bias_chunks(np.asarray(inp["router_b1"]) * RT2)
    aW1 = np.concatenate([inp["gW1"], inp["lW1"]], 0)
    h["w_aW1"] = _to_bf(np.stack([_feat_major(aW1[n]) for n in range(NE)], 2))
    ab1 = np.concatenate([inp["gb1"], inp["lb1"]], 0)
    if np.any(ab1 != 0):
        h["w_ab1"] = np.ascontiguousarray(
            np.stack([_bias_chunks(ab1[n]) for n in range(NE)], 1))
    h["w_gW2"] = _to_bf(np.stack(
        [_feat_major(np.asarray(inp["gW2"])[n]) for n in range(NS)], 2))
    if np.any(np.asarray(inp["gb2"]) != 0):
        h["w_gb2"] = np.ascontiguousarray(np.stack(
            [_bias_chunks(np.asarray(inp["gb2"])[n]) for n in range(NS)], 1))
    aWo = np.concatenate([inp["gW3"], inp["lW2"]], 0)
    h["w_aWo"] = _to_bf(np.stack([_feat_major(aWo[n]) for n in range(NE)], 2))
    h["w_tew"] = _to_bf(_feat_major(np.asarray(inp["te_w"], np.float32) / C))
    h["w_teb"] = _bias_chunks(inp["te_b"])
    wcw = np.zeros((896, S), np.float32)
    wcw[:768 + S] = np.asarray(inp["wc_w"], np.float32)
    h["w_wcw"] = _to_bf(_feat_major(wcw))
    wcb = np.zeros((128, 1), np.float32)
    wcb[:S, 0] = np.asarray(inp["wc_b"])
    h["w_wcb"] = wcb
    h["w_ow1"] = _to_bf(_feat_major(inp["out_w1"]))
    h["w_ob1"] = _bias_chunks(inp["out_b1"])
    h["w_ow2"] = _to_bf(_feat_major(inp["out_w2"]))
    h["w_ident"] = _to_bf(np.eye(128, dtype=np.float32))
    h["w_wmbc"] = _to_bf(np.broadcast_to(
        np.asarray(inp["weight_memory"], np.float32).reshape(S, 1), (S, BC)))
    if np.any(np.asarray(inp["router_b2"]) != 0):
        h["w_rb2bc"] = np.ascontiguousarray(np.broadcast_to(
            np.asarray(inp["router_b2"], np.float32), (128, NE)))
    bout = np.concatenate([inp["gb3"], inp["lb2"]], 0)
    if np.any(bout != 0):
        h["w_boutbc"] = np.ascontiguousarray(np.broadcast_to(
            np.asarray(bout, np.float32).reshape(1, NE * O), (128, NE * O)))
    if np.any(np.asarray(inp["out_b2"]) != 0):
        h["w_ob2bc"] = np.ascontiguousarray(np.broadcast_to(
            np.asarray(inp["out_b2"], np.float32), (128, O)))
    return h


def prep_x(xs):
    """xs [S,B,C,E] f32 -> per-core (xhi, xlo), each [NT, ECH, 128, T] bf16."""
    out = []
    for c in range(NCORES):
        blk = np.asarray(xs, np.float32)[:, c * BC:(c + 1) * BC]
        xt = blk.transpose(0, 1, 3, 2).reshape(NT, ECH, 128, T)
        hi = xt.astype(_bf)
        lo = (xt - hi.astype(np.float32)).astype(_bf)
        out.append((np.ascontiguousarray(hi), np.ascontiguousarray(lo)))
    return out


_CACHE = {}


def build_in_maps(inputs):
    host = prep_weights(inputs)
    key = "graph:" + ",".join(sorted(host))
    if key not in _CACHE:
        _CACHE[key] = build_graph(host)
    nc = _CACHE[key]
    xs_shards = prep_x(inputs["xs"])
    in_maps = []
    for c in range(NCORES):
        m = dict(host)
        m["xhi"], m["xlo"] = xs_shards[c]
        in_maps.append(m)
    return nc, in_maps


def assemble(results):
    outs = np.concatenate(
        [results[c]["out"][None] for c in range(NCORES)], 0).reshape(B, C, O)
    ent_sum = sum(float(results[c]["ent"][0, 0]) for c in range(NCORES))
    bl = np.float32(0.1 * ent_sum / (S * B * C))
    return outs, bl


def kernel(**inputs):
    nc, in_maps = build_in_maps(inputs)
    res = run_bass_kernel_spmd(nc, in_maps, core_ids=list(range(NCORES)))
    return assemble(res.results)
